# revision 1
# baseline (speedup 1.0000x reference)
"""Trainium2 Bass kernel for nn_LongformerEncoder (optimized v2).

Sharding: 8 cores = (batch b in 0..3, seq-half p in 0..1).
Stage A (longformer layer) runs on 1024 own tokens (+256-token halo).
A pairwise AllGather exchanges stage-A output; stage B (4-head/768-dim
MHA + max-pool) runs seq-split on queries with full keys, partial max
per core, final max across the pair on host.

v2 changes vs baseline:
- Host: LN_e + feature-major layout of embeddings; folded per-head
  matrices A_h = wq_h @ wk_h^T and B_h = wv_h @ fc_h (halves stage-B
  projection work); all weights pre-arranged for contiguous DMA.
- Stage B: k-major scores so mask+exp fuse into one scalar-engine
  activation (per-partition key bias, no max pass, no probs
  transposes); softmax sum via a ones-column in the PV matmul.
- Stage A: no-max softmax (scores are bounded); probs transpose and
  1/Z normalization fused by passing diag(1/Z) as the transpose
  "identity"; loops software-pipelined to keep the PE p-state high.
"""

import sys

sys.path.insert(0, "/opt/trn_rl_repo")

import numpy as np
import ml_dtypes

import concourse.bass as bass
import concourse.tile as tile
from concourse import bacc, mybir
from concourse.bass_utils import run_bass_kernel_spmd
from concourse.masks import make_identity

F32 = mybir.dt.float32
BF16 = mybir.dt.bfloat16
AX = mybir.AxisListType
ALU = mybir.AluOpType
ACTF = mybir.ActivationFunctionType

B, S, D = 4, 2048, 768
W = 256
DFF = 3072
NH, DK = 4, 768
T = 1024            # own tokens per core
EXT = 1536          # own + 256 halo each side
NEG = -1e9
EPS = 1e-5
NCORES = 8
C6 = D // 128        # 6 feature chunks
KC = DFF // 128      # 24 dff chunks
ISQ_DH = 0.125       # 1/sqrt(64)
ISQ_DK = 1.0 / float(np.sqrt(DK))


def build(debug=False):
    nc = bacc.Bacc("TRN2", target_bir_lowering=False, debug=False,
                   num_devices=NCORES)

    hxT_d = nc.dram_tensor("hxT", [128, C6 * EXT], BF16, kind="ExternalInput")
    m640_d = nc.dram_tensor("m640", [8, 128, 640], F32, kind="ExternalInput")
    kb16_d = nc.dram_tensor("kb16", [128, 16], F32, kind="ExternalInput")
    lfw_d = {}
    for nm in ["lfwq", "lfwk", "lfwv", "lfwo"]:
        lfw_d[nm] = nc.dram_tensor(nm, [128, C6 * D], BF16,
                                   kind="ExternalInput")
    w1_d = nc.dram_tensor("w1", [128, KC * D], BF16, kind="ExternalInput")
    w2_d = nc.dram_tensor("w2", [DFF, D], BF16, kind="ExternalInput")
    mwA_d = nc.dram_tensor("mwA", [128, NH * C6 * D], BF16,
                           kind="ExternalInput")
    mwB_d = nc.dram_tensor("mwB", [128, NH * C6 * D], BF16,
                           kind="ExternalInput")
    out_d = nc.dram_tensor("out", [128, 6], F32, kind="ExternalOutput")
    taps = {}
    if debug:
        taps["tap_olf"] = nc.dram_tensor("tap_olf", [T, D], F32,
                                         kind="ExternalOutput")
        taps["tap_attn"] = nc.dram_tensor("tap_attn", [T, D], F32,
                                          kind="ExternalOutput")

    with tile.TileContext(nc) as tc:
        _body(nc, tc, hxT_d, m640_d, kb16_d, lfw_d, w1_d, w2_d,
              mwA_d, mwB_d, out_d, taps)
    nc.compile()
    return nc


def _ln_tile(nc, pool, x_ap, out_tile, eps_ap):
    """out = (x - mean)/sqrt(var+eps) over free dim (768). g==1, b==0."""
    stats = pool.tile([128, 2, 6], F32, tag="lnstats")
    nc.vector.bn_stats(out=stats[:, 0, :], in_=x_ap[:, 0:384])
    nc.vector.bn_stats(out=stats[:, 1, :], in_=x_ap[:, 384:768])
    mv = pool.tile([128, 2], F32, tag="lnmv")
    nc.vector.bn_aggr(out=mv, in_=stats)
    rstd = pool.tile([128, 1], F32, tag="lnrstd")
    nc.scalar.activation(out=rstd, in_=mv[:, 1:2], func=ACTF.Sqrt, bias=eps_ap)
    nc.vector.reciprocal(out=rstd, in_=rstd)
    nc.vector.tensor_scalar(out=out_tile, in0=x_ap, scalar1=mv[:, 0:1],
                            scalar2=rstd, op0=ALU.subtract, op1=ALU.mult)


def _body(nc, tc, hxT_d, m640_d, kb16_d, lfw_d, w1_d, w2_d,
          mwA_d, mwB_d, out_d, taps):
    import contextlib
    ctx = contextlib.ExitStack()
    with ctx:
        constg = ctx.enter_context(tc.tile_pool(name="constg", bufs=1))
        outer = ctx.enter_context(tc.tile_pool(name="outer", bufs=1))
        dram = ctx.enter_context(tc.tile_pool(name="dram", bufs=1,
                                              space="DRAM"))

        id_bf = constg.tile([128, 128], BF16, tag="id_bf")
        make_identity(nc, id_bf)
        id_f32 = constg.tile([128, 128], F32, tag="id_f32")
        make_identity(nc, id_f32)
        eps_sb = constg.tile([128, 1], F32, tag="eps")
        nc.vector.memset(eps_sb, EPS)
        kb16 = constg.tile([128, 16], F32, tag="kb16")
        nc.sync.dma_start(kb16, kb16_d.ap())

        # cross-stage tiles
        olfT = outer.tile([128, C6, S], BF16, tag="olfT")        # 24K/part
        olftok = outer.tile([128, 16, 776], BF16, tag="olftok")  # 24.25K
        nc.vector.memset(olftok[:, :, 768:776], 0.0)
        nc.vector.memset(olftok[:, :, 768:769], 1.0)

        # DRAM bounce for the collective
        src_olf = dram.tile([T, D], BF16)
        dst_olf = dram.tile([2 * T, D], BF16)

        # ============ STAGE A ============
        with tc.tile_pool(name="mid", bufs=1) as mid, \
             tc.tile_pool(name="sm", bufs=4) as sm, \
             tc.tile_pool(name="work", bufs=2) as work:

            with tc.tile_pool(name="inA", bufs=1) as inA, \
                 tc.tile_pool(name="attA2", bufs=1) as attA2, \
                 tc.tile_pool(name="lfw", bufs=2) as lfw:

                hxT = inA.tile([128, C6, EXT], BF16, tag="hxT")
                nc.sync.dma_start(hxT, hxT_d.ap())
                aT = attA2.tile([128, C6, T], BF16, tag="aT")

                with tc.tile_pool(name="attA1", bufs=1) as attA1:
                    # ---- q/k feature-major, v token-major
                    ps1 = tc.tile_pool(name="ps1", bufs=2, space="PSUM")
                    psG = ps1.__enter__()
                    wq_sb = lfw.tile([128, C6, D], BF16, tag="lfw")
                    nc.sync.dma_start(wq_sb, lfw_d["lfwq"].ap())
                    qT = attA1.tile([128, C6, T], BF16, tag="qT")
                    for f in range(C6):
                        for nch in range(2):
                            ps = psG.tile([128, 512], F32, tag="g")
                            for k in range(C6):
                                nc.tensor.matmul(
                                    ps, wq_sb[:, k, f * 128:(f + 1) * 128],
                                    hxT[:, k, 256 + nch * 512:
                                        256 + (nch + 1) * 512],
                                    start=(k == 0), stop=(k == 5))
                            nc.any.tensor_copy(
                                out=qT[:, f, nch * 512:(nch + 1) * 512],
                                in_=ps)
                    wk_sb = lfw.tile([128, C6, D], BF16, tag="lfw")
                    nc.sync.dma_start(wk_sb, lfw_d["lfwk"].ap())
                    kT = attA1.tile([128, C6, EXT], BF16, tag="kT")
                    for f in range(C6):
                        for nch in range(3):
                            ps = psG.tile([128, 512], F32, tag="g")
                            for k in range(C6):
                                nc.tensor.matmul(
                                    ps, wk_sb[:, k, f * 128:(f + 1) * 128],
                                    hxT[:, k, nch * 512:(nch + 1) * 512],
                                    start=(k == 0), stop=(k == 5))
                            nc.any.tensor_copy(
                                out=kT[:, f, nch * 512:(nch + 1) * 512],
                                in_=ps)
                    wv_sb = lfw.tile([128, C6, D], BF16, tag="lfw")
                    nc.sync.dma_start(wv_sb, lfw_d["lfwv"].ap())
                    vtok = attA1.tile([128, 12, D], BF16, tag="vtok")
                    for t in range(12):
                        for (n0, nn) in ((0, 512), (512, 256)):
                            ps = psG.tile([128, 512], F32, tag="g")
                            for k in range(C6):
                                nc.tensor.matmul(
                                    ps[:, :nn],
                                    hxT[:, k, t * 128:(t + 1) * 128],
                                    wv_sb[:, k, n0:n0 + nn],
                                    start=(k == 0), stop=(k == 5))
                            nc.any.tensor_copy(out=vtok[:, t, n0:n0 + nn],
                                               in_=ps[:, :nn])
                    ps1.__exit__(None, None, None)

                    # ---- sliding-window attention, software-pipelined
                    ps2 = tc.tile_pool(name="ps2", bufs=2, space="PSUM")
                    psS = ps2.__enter__()
                    ps2b = tc.tile_pool(name="ps2b", bufs=1, space="PSUM")
                    psT = ps2b.__enter__()
                    ps2c = tc.tile_pool(name="ps2c", bufs=2, space="PSUM")
                    psV = ps2c.__enter__()

                    m640_t = [None] * 8

                    def a_scores(qt, pair, h2):
                        if pair == 0 and h2 == 0:
                            m640_t[qt] = work.tile([128, 640], F32,
                                                   tag="m640", name="m640")
                            nc.sync.dma_start(m640_t[qt], m640_d.ap()[qt])
                        ps = psS.tile([128, 640], F32, tag="sc")
                        lhs = qT[h2 * 64:(h2 + 1) * 64, pair,
                                 qt * 128:(qt + 1) * 128]
                        nc.tensor.matmul(
                            ps[:, 0:512], lhs,
                            kT[h2 * 64:(h2 + 1) * 64, pair,
                               qt * 128: qt * 128 + 512],
                            start=True, stop=True,
                            tile_position=(h2 * 64, 0))
                        nc.tensor.matmul(
                            ps[:, 512:640], lhs,
                            kT[h2 * 64:(h2 + 1) * 64, pair,
                               qt * 128 + 512: qt * 128 + 640],
                            start=True, stop=True,
                            tile_position=(h2 * 64, 0))
                        return ps

                    def a_rest(qt, pair, h2, ps):
                        h = 2 * pair + h2
                        sb = work.tile([128, 640], F32, tag="sb")
                        nc.vector.tensor_tensor(sb, ps, m640_t[qt], ALU.add)
                        probs = work.tile([128, 640], BF16, tag="probs")
                        sme = sm.tile([128, 1], F32, tag="sme")
                        nc.scalar.activation(out=probs, in_=sb, func=ACTF.Exp,
                                             scale=ISQ_DH, accum_out=sme)
                        rs = sm.tile([128, 1], F32, tag="rs")
                        nc.vector.reciprocal(rs, sme)
                        dg = work.tile([128, 128], BF16, tag="dg")
                        nc.vector.tensor_scalar_mul(dg, id_bf, rs)
                        # scaled transpose: REGULAR matmul probs^T @ diag(rs)
                        # (is_transpose ignores rhs values, so can't be used)
                        tp = psT.tile([128, 5, 128], F32, tag="tp")
                        for dx in range(5):
                            nc.tensor.matmul(
                                tp[:, dx, :],
                                probs[:, dx * 128:(dx + 1) * 128], dg,
                                start=True, stop=True)
                        pt_sb = work.tile([128, 5, 128], BF16, tag="ptsb")
                        nc.any.tensor_copy(out=pt_sb, in_=tp)
                        pvt = psV.tile([128, 128], F32, tag="pv")
                        for dx in range(5):
                            nc.tensor.matmul(
                                pvt[h2 * 64:(h2 + 1) * 64, :],
                                vtok[:, qt + dx, h * 64:(h + 1) * 64],
                                pt_sb[:, dx, :], start=(dx == 0),
                                stop=(dx == 4),
                                tile_position=(0, h2 * 64))
                        nc.any.tensor_copy(
                            out=aT[h2 * 64:(h2 + 1) * 64, pair,
                                   qt * 128:(qt + 1) * 128],
                            in_=pvt[h2 * 64:(h2 + 1) * 64, :])

                    its = [(qt, pair, h2) for qt in range(8)
                           for pair in range(6) for h2 in range(2)]
                    prev = None
                    for it in its:
                        ps = a_scores(*it)
                        if prev is not None:
                            a_rest(prev[0][0], prev[0][1], prev[0][2],
                                   prev[1])
                        prev = (it, ps)
                    a_rest(prev[0][0], prev[0][1], prev[0][2], prev[1])

                    ps2c.__exit__(None, None, None)
                    ps2b.__exit__(None, None, None)
                    ps2.__exit__(None, None, None)

                # ---- wo + residual (feature-major)
                ps3 = tc.tile_pool(name="ps3", bufs=2, space="PSUM")
                psG = ps3.__enter__()
                wo_sb = lfw.tile([128, C6, D], BF16, tag="lfw")
                nc.sync.dma_start(wo_sb, lfw_d["lfwo"].ap())
                r1T = mid.tile([128, C6, T], BF16, tag="resT")
                for f in range(C6):
                    for nch in range(2):
                        ps = psG.tile([128, 512], F32, tag="g")
                        for k in range(C6):
                            nc.tensor.matmul(
                                ps, wo_sb[:, k, f * 128:(f + 1) * 128],
                                aT[:, k, nch * 512:(nch + 1) * 512],
                                start=(k == 0), stop=(k == 5))
                        nc.vector.tensor_tensor(
                            r1T[:, f, nch * 512:(nch + 1) * 512], ps,
                            hxT[:, f, 256 + nch * 512: 256 + (nch + 1) * 512],
                            ALU.add)
                ps3.__exit__(None, None, None)

            # ---- LN1 (transpose to token-major, LN, transpose back)
            ps3b = tc.tile_pool(name="ps3b", bufs=2, space="PSUM")
            psT = ps3b.__enter__()
            h1T = mid.tile([128, C6, T], BF16, tag="h1T")
            for t in range(8):
                rtok = work.tile([128, D], BF16, tag="rtok")
                tp = psT.tile([128, C6, 128], BF16, tag="tp3")
                for c in range(C6):
                    nc.tensor.transpose(tp[:, c, :],
                                        r1T[:, c, t * 128:(t + 1) * 128],
                                        id_bf)
                nc.any.tensor_copy(out=rtok, in_=tp)
                ltok = work.tile([128, D], BF16, tag="ltok")
                _ln_tile(nc, sm, rtok, ltok, eps_sb)
                tp2 = psT.tile([128, C6, 128], BF16, tag="tp3")
                for c in range(C6):
                    nc.tensor.transpose(tp2[:, c, :],
                                        ltok[:, c * 128:(c + 1) * 128],
                                        id_bf)
                nc.any.tensor_copy(out=h1T[:, :, t * 128:(t + 1) * 128],
                                   in_=tp2)
            ps3b.__exit__(None, None, None)

            # ---- FFN (streamed over dff chunks) + residual
            ps4 = tc.tile_pool(name="ps4", bufs=2, space="PSUM")
            psG = ps4.__enter__()
            ps4b = tc.tile_pool(name="ps4b", bufs=1, space="PSUM")
            psF = ps4b.__enter__()
            with tc.tile_pool(name="bigw", bufs=3) as bigw:
                r2T = mid.tile([128, C6, T], BF16, tag="resT")
                for nch in range(2):
                    f2ps = [psF.tile([128, 512], F32, tag=f"f2_{m}",
                                     name=f"f2_{m}") for m in range(C6)]
                    for kc in range(KC):
                        w1c = bigw.tile([128, C6, 128], BF16, tag="w1c")
                        nc.sync.dma_start(
                            w1c, w1_d.ap()[:, kc * D:(kc + 1) * D])
                        w2c = bigw.tile([128, D], BF16, tag="w2c")
                        nc.sync.dma_start(
                            w2c, w2_d.ap()[kc * 128:(kc + 1) * 128, :])
                        g1p = psG.tile([128, 512], F32, tag="g")
                        for k in range(C6):
                            nc.tensor.matmul(
                                g1p, w1c[:, k, :],
                                h1T[:, k, nch * 512:(nch + 1) * 512],
                                start=(k == 0), stop=(k == 5))
                        g1c = work.tile([128, 512], BF16, tag="g1c")
                        nc.scalar.activation(out=g1c, in_=g1p,
                                             func=ACTF.Gelu_apprx_tanh)
                        for m in range(C6):
                            nc.tensor.matmul(
                                f2ps[m], w2c[:, m * 128:(m + 1) * 128],
                                g1c, start=(kc == 0), stop=(kc == KC - 1))
                    for m in range(C6):
                        nc.vector.tensor_tensor(
                            r2T[:, m, nch * 512:(nch + 1) * 512], f2ps[m],
                            h1T[:, m, nch * 512:(nch + 1) * 512], ALU.add)
            ps4b.__exit__(None, None, None)
            ps4.__exit__(None, None, None)

            # ---- LN2 -> src_olf
            ps5 = tc.tile_pool(name="ps5", bufs=2, space="PSUM")
            psT = ps5.__enter__()
            for t in range(8):
                rtok = work.tile([128, D], BF16, tag="rtok")
                tp = psT.tile([128, C6, 128], BF16, tag="tp5")
                for c in range(C6):
                    nc.tensor.transpose(tp[:, c, :],
                                        r2T[:, c, t * 128:(t + 1) * 128],
                                        id_bf)
                nc.any.tensor_copy(out=rtok, in_=tp)
                otok = work.tile([128, D], BF16, tag="ltok")
                _ln_tile(nc, sm, rtok, otok, eps_sb)
                nc.sync.dma_start(src_olf[t * 128:(t + 1) * 128, :], otok)
                if "tap_olf" in taps:
                    of = work.tile([128, D], F32, tag="tapolf")
                    nc.vector.tensor_copy(out=of, in_=otok)
                    nc.sync.dma_start(
                        taps["tap_olf"].ap()[t * 128:(t + 1) * 128, :], of)
            ps5.__exit__(None, None, None)

        # ---- pairwise exchange of stage-A output
        nc.gpsimd.collective_compute(
            "AllGather", ALU.bypass,
            replica_groups=[[0, 1], [2, 3], [4, 5], [6, 7]],
            ins=[src_olf[:].opt()], outs=[dst_olf[:].opt()])

        # ============ STAGE B ============
        with tc.tile_pool(name="resB", bufs=1) as resB, \
             tc.tile_pool(name="whead", bufs=2) as whead, \
             tc.tile_pool(name="hb", bufs=2) as hb, \
             tc.tile_pool(name="workB", bufs=2) as workB, \
             tc.tile_pool(name="smB", bufs=4) as smB:

            ps6 = tc.tile_pool(name="psG2", bufs=2, space="PSUM")
            psG2 = ps6.__enter__()
            ps8 = tc.tile_pool(name="psT2", bufs=2, space="PSUM")
            psT2 = ps8.__enter__()
            ps7 = tc.tile_pool(name="psP2", bufs=2, space="PSUM")
            psP2 = ps7.__enter__()

            # --- pre-collective work: own-half feature-major + q2(h0)
            ownT = resB.tile([128, C6, T], BF16, tag="ownT")
            for t in range(8):
                otokB = workB.tile([128, D], BF16, tag="otokB")
                nc.sync.dma_start(otokB, src_olf[t * 128:(t + 1) * 128, :])
                tp = psT2.tile([128, C6, 128], BF16, tag="tpB")
                for c in range(C6):
                    nc.tensor.transpose(tp[:, c, :],
                                        otokB[:, c * 128:(c + 1) * 128],
                                        id_bf)
                for c in range(C6):
                    nc.any.tensor_copy(
                        out=ownT[:, c, t * 128:(t + 1) * 128],
                        in_=tp[:, c, :])

            def q2_issue(h, dst):
                wh = whead.tile([128, C6, D], BF16, tag="wh")
                nc.sync.dma_start(
                    wh, mwA_d.ap()[:, h * C6 * D:(h + 1) * C6 * D])
                for f in range(C6):
                    for nch in range(2):
                        ps = psG2.tile([128, 512], F32, tag="g2")
                        for k in range(C6):
                            nc.tensor.matmul(
                                ps, wh[:, k, f * 128:(f + 1) * 128],
                                ownT[:, k, nch * 512:(nch + 1) * 512],
                                start=(k == 0), stop=(k == 5))
                        nc.any.tensor_copy(
                            out=dst[:, f, nch * 512:(nch + 1) * 512], in_=ps)

            q2_0 = hb.tile([128, C6, T], BF16, tag="q2T")
            q2_issue(0, q2_0)

            # --- post-collective: gathered sequence token-major + transposed
            for tt in range(16):
                nc.sync.dma_start(olftok[:, tt, 0:768],
                                  dst_olf[tt * 128:(tt + 1) * 128, :])
            for tt in range(16):
                tp = psT2.tile([128, C6, 128], BF16, tag="tpB")
                for c in range(C6):
                    nc.tensor.transpose(tp[:, c, :],
                                        olftok[:, tt, c * 128:(c + 1) * 128],
                                        id_bf)
                for c in range(C6):
                    nc.any.tensor_copy(
                        out=olfT[:, c, tt * 128:(tt + 1) * 128],
                        in_=tp[:, c, :])

            fcacc = resB.tile([128, C6, T], BF16, tag="fcacc")

            def sc_issue(h, qch, q2T):
                expT = hb.tile([128, 16, 512], BF16, tag="expT")
                for kt in range(16):
                    ps = psG2.tile([128, 512], F32, tag="g2")
                    for k in range(C6):
                        nc.tensor.matmul(
                            ps, olfT[:, k, kt * 128:(kt + 1) * 128],
                            q2T[:, k, qch * 512:(qch + 1) * 512],
                            start=(k == 0), stop=(k == 5))
                    nc.scalar.activation(out=expT[:, kt, :], in_=ps,
                                         func=ACTF.Exp,
                                         bias=kb16[:, kt:kt + 1],
                                         scale=ISQ_DK)
                return expT

            def pv_issue(qch, expT, poT_all):
                def pv_norm(qtl, pvp):
                    qg = qch * 4 + qtl
                    rs = smB.tile([128, 1], F32, tag="rs2")
                    nc.vector.reciprocal(rs, pvp[:, 768:769])
                    po = workB.tile([128, D], BF16, tag="po")
                    nc.vector.tensor_scalar_mul(po, pvp[:, 0:768], rs)
                    tp = psT2.tile([128, C6, 128], BF16, tag="tpB")
                    for c in range(C6):
                        nc.tensor.transpose(tp[:, c, :],
                                            po[:, c * 128:(c + 1) * 128],
                                            id_bf)
                    nc.any.tensor_copy(
                        out=poT_all[:, :, qg * 128:(qg + 1) * 128], in_=tp)

                prevq = None
                for qtl in range(4):
                    pvp = psP2.tile([128, 776], F32, tag="pv2")
                    for kt in range(16):
                        nc.tensor.matmul(
                            pvp[:, 0:512],
                            expT[:, kt, qtl * 128:(qtl + 1) * 128],
                            olftok[:, kt, 0:512],
                            start=(kt == 0), stop=(kt == 15))
                        nc.tensor.matmul(
                            pvp[:, 512:769],
                            expT[:, kt, qtl * 128:(qtl + 1) * 128],
                            olftok[:, kt, 512:769],
                            start=(kt == 0), stop=(kt == 15))
                    if prevq is not None:
                        pv_norm(*prevq)
                    prevq = (qtl, pvp)
                pv_norm(*prevq)

            def fc_issue(h, poT_all):
                wb = whead.tile([128, C6, D], BF16, tag="wh")
                nc.sync.dma_start(
                    wb, mwB_d.ap()[:, h * C6 * D:(h + 1) * C6 * D])
                for m in range(C6):
                    for nch in range(2):
                        ps = psG2.tile([128, 512], F32, tag="g2")
                        for k in range(C6):
                            nc.tensor.matmul(
                                ps, wb[:, k, m * 128:(m + 1) * 128],
                                poT_all[:, k, nch * 512:(nch + 1) * 512],
                                start=(k == 0), stop=(k == 5))
                        dst = fcacc[:, m, nch * 512:(nch + 1) * 512]
                        if h == 0:
                            nc.any.tensor_copy(out=dst, in_=ps)
                        else:
                            nc.vector.tensor_tensor(dst, dst, ps, ALU.add)

            # head pipeline: sc(h,qch) issued one step ahead of pv
            q2T_cur = q2_0
            prev = None
            poT = {}
            for h in range(NH):
                if h > 0:
                    q2T_cur = hb.tile([128, C6, T], BF16, tag="q2T")
                    q2_issue(h, q2T_cur)
                for qch in range(2):
                    expT = sc_issue(h, qch, q2T_cur)
                    if prev is not None:
                        ph, pqch, pexp = prev
                        if pqch == 0:
                            poT[ph] = hb.tile([128, C6, T], BF16,
                                              tag="poT", name="poT")
                        pv_issue(pqch, pexp, poT[ph])
                        if pqch == 1:
                            fc_issue(ph, poT[ph])
                    prev = (h, qch, expT)
            ph, pqch, pexp = prev
            pv_issue(pqch, pexp, poT[ph])
            fc_issue(ph, poT[ph])

            ps7.__exit__(None, None, None)
            ps9 = tc.tile_pool(name="psTail", bufs=2, space="PSUM")
            psTail = ps9.__enter__()

            # residual + LN + running max over own tokens
            maxacc = resB.tile([128, D], F32, tag="maxacc")
            for m in range(C6):
                nc.vector.tensor_tensor(fcacc[:, m, :], fcacc[:, m, :],
                                        ownT[:, m, :], ALU.add)
            for t in range(8):
                rtok = workB.tile([128, D], BF16, tag="rtokB")
                tp = psT2.tile([128, C6, 128], BF16, tag="tpB")
                for c in range(C6):
                    nc.tensor.transpose(tp[:, c, :],
                                        fcacc[:, c, t * 128:(t + 1) * 128],
                                        id_bf)
                nc.any.tensor_copy(out=rtok, in_=tp)
                ltok = workB.tile([128, D], F32, tag="ltokB")
                _ln_tile(nc, smB, rtok, ltok, eps_sb)
                if "tap_attn" in taps:
                    nc.sync.dma_start(
                        taps["tap_attn"].ap()[t * 128:(t + 1) * 128, :], ltok)
                if t == 0:
                    nc.vector.tensor_copy(out=maxacc, in_=ltok)
                else:
                    nc.vector.tensor_tensor(maxacc, maxacc, ltok, ALU.max)
            outsb = resB.tile([128, 6], F32, tag="outsb")
            for c in range(C6):
                pt = psTail.tile([128, 128], F32, tag="tpf")
                nc.tensor.transpose(pt, maxacc[:, c * 128:(c + 1) * 128],
                                    id_f32)
                nc.vector.tensor_reduce(out=outsb[:, c:c + 1], in_=pt,
                                        axis=AX.X, op=ALU.max)
            nc.sync.dma_start(out_d.ap(), outsb)
            ps9.__exit__(None, None, None)
            ps8.__exit__(None, None, None)
            ps6.__exit__(None, None, None)

    return


# ---------------- host side ----------------

_NC_CACHE = {}


def _get_nc(debug=False):
    key = bool(debug)
    if key not in _NC_CACHE:
        _NC_CACHE[key] = build(debug=debug)
    return _NC_CACHE[key]


def _prep_in_maps(inputs):
    bf = ml_dtypes.bfloat16
    x = np.asarray(inputs["x"])
    emb = np.asarray(inputs["emb"], np.float32)
    pos = np.asarray(inputs["pos"], np.float32)
    g_e = np.asarray(inputs["ln_e_g"], np.float32)
    b_e = np.asarray(inputs["ln_e_b"], np.float32)

    def parr(w):
        w = np.ascontiguousarray(
            np.asarray(w, np.float32).reshape(C6, 128, -1)
            .transpose(1, 0, 2)).astype(bf)
        return w.reshape(128, -1)

    wts = {
        "lfwq": parr(inputs["lf_wq"]), "lfwk": parr(inputs["lf_wk"]),
        "lfwv": parr(inputs["lf_wv"]), "lfwo": parr(inputs["lf_wo"]),
    }
    w1 = np.asarray(inputs["w1"], np.float32)
    wts["w1"] = np.ascontiguousarray(
        w1.reshape(C6, 128, KC, 128).transpose(1, 2, 0, 3)
    ).astype(bf).reshape(128, KC * D)
    wts["w2"] = np.asarray(inputs["w2"], np.float32).astype(bf)

    # folded stage-B matrices
    wq = np.asarray(inputs["mha_wq"], np.float32).reshape(D, NH, DK)
    wk = np.asarray(inputs["mha_wk"], np.float32).reshape(D, NH, DK)
    wv = np.asarray(inputs["mha_wv"], np.float32).reshape(D, NH, DK)
    fc = np.asarray(inputs["mha_fc"], np.float32).reshape(NH, DK, D)
    mwA = np.concatenate(
        [parr(wq[:, h, :] @ wk[:, h, :].T) for h in range(NH)], axis=1)
    mwB = np.concatenate(
        [parr(wv[:, h, :] @ fc[h]) for h in range(NH)], axis=1)
    wts["mwA"] = np.ascontiguousarray(mwA)
    wts["mwB"] = np.ascontiguousarray(mwB)

    in_maps = []
    for b in range(B):
        h0 = emb[x[b]] + pos                        # [S, D] f32
        mu = h0.mean(-1, keepdims=True)
        var = h0.var(-1, keepdims=True)
        hn = (h0 - mu) / np.sqrt(var + EPS) * g_e + b_e
        kbias = np.where(x[b] != 0, 0.0, NEG).astype(np.float32)
        kb16 = np.ascontiguousarray(kbias.reshape(16, 128).T)
        for p in range(2):
            start = p * T - 256
            hxe = np.zeros((EXT, D), np.float32)
            lo, hi = max(0, start), min(S, start + EXT)
            hxe[lo - start: hi - start] = hn[lo:hi]
            hxT = np.ascontiguousarray(
                hxe.reshape(EXT, C6, 128).transpose(2, 1, 0)
            ).astype(bf).reshape(128, C6 * EXT)

            qi = np.arange(128)
            kj = np.arange(640)
            m640 = np.zeros((8, 128, 640), np.float32)
            for qt in range(8):
                qg = p * T + qt * 128 + qi[:, None]
                kg = start + qt * 128 + kj[None, :]
                ok = (np.abs(kg - qg) <= W) & (kg >= 0) & (kg < S)
                m640[qt] = np.where(ok, 0.0, NEG)

            m = {"hxT": hxT, "m640": m640, "kb16": kb16}
            m.update(wts)
            in_maps.append(m)
    return in_maps


def _postprocess(results):
    out = np.zeros((B, D), np.float32)
    for b in range(B):
        m0 = np.asarray(results[2 * b]["out"]).T.reshape(D)
        m1 = np.asarray(results[2 * b + 1]["out"]).T.reshape(D)
        out[b] = np.maximum(m0, m1)
    return out


def run(inputs, debug=False, trace=False):
    nc = _get_nc(debug=debug)
    in_maps = _prep_in_maps(inputs)
    res = run_bass_kernel_spmd(nc, in_maps, core_ids=list(range(NCORES)),
                               trace=trace)
    return res


def kernel(**inputs):
    res = run(inputs, debug=False, trace=False)
    return _postprocess(res.results)



# revision 16
# speedup vs baseline: 1.2449x; 1.2449x over previous
"""Trainium2 Bass kernel for nn_LongformerEncoder (optimized v3).

Sharding: 8 cores = (batch b in 0..3, seq-half p in 0..1).
Stage A (longformer layer) runs on 1024 own tokens (+256-token halo).
A pairwise AllGather exchanges stage-A output; stage B (4-head/768-dim
MHA + max-pool) runs seq-split on queries with full keys, partial max
per core, final max across the pair on host.

v3 changes vs v2:
- Stage B entirely in fp8 (e4m3) with DoubleRow matmuls (2 contraction
  rows per PE pass): q2 projection, scores, PV, and the concatenated
  output projection. Attention contributes ~1.3% of the pre-LN signal,
  so fp8 error is negligible in the final output.
- The collective payload carries the stage-A output in fp8 in BOTH
  layouts (token-major for PV values, feature-major for score keys),
  eliminating all post-collective PE transposes in stage B.
- PV computed feature-major (lhsT = values chunk), so the attention
  output lands pre-transposed for the output projection; softmax
  denominator via a dedicated Z-column matmul, normalization via a
  ones-broadcast matmul + one DVE multiply per chunk.
- fc done once over the 4 heads' concatenated poT (single PSUM
  accumulation group; no inter-head DVE adds).
"""

import sys

sys.path.insert(0, "/opt/trn_rl_repo")

import numpy as np
import ml_dtypes

import concourse.bass as bass
import concourse.tile as tile
from concourse import bacc, mybir
from concourse.bass_utils import run_bass_kernel_spmd
from concourse.masks import make_identity

F32 = mybir.dt.float32
BF16 = mybir.dt.bfloat16
FP8 = mybir.dt.float8e4
AX = mybir.AxisListType
ALU = mybir.AluOpType
ACTF = mybir.ActivationFunctionType
DR = mybir.MatmulPerfMode.DoubleRow

B, S, D = 4, 2048, 768
W = 256
DFF = 3072
NH, DK = 4, 768
T = 1024            # own tokens per core
EXT = 1536          # own + 256 halo each side
NEG = -1e9
EPS = 1e-5
NCORES = 8
C6 = D // 128        # 6 feature chunks
KC = DFF // 128      # 24 dff chunks
ISQ_DH = 0.125       # 1/sqrt(64)
ISQ_DK = 1.0 / float(np.sqrt(DK))
SC_OLF = 16.0        # fp8 scale of stage-A output (both layouts)
SC_W = 256.0         # fp8 scale of folded stage-B weights
ZCOL = 1.0 / 16.0    # Z-helper column value so po lands at 256x true
DQ_Q2 = 1.0 / 256.0  # psum(16*256*q2) -> 16*q2
DQ_FC = 1.0 / 65536.0  # psum(256*256*fc) -> fc
SC_HX = 16.0         # fp8 scale of LN'd embeddings (stage-A input)
SC_LFW = 1024.0      # fp8 scale of longformer q/k/v/o weights
SC_AT = 32.0         # fp8 scale of stage-A attention output
DQ_QKV = 1.0 / (SC_HX * SC_LFW)
DQ_WO = 1.0 / (SC_AT * SC_LFW)


def build(debug=False):
    nc = bacc.Bacc("TRN2", target_bir_lowering=False, debug=False,
                   num_devices=NCORES)

    hxT_d = nc.dram_tensor("hxT", [128, C6 * EXT], BF16, kind="ExternalInput")
    hx8_d = nc.dram_tensor("hx8", [128, C6 * EXT], FP8, kind="ExternalInput")
    m640_d = nc.dram_tensor("m640", [8, 128, 640], F32, kind="ExternalInput")
    kb16_d = nc.dram_tensor("kb16", [128, 16], F32, kind="ExternalInput")
    lfw_d = {}
    for nm in ["lfwq", "lfwk", "lfwv", "lfwo"]:
        lfw_d[nm] = nc.dram_tensor(nm, [128, C6 * D], FP8,
                                   kind="ExternalInput")
    w1_d = nc.dram_tensor("w1", [128, KC * D], BF16, kind="ExternalInput")
    w2_d = nc.dram_tensor("w2", [DFF, D], BF16, kind="ExternalInput")
    mwA_d = nc.dram_tensor("mwA", [128, NH * C6 * D], FP8,
                           kind="ExternalInput")
    mwB_d = nc.dram_tensor("mwB", [128, NH * C6 * D], FP8,
                           kind="ExternalInput")
    out_d = nc.dram_tensor("out", [128, 6], F32, kind="ExternalOutput")
    taps = {}
    if debug:
        taps["tap_olf"] = nc.dram_tensor("tap_olf", [T, D], F32,
                                         kind="ExternalOutput")
        taps["tap_attn"] = nc.dram_tensor("tap_attn", [T, D], F32,
                                          kind="ExternalOutput")

    with tile.TileContext(nc) as tc:
        _body(nc, tc, hxT_d, hx8_d, m640_d, kb16_d, lfw_d, w1_d, w2_d,
              mwA_d, mwB_d, out_d, taps)
    nc.compile()
    return nc


def _ln_tile(nc, pool, x_ap, out_tile, eps_ap):
    """out = (x - mean)/sqrt(var+eps) over free dim (768). g==1, b==0."""
    stats = pool.tile([128, 2, 6], F32, tag="lnstats")
    nc.vector.bn_stats(out=stats[:, 0, :], in_=x_ap[:, 0:384])
    nc.vector.bn_stats(out=stats[:, 1, :], in_=x_ap[:, 384:768])
    mv = pool.tile([128, 2], F32, tag="lnmv")
    nc.vector.bn_aggr(out=mv, in_=stats)
    rstd = pool.tile([128, 1], F32, tag="lnrstd")
    nc.scalar.activation(out=rstd, in_=mv[:, 1:2], func=ACTF.Sqrt, bias=eps_ap)
    nc.vector.reciprocal(out=rstd, in_=rstd)
    nc.vector.tensor_scalar(out=out_tile, in0=x_ap, scalar1=mv[:, 0:1],
                            scalar2=rstd, op0=ALU.subtract, op1=ALU.mult)


def _body(nc, tc, hxT_d, hx8_d, m640_d, kb16_d, lfw_d, w1_d, w2_d,
          mwA_d, mwB_d, out_d, taps):
    import contextlib
    ctx = contextlib.ExitStack()
    with ctx:
        constg = ctx.enter_context(tc.tile_pool(name="constg", bufs=1))
        outer = ctx.enter_context(tc.tile_pool(name="outer", bufs=1))
        dram = ctx.enter_context(tc.tile_pool(name="dram", bufs=1,
                                              space="DRAM"))

        id_bf = constg.tile([128, 128], BF16, tag="id_bf")
        make_identity(nc, id_bf)
        id_f32 = constg.tile([128, 128], F32, tag="id_f32")
        make_identity(nc, id_f32)
        eps_sb = constg.tile([128, 1], F32, tag="eps")
        nc.vector.memset(eps_sb, EPS)
        kb16 = constg.tile([128, 16], F32, tag="kb16")
        nc.sync.dma_start(kb16, kb16_d.ap())
        ones_bf = constg.tile([1, 128], BF16, tag="ones_bf")
        nc.vector.memset(ones_bf, 1.0)

        # cross-stage tiles (stage-A output for stage B)
        ownT = outer.tile([128, C6, T], BF16, tag="ownT")    # 12K/part
        oT8 = outer.tile([128, C6, T], FP8, tag="oT8")       # 6K/part

        # DRAM bounce for the collectives (fp8, both layouts)
        srcA = dram.tile([T, D], FP8)
        dstA = dram.tile([2 * T, D], FP8)
        srcB = dram.tile([128, C6 * T], FP8)
        dstB = dram.tile([256, C6 * T], FP8)

        # ============ STAGE A ============
        with tc.tile_pool(name="mid", bufs=1) as mid, \
             tc.tile_pool(name="sm", bufs=4) as sm, \
             tc.tile_pool(name="work", bufs=2) as work:

            with tc.tile_pool(name="inA", bufs=1) as inA, \
                 tc.tile_pool(name="attA2", bufs=1) as attA2, \
                 tc.tile_pool(name="lfw", bufs=2) as lfw:

                hx8 = inA.tile([128, C6, EXT], FP8, tag="hx8")
                nc.sync.dma_start(hx8, hx8_d.ap())
                hxT = inA.tile([128, C6, EXT], BF16, tag="hxT")
                aT8 = attA2.tile([128, C6, T], FP8, tag="aT8")

                with tc.tile_pool(name="attA1", bufs=1) as attA1:
                    # ---- q/k feature-major, v token-major (fp8 DoubleRow)
                    ps1 = tc.tile_pool(name="ps1", bufs=2, space="PSUM")
                    psG = ps1.__enter__()
                    wq_sb = lfw.tile([128, C6, D], FP8, tag="lfw")
                    nc.sync.dma_start(wq_sb, lfw_d["lfwq"].ap())
                    qT = attA1.tile([128, C6, T], BF16, tag="qT")
                    for f in range(C6):
                        for nch in range(2):
                            ps = psG.tile([128, 512], F32, tag="g")
                            for j in range(3):
                                nc.tensor.matmul(
                                    ps,
                                    wq_sb[:, 2 * j:2 * j + 2,
                                          f * 128:(f + 1) * 128],
                                    hx8[:, 2 * j:2 * j + 2,
                                        256 + nch * 512:
                                        256 + (nch + 1) * 512],
                                    start=(j == 0), stop=(j == 2),
                                    perf_mode=DR)
                            nc.scalar.activation(
                                out=qT[:, f, nch * 512:(nch + 1) * 512],
                                in_=ps, func=ACTF.Copy, scale=DQ_QKV)
                    wk_sb = lfw.tile([128, C6, D], FP8, tag="lfw")
                    nc.sync.dma_start(wk_sb, lfw_d["lfwk"].ap())
                    # hxT (bf16 residual) only needed at wo; load now
                    nc.sync.dma_start(hxT, hxT_d.ap())
                    kT = attA1.tile([128, C6, EXT], BF16, tag="kT")
                    for f in range(C6):
                        for nch in range(3):
                            ps = psG.tile([128, 512], F32, tag="g")
                            for j in range(3):
                                nc.tensor.matmul(
                                    ps,
                                    wk_sb[:, 2 * j:2 * j + 2,
                                          f * 128:(f + 1) * 128],
                                    hx8[:, 2 * j:2 * j + 2,
                                        nch * 512:(nch + 1) * 512],
                                    start=(j == 0), stop=(j == 2),
                                    perf_mode=DR)
                            nc.scalar.activation(
                                out=kT[:, f, nch * 512:(nch + 1) * 512],
                                in_=ps, func=ACTF.Copy, scale=DQ_QKV)
                    wv_sb = lfw.tile([128, C6, D], FP8, tag="lfw")
                    nc.sync.dma_start(wv_sb, lfw_d["lfwv"].ap())
                    vtok = attA1.tile([128, 12, D], BF16, tag="vtok")
                    for t in range(12):
                        for (n0, nn) in ((0, 512), (512, 256)):
                            ps = psG.tile([128, 512], F32, tag="g")
                            for j in range(3):
                                nc.tensor.matmul(
                                    ps[:, :nn],
                                    hx8[:, 2 * j:2 * j + 2,
                                        t * 128:(t + 1) * 128],
                                    wv_sb[:, 2 * j:2 * j + 2, n0:n0 + nn],
                                    start=(j == 0), stop=(j == 2),
                                    perf_mode=DR)
                            nc.scalar.activation(
                                out=vtok[:, t, n0:n0 + nn], in_=ps[:, :nn],
                                func=ACTF.Copy, scale=DQ_QKV)
                    ps1.__exit__(None, None, None)

                    # ---- sliding-window attention, software-pipelined
                    ps2 = tc.tile_pool(name="ps2", bufs=2, space="PSUM")
                    psS = ps2.__enter__()
                    ps2b = tc.tile_pool(name="ps2b", bufs=1, space="PSUM")
                    psT = ps2b.__enter__()
                    ps2c = tc.tile_pool(name="ps2c", bufs=2, space="PSUM")
                    psV = ps2c.__enter__()

                    m640_t = [None] * 8

                    def a_scores(qt, pair, h2):
                        if pair == 0 and h2 == 0:
                            m640_t[qt] = work.tile([128, 640], F32,
                                                   tag="m640", name="m640")
                            nc.sync.dma_start(m640_t[qt], m640_d.ap()[qt])
                        ps = psS.tile([128, 640], F32, tag="sc")
                        lhs = qT[h2 * 64:(h2 + 1) * 64, pair,
                                 qt * 128:(qt + 1) * 128]
                        nc.tensor.matmul(
                            ps[:, 0:512], lhs,
                            kT[h2 * 64:(h2 + 1) * 64, pair,
                               qt * 128: qt * 128 + 512],
                            start=True, stop=True,
                            tile_position=(h2 * 64, 0))
                        nc.tensor.matmul(
                            ps[:, 512:640], lhs,
                            kT[h2 * 64:(h2 + 1) * 64, pair,
                               qt * 128 + 512: qt * 128 + 640],
                            start=True, stop=True,
                            tile_position=(h2 * 64, 0))
                        return ps

                    def a_rest(qt, pair, h2, ps):
                        h = 2 * pair + h2
                        sb = work.tile([128, 640], F32, tag="sb")
                        nc.vector.tensor_tensor(sb, ps, m640_t[qt], ALU.add)
                        probs = work.tile([128, 640], BF16, tag="probs")
                        sme = sm.tile([128, 1], F32, tag="sme")
                        nc.scalar.activation(out=probs, in_=sb, func=ACTF.Exp,
                                             scale=ISQ_DH, accum_out=sme)
                        rs = sm.tile([128, 1], F32, tag="rs")
                        nc.vector.reciprocal(rs, sme)
                        dg = work.tile([128, 128], BF16, tag="dg")
                        nc.vector.tensor_scalar_mul(dg, id_bf, rs)
                        # scaled transpose: REGULAR matmul probs^T @ diag(rs)
                        # (is_transpose ignores rhs values, so can't be used)
                        tp = psT.tile([128, 5, 128], F32, tag="tp")
                        for dx in range(5):
                            nc.tensor.matmul(
                                tp[:, dx, :],
                                probs[:, dx * 128:(dx + 1) * 128], dg,
                                start=True, stop=True)
                        pt_sb = work.tile([128, 5, 128], BF16, tag="ptsb")
                        nc.any.tensor_copy(out=pt_sb, in_=tp)
                        pvt = psV.tile([128, 128], F32, tag="pv")
                        for dx in range(5):
                            nc.tensor.matmul(
                                pvt[h2 * 64:(h2 + 1) * 64, :],
                                vtok[:, qt + dx, h * 64:(h + 1) * 64],
                                pt_sb[:, dx, :], start=(dx == 0),
                                stop=(dx == 4),
                                tile_position=(0, h2 * 64))
                        nc.scalar.activation(
                            out=aT8[h2 * 64:(h2 + 1) * 64, pair,
                                    qt * 128:(qt + 1) * 128],
                            in_=pvt[h2 * 64:(h2 + 1) * 64, :],
                            func=ACTF.Copy, scale=SC_AT)

                    its = [(qt, pair, h2) for qt in range(8)
                           for pair in range(6) for h2 in range(2)]
                    prev = None
                    for it in its:
                        ps = a_scores(*it)
                        if prev is not None:
                            a_rest(prev[0][0], prev[0][1], prev[0][2],
                                   prev[1])
                        prev = (it, ps)
                    a_rest(prev[0][0], prev[0][1], prev[0][2], prev[1])

                    ps2c.__exit__(None, None, None)
                    ps2b.__exit__(None, None, None)
                    ps2.__exit__(None, None, None)

                # ---- wo + residual (feature-major, fp8 DoubleRow)
                ps3 = tc.tile_pool(name="ps3", bufs=2, space="PSUM")
                psG = ps3.__enter__()
                wo_sb = lfw.tile([128, C6, D], FP8, tag="lfw")
                nc.sync.dma_start(wo_sb, lfw_d["lfwo"].ap())
                # w1 stays resident across both FFN passes; start its DMA
                # here so it loads behind the wo matmuls
                w1sb = mid.tile([128, KC, C6, 128], BF16, tag="w1sb")
                nc.sync.dma_start(w1sb, w1_d.ap())
                r1T = mid.tile([128, C6, T], BF16, tag="resT")
                for f in range(C6):
                    for nch in range(2):
                        ps = psG.tile([128, 512], F32, tag="g")
                        for j in range(3):
                            nc.tensor.matmul(
                                ps,
                                wo_sb[:, 2 * j:2 * j + 2,
                                      f * 128:(f + 1) * 128],
                                aT8[:, 2 * j:2 * j + 2,
                                    nch * 512:(nch + 1) * 512],
                                start=(j == 0), stop=(j == 2), perf_mode=DR)
                        t0 = work.tile([128, 512], BF16, tag="t0")
                        nc.scalar.activation(out=t0, in_=ps, func=ACTF.Copy,
                                             scale=DQ_WO)
                        nc.vector.tensor_tensor(
                            r1T[:, f, nch * 512:(nch + 1) * 512], t0,
                            hxT[:, f, 256 + nch * 512: 256 + (nch + 1) * 512],
                            ALU.add)
                ps3.__exit__(None, None, None)

            # ---- LN1 (transpose to token-major, LN, transpose back)
            ps3b = tc.tile_pool(name="ps3b", bufs=2, space="PSUM")
            psT = ps3b.__enter__()
            h1T = mid.tile([128, C6, T], BF16, tag="h1T")
            for t in range(8):
                rtok = work.tile([128, D], BF16, tag="rtok")
                tp = psT.tile([128, C6, 128], BF16, tag="tp3")
                for c in range(C6):
                    nc.tensor.transpose(tp[:, c, :],
                                        r1T[:, c, t * 128:(t + 1) * 128],
                                        id_bf)
                nc.any.tensor_copy(out=rtok, in_=tp)
                ltok = work.tile([128, D], BF16, tag="ltok")
                _ln_tile(nc, sm, rtok, ltok, eps_sb)
                tp2 = psT.tile([128, C6, 128], BF16, tag="tp3")
                for c in range(C6):
                    nc.tensor.transpose(tp2[:, c, :],
                                        ltok[:, c * 128:(c + 1) * 128],
                                        id_bf)
                nc.any.tensor_copy(out=h1T[:, :, t * 128:(t + 1) * 128],
                                   in_=tp2)
            ps3b.__exit__(None, None, None)

            # ---- FFN (streamed over dff chunks) + residual
            ps4 = tc.tile_pool(name="ps4", bufs=2, space="PSUM")
            psG = ps4.__enter__()
            ps4b = tc.tile_pool(name="ps4b", bufs=1, space="PSUM")
            psF = ps4b.__enter__()
            with tc.tile_pool(name="bigw", bufs=4) as bigw:
                r2T = mid.tile([128, C6, T], BF16, tag="resT")
                for nch in range(2):
                    f2ps = [psF.tile([128, 512], F32, tag=f"f2_{m}",
                                     name=f"f2_{m}") for m in range(C6)]
                    for kc in range(KC):
                        w2c = bigw.tile([128, D], BF16, tag="w2c")
                        nc.sync.dma_start(
                            w2c, w2_d.ap()[kc * 128:(kc + 1) * 128, :])
                        g1p = psG.tile([128, 512], F32, tag="g")
                        for k in range(C6):
                            nc.tensor.matmul(
                                g1p, w1sb[:, kc, k, :],
                                h1T[:, k, nch * 512:(nch + 1) * 512],
                                start=(k == 0), stop=(k == 5))
                        g1c = work.tile([128, 512], BF16, tag="g1c")
                        nc.scalar.activation(out=g1c, in_=g1p,
                                             func=ACTF.Gelu_apprx_tanh)
                        for m in range(C6):
                            nc.tensor.matmul(
                                f2ps[m], w2c[:, m * 128:(m + 1) * 128],
                                g1c, start=(kc == 0), stop=(kc == KC - 1))
                    for m in range(C6):
                        nc.vector.tensor_tensor(
                            r2T[:, m, nch * 512:(nch + 1) * 512], f2ps[m],
                            h1T[:, m, nch * 512:(nch + 1) * 512], ALU.add)
            ps4b.__exit__(None, None, None)
            ps4.__exit__(None, None, None)

            # ---- LN2 -> ownT (bf16) + oT8/otok8 (fp8 x16) + DMA payloads
            ps5 = tc.tile_pool(name="ps5", bufs=2, space="PSUM")
            psT = ps5.__enter__()
            for t in range(8):
                rtok = work.tile([128, D], BF16, tag="rtok")
                tp = psT.tile([128, C6, 128], BF16, tag="tp5")
                for c in range(C6):
                    nc.tensor.transpose(tp[:, c, :],
                                        r2T[:, c, t * 128:(t + 1) * 128],
                                        id_bf)
                nc.any.tensor_copy(out=rtok, in_=tp)
                otok = work.tile([128, D], BF16, tag="ltok")
                _ln_tile(nc, sm, rtok, otok, eps_sb)
                otok8 = work.tile([128, D], FP8, tag="otok8")
                nc.scalar.activation(out=otok8, in_=otok, func=ACTF.Copy,
                                     scale=SC_OLF)
                nc.sync.dma_start(srcA[t * 128:(t + 1) * 128, :], otok8)
                tp2 = psT.tile([128, C6, 128], BF16, tag="tp5")
                for c in range(C6):
                    nc.tensor.transpose(tp2[:, c, :],
                                        otok[:, c * 128:(c + 1) * 128],
                                        id_bf)
                nc.any.tensor_copy(out=ownT[:, :, t * 128:(t + 1) * 128],
                                   in_=tp2)
                nc.scalar.activation(
                    out=oT8[:, :, t * 128:(t + 1) * 128], in_=tp2,
                    func=ACTF.Copy, scale=SC_OLF)
                if "tap_olf" in taps:
                    of = work.tile([128, D], F32, tag="tapolf")
                    nc.vector.tensor_copy(out=of, in_=otok)
                    nc.sync.dma_start(
                        taps["tap_olf"].ap()[t * 128:(t + 1) * 128, :], of)
            nc.sync.dma_start(srcB, oT8)
            ps5.__exit__(None, None, None)

        # ---- pairwise exchange of stage-A output (fp8, both layouts)
        nc.gpsimd.collective_compute(
            "AllGather", ALU.bypass,
            replica_groups=[[0, 1], [2, 3], [4, 5], [6, 7]],
            ins=[srcA[:].opt()], outs=[dstA[:].opt()])
        nc.gpsimd.collective_compute(
            "AllGather", ALU.bypass,
            replica_groups=[[0, 1], [2, 3], [4, 5], [6, 7]],
            ins=[srcB[:].opt()], outs=[dstB[:].opt()])

        # ============ STAGE B ============
        with tc.tile_pool(name="resB", bufs=1) as resB, \
             tc.tile_pool(name="whead", bufs=2) as whead, \
             tc.tile_pool(name="hb", bufs=2) as hb, \
             tc.tile_pool(name="workB", bufs=2) as workB, \
             tc.tile_pool(name="smB", bufs=4) as smB:

            ps6 = tc.tile_pool(name="psG2", bufs=2, space="PSUM")
            psG2 = ps6.__enter__()

            # mwB needed only for fc at the end; start the DMA early
            mwB8_sb = resB.tile([128, NH * C6, D], FP8, tag="mwB8")
            nc.sync.dma_start(mwB8_sb, mwB_d.ap())

            # --- pre-collective: q2 for all 4 heads from local oT8
            q2T8 = []
            for h in range(NH):
                wh8 = whead.tile([128, C6, D], FP8, tag="wh")
                nc.sync.dma_start(
                    wh8, mwA_d.ap()[:, h * C6 * D:(h + 1) * C6 * D])
                q2 = resB.tile([128, C6, T], FP8, tag=f"q2T8_{h}")
                for f in range(C6):
                    for nch in range(2):
                        ps = psG2.tile([128, 512], F32, tag="g2")
                        for j in range(3):
                            nc.tensor.matmul(
                                ps,
                                wh8[:, 2 * j:2 * j + 2,
                                    f * 128:(f + 1) * 128],
                                oT8[:, 2 * j:2 * j + 2,
                                    nch * 512:(nch + 1) * 512],
                                start=(j == 0), stop=(j == 2), perf_mode=DR)
                        nc.scalar.activation(
                            out=q2[:, f, nch * 512:(nch + 1) * 512],
                            in_=ps, func=ACTF.Copy, scale=DQ_Q2)
                q2T8.append(q2)

            # --- land collective results (no transposes needed)
            olfT8 = resB.tile([128, C6, 2 * T], FP8, tag="olfT8")
            olftok8 = resB.tile([128, 16, 784], FP8, tag="olftok8")
            nc.vector.memset(olftok8[:, :, 768:769], ZCOL)
            for g in range(2):
                nc.sync.dma_start(olfT8[:, :, g * T:(g + 1) * T],
                                  dstB[g * 128:(g + 1) * 128, :])
            for tt in range(16):
                nc.sync.dma_start(olftok8[:, tt, 0:768],
                                  dstA[tt * 128:(tt + 1) * 128, :])

            poT8 = resB.tile([128, NH * C6, T], FP8, tag="poT8")
            fcacc = resB.tile([128, C6, T], BF16, tag="fcacc")

            ps7z = tc.tile_pool(name="psZ", bufs=2, space="PSUM")
            psZ = ps7z.__enter__()
            ps7b = tc.tile_pool(name="psBC", bufs=2, space="PSUM")
            psBC = ps7b.__enter__()
            ps7 = tc.tile_pool(name="psPV", bufs=2, space="PSUM")
            psPV = ps7.__enter__()

            def sc_issue(h, qch):
                expT8 = hb.tile([128, 16, 512], FP8, tag="expT8")
                for kt in range(16):
                    ps = psG2.tile([128, 512], F32, tag="g2")
                    for j in range(3):
                        nc.tensor.matmul(
                            ps,
                            olfT8[:, 2 * j:2 * j + 2,
                                  kt * 128:(kt + 1) * 128],
                            q2T8[h][:, 2 * j:2 * j + 2,
                                    qch * 512:(qch + 1) * 512],
                            start=(j == 0), stop=(j == 2), perf_mode=DR)
                    nc.scalar.activation(out=expT8[:, kt, :], in_=ps,
                                         func=ACTF.Exp,
                                         bias=kb16[:, kt:kt + 1],
                                         scale=ISQ_DK / 256.0)
                return expT8

            def pv_issue(h, qch, expT8):
                zp = psZ.tile([1, 512], F32, tag="z")
                for i in range(8):
                    nc.tensor.matmul(
                        zp, olftok8[:, 2 * i:2 * i + 2, 768:769],
                        expT8[:, 2 * i:2 * i + 2, :],
                        start=(i == 0), stop=(i == 7), perf_mode=DR)
                rs = smB.tile([1, 512], BF16, tag="rs2")
                with nc.allow_low_precision(
                        reason="1/Z feeds fp8-precision normalization"):
                    nc.vector.reciprocal(rs, zp)
                bc = psBC.tile([128, 512], F32, tag="bc")
                bc_sb = workB.tile([128, 512], BF16, tag="bc_sb")
                for c in range(C6):
                    pp = psPV.tile([128, 512], F32, tag="pv")
                    for i in range(8):
                        nc.tensor.matmul(
                            pp,
                            olftok8[:, 2 * i:2 * i + 2,
                                    c * 128:(c + 1) * 128],
                            expT8[:, 2 * i:2 * i + 2, :],
                            start=(i == 0), stop=(i == 7), perf_mode=DR)
                    if c == 0:
                        nc.tensor.matmul(bc, ones_bf, rs,
                                         start=True, stop=True)
                        nc.scalar.activation(out=bc_sb, in_=bc,
                                             func=ACTF.Copy)
                    nc.vector.tensor_tensor(
                        poT8[:, h * C6 + c, qch * 512:(qch + 1) * 512],
                        pp, bc_sb, ALU.mult)

            # head loop: sc issued one step ahead of pv
            prev = None
            for h in range(NH):
                for qch in range(2):
                    expT8 = sc_issue(h, qch)
                    if prev is not None:
                        pv_issue(*prev)
                    prev = (h, qch, expT8)
            pv_issue(*prev)

            # --- output projection over concatenated heads + residual
            for m in range(C6):
                for nch in range(2):
                    ps = psG2.tile([128, 512], F32, tag="g2")
                    for j in range(NH * C6 // 2):
                        nc.tensor.matmul(
                            ps,
                            mwB8_sb[:, 2 * j:2 * j + 2,
                                    m * 128:(m + 1) * 128],
                            poT8[:, 2 * j:2 * j + 2,
                                 nch * 512:(nch + 1) * 512],
                            start=(j == 0), stop=(j == NH * C6 // 2 - 1),
                            perf_mode=DR)
                    t1 = workB.tile([128, 512], BF16, tag="t1")
                    nc.scalar.activation(out=t1, in_=ps, func=ACTF.Copy,
                                         scale=DQ_FC)
                    nc.vector.tensor_tensor(
                        fcacc[:, m, nch * 512:(nch + 1) * 512], t1,
                        ownT[:, m, nch * 512:(nch + 1) * 512], ALU.add)

            ps7.__exit__(None, None, None)
            ps7b.__exit__(None, None, None)
            ps7z.__exit__(None, None, None)

            # --- residual + LN + running max over own tokens
            ps8 = tc.tile_pool(name="psT2", bufs=2, space="PSUM")
            psT2 = ps8.__enter__()
            ps9 = tc.tile_pool(name="psTail", bufs=2, space="PSUM")
            psTail = ps9.__enter__()
            maxacc = resB.tile([128, D], F32, tag="maxacc")
            for t in range(8):
                rtok = workB.tile([128, D], BF16, tag="rtokB")
                tp = psT2.tile([128, C6, 128], BF16, tag="tpB")
                for c in range(C6):
                    nc.tensor.transpose(tp[:, c, :],
                                        fcacc[:, c, t * 128:(t + 1) * 128],
                                        id_bf)
                nc.any.tensor_copy(out=rtok, in_=tp)
                ltok = workB.tile([128, D], F32, tag="ltokB")
                _ln_tile(nc, smB, rtok, ltok, eps_sb)
                if "tap_attn" in taps:
                    nc.sync.dma_start(
                        taps["tap_attn"].ap()[t * 128:(t + 1) * 128, :], ltok)
                if t == 0:
                    nc.vector.tensor_copy(out=maxacc, in_=ltok)
                else:
                    nc.vector.tensor_tensor(maxacc, maxacc, ltok, ALU.max)
            outsb = resB.tile([128, 6], F32, tag="outsb")
            for c in range(C6):
                pt = psTail.tile([128, 128], F32, tag="tpf")
                nc.tensor.transpose(pt, maxacc[:, c * 128:(c + 1) * 128],
                                    id_f32)
                nc.vector.tensor_reduce(out=outsb[:, c:c + 1], in_=pt,
                                        axis=AX.X, op=ALU.max)
            nc.sync.dma_start(out_d.ap(), outsb)
            ps9.__exit__(None, None, None)
            ps8.__exit__(None, None, None)
            ps6.__exit__(None, None, None)

    return


# ---------------- host side ----------------

_NC_CACHE = {}


def _get_nc(debug=False):
    key = bool(debug)
    if key not in _NC_CACHE:
        _NC_CACHE[key] = build(debug=debug)
    return _NC_CACHE[key]


def _prep_in_maps(inputs):
    bf = ml_dtypes.bfloat16
    f8 = ml_dtypes.float8_e4m3
    x = np.asarray(inputs["x"])
    emb = np.asarray(inputs["emb"], np.float32)
    pos = np.asarray(inputs["pos"], np.float32)
    g_e = np.asarray(inputs["ln_e_g"], np.float32)
    b_e = np.asarray(inputs["ln_e_b"], np.float32)

    def parr(w):
        w = np.ascontiguousarray(
            np.asarray(w, np.float32).reshape(C6, 128, -1)
            .transpose(1, 0, 2)).astype(bf)
        return w.reshape(128, -1)

    def parr8(w, scale):
        w = np.clip(np.asarray(w, np.float32) * scale, -240.0, 240.0)
        w = np.ascontiguousarray(
            w.reshape(-1, 128, w.shape[-1]).transpose(1, 0, 2)).astype(f8)
        return w.reshape(128, -1)

    wts = {
        "lfwq": parr8(inputs["lf_wq"], SC_LFW),
        "lfwk": parr8(inputs["lf_wk"], SC_LFW),
        "lfwv": parr8(inputs["lf_wv"], SC_LFW),
        "lfwo": parr8(inputs["lf_wo"], SC_LFW),
    }
    w1 = np.asarray(inputs["w1"], np.float32)
    wts["w1"] = np.ascontiguousarray(
        w1.reshape(C6, 128, KC, 128).transpose(1, 2, 0, 3)
    ).astype(bf).reshape(128, KC * D)
    wts["w2"] = np.asarray(inputs["w2"], np.float32).astype(bf)

    # folded stage-B matrices, fp8 x256
    wq = np.asarray(inputs["mha_wq"], np.float32).reshape(D, NH, DK)
    wk = np.asarray(inputs["mha_wk"], np.float32).reshape(D, NH, DK)
    wv = np.asarray(inputs["mha_wv"], np.float32).reshape(D, NH, DK)
    fc = np.asarray(inputs["mha_fc"], np.float32).reshape(NH, DK, D)
    mwA = np.concatenate(
        [parr8(wq[:, h, :] @ wk[:, h, :].T, SC_W) for h in range(NH)], axis=1)
    Bcat = np.concatenate([wv[:, h, :] @ fc[h] for h in range(NH)], axis=0)
    wts["mwA"] = np.ascontiguousarray(mwA)
    wts["mwB"] = np.ascontiguousarray(parr8(Bcat, SC_W))

    in_maps = []
    for b in range(B):
        h0 = emb[x[b]] + pos                        # [S, D] f32
        mu = h0.mean(-1, keepdims=True)
        var = h0.var(-1, keepdims=True)
        hn = (h0 - mu) / np.sqrt(var + EPS) * g_e + b_e
        kbias = np.where(x[b] != 0, 0.0, NEG).astype(np.float32)
        kb16 = np.ascontiguousarray(kbias.reshape(16, 128).T)
        for p in range(2):
            start = p * T - 256
            hxe = np.zeros((EXT, D), np.float32)
            lo, hi = max(0, start), min(S, start + EXT)
            hxe[lo - start: hi - start] = hn[lo:hi]
            hxf = np.ascontiguousarray(
                hxe.reshape(EXT, C6, 128).transpose(2, 1, 0))
            hxT = hxf.astype(bf).reshape(128, C6 * EXT)
            hx8 = np.clip(hxf * SC_HX, -240.0, 240.0).astype(f8).reshape(
                128, C6 * EXT)

            qi = np.arange(128)
            kj = np.arange(640)
            m640 = np.zeros((8, 128, 640), np.float32)
            for qt in range(8):
                qg = p * T + qt * 128 + qi[:, None]
                kg = start + qt * 128 + kj[None, :]
                ok = (np.abs(kg - qg) <= W) & (kg >= 0) & (kg < S)
                m640[qt] = np.where(ok, 0.0, NEG)

            m = {"hxT": hxT, "hx8": hx8, "m640": m640, "kb16": kb16}
            m.update(wts)
            in_maps.append(m)
    return in_maps


def _postprocess(results):
    out = np.zeros((B, D), np.float32)
    for b in range(B):
        m0 = np.asarray(results[2 * b]["out"]).T.reshape(D)
        m1 = np.asarray(results[2 * b + 1]["out"]).T.reshape(D)
        out[b] = np.maximum(m0, m1)
    return out


def run(inputs, debug=False, trace=False):
    nc = _get_nc(debug=debug)
    in_maps = _prep_in_maps(inputs)
    res = run_bass_kernel_spmd(nc, in_maps, core_ids=list(range(NCORES)),
                               trace=trace)
    return res


def kernel(**inputs):
    res = run(inputs, debug=False, trace=False)
    return _postprocess(res.results)


# revision 28
# speedup vs baseline: 1.3577x; 1.0906x over previous
"""Trainium2 Bass kernel for nn_LongformerEncoder (optimized v3).

Sharding: 8 cores = (batch b in 0..3, seq-half p in 0..1).
Stage A (longformer layer) runs on 1024 own tokens (+256-token halo).
A pairwise AllGather exchanges stage-A output; stage B (4-head/768-dim
MHA + max-pool) runs seq-split on queries with full keys, partial max
per core, final max across the pair on host.

v3 changes vs v2:
- Stage B entirely in fp8 (e4m3) with DoubleRow matmuls (2 contraction
  rows per PE pass): q2 projection, scores, PV, and the concatenated
  output projection. Attention contributes ~1.3% of the pre-LN signal,
  so fp8 error is negligible in the final output.
- The collective payload carries the stage-A output in fp8 in BOTH
  layouts (token-major for PV values, feature-major for score keys),
  eliminating all post-collective PE transposes in stage B.
- PV computed feature-major (lhsT = values chunk), so the attention
  output lands pre-transposed for the output projection; softmax
  denominator via a dedicated Z-column matmul, normalization via a
  ones-broadcast matmul + one DVE multiply per chunk.
- fc done once over the 4 heads' concatenated poT (single PSUM
  accumulation group; no inter-head DVE adds).
"""

import sys

sys.path.insert(0, "/opt/trn_rl_repo")

import numpy as np
import ml_dtypes

import concourse.bass as bass
import concourse.tile as tile
from concourse import bacc, mybir
from concourse.bass_utils import run_bass_kernel_spmd
from concourse.masks import make_identity

F32 = mybir.dt.float32
BF16 = mybir.dt.bfloat16
FP8 = mybir.dt.float8e4
AX = mybir.AxisListType
ALU = mybir.AluOpType
ACTF = mybir.ActivationFunctionType
DR = mybir.MatmulPerfMode.DoubleRow

B, S, D = 4, 2048, 768
W = 256
DFF = 3072
NH, DK = 4, 768
T = 1024            # own tokens per core
EXT = 1536          # own + 256 halo each side
NEG = -1e9
EPS = 1e-5
NCORES = 8
C6 = D // 128        # 6 feature chunks
KC = DFF // 128      # 24 dff chunks
ISQ_DH = 0.125       # 1/sqrt(64)
ISQ_DK = 1.0 / float(np.sqrt(DK))
SC_OLF = 16.0        # fp8 scale of stage-A output (both layouts)
SC_W = 256.0         # fp8 scale of folded stage-B weights
ZCOL = 1.0 / 16.0    # Z-helper column value so po lands at 256x true
DQ_Q2 = 1.0 / 256.0  # psum(16*256*q2) -> 16*q2
DQ_FC = 1.0 / 65536.0  # psum(256*256*fc) -> fc
SC_HX = 16.0         # fp8 scale of LN'd embeddings (stage-A input)
SC_LFW = 1024.0      # fp8 scale of longformer q/k/v/o weights
SC_AT = 32.0         # fp8 scale of stage-A attention output
DQ_QKV = 1.0 / (SC_HX * SC_LFW)
DQ_WO = 1.0 / (SC_AT * SC_LFW)


def build(debug=False):
    nc = bacc.Bacc("TRN2", target_bir_lowering=False, debug=False,
                   num_devices=NCORES)

    hxT_d = nc.dram_tensor("hxT", [128, C6 * EXT], BF16, kind="ExternalInput")
    hx8_d = nc.dram_tensor("hx8", [128, C6 * EXT], FP8, kind="ExternalInput")
    m640_d = nc.dram_tensor("m640", [8, 128, 640], F32, kind="ExternalInput")
    kb16_d = nc.dram_tensor("kb16", [128, 16], F32, kind="ExternalInput")
    lfw_d = {}
    for nm in ["lfwq", "lfwk", "lfwv", "lfwo"]:
        lfw_d[nm] = nc.dram_tensor(nm, [128, C6 * D], FP8,
                                   kind="ExternalInput")
    w1_d = nc.dram_tensor("w1", [128, KC * D], BF16, kind="ExternalInput")
    w2_d = nc.dram_tensor("w2", [DFF, D], BF16, kind="ExternalInput")
    mwA_d = nc.dram_tensor("mwA", [128, NH * C6 * D], FP8,
                           kind="ExternalInput")
    mwB_d = nc.dram_tensor("mwB", [128, NH * C6 * D], FP8,
                           kind="ExternalInput")
    out_d = nc.dram_tensor("out", [128, 6], F32, kind="ExternalOutput")
    taps = {}
    if debug:
        taps["tap_olf"] = nc.dram_tensor("tap_olf", [T, D], F32,
                                         kind="ExternalOutput")
        taps["tap_attn"] = nc.dram_tensor("tap_attn", [T, D], F32,
                                          kind="ExternalOutput")

    with tile.TileContext(nc) as tc:
        _body(nc, tc, hxT_d, hx8_d, m640_d, kb16_d, lfw_d, w1_d, w2_d,
              mwA_d, mwB_d, out_d, taps)
    nc.compile()
    return nc


def _ln_tile(nc, pool, x_ap, out_tile, eps_ap):
    """out = (x - mean)/sqrt(var+eps) over free dim (768). g==1, b==0."""
    stats = pool.tile([128, 2, 6], F32, tag="lnstats")
    nc.vector.bn_stats(out=stats[:, 0, :], in_=x_ap[:, 0:384])
    nc.vector.bn_stats(out=stats[:, 1, :], in_=x_ap[:, 384:768])
    mv = pool.tile([128, 2], F32, tag="lnmv")
    nc.vector.bn_aggr(out=mv, in_=stats)
    rstd = pool.tile([128, 1], F32, tag="lnrstd")
    nc.scalar.activation(out=rstd, in_=mv[:, 1:2], func=ACTF.Sqrt, bias=eps_ap)
    nc.vector.reciprocal(out=rstd, in_=rstd)
    nc.vector.tensor_scalar(out=out_tile, in0=x_ap, scalar1=mv[:, 0:1],
                            scalar2=rstd, op0=ALU.subtract, op1=ALU.mult)


def _body(nc, tc, hxT_d, hx8_d, m640_d, kb16_d, lfw_d, w1_d, w2_d,
          mwA_d, mwB_d, out_d, taps):
    import contextlib
    ctx = contextlib.ExitStack()
    with ctx:
        constg = ctx.enter_context(tc.tile_pool(name="constg", bufs=1))
        outer = ctx.enter_context(tc.tile_pool(name="outer", bufs=1))
        dram = ctx.enter_context(tc.tile_pool(name="dram", bufs=1,
                                              space="DRAM"))

        id_bf = constg.tile([128, 128], BF16, tag="id_bf")
        make_identity(nc, id_bf)
        id_f32 = constg.tile([128, 128], F32, tag="id_f32")
        make_identity(nc, id_f32)
        eps_sb = constg.tile([128, 1], F32, tag="eps")
        nc.vector.memset(eps_sb, EPS)
        kb16 = constg.tile([128, 16], F32, tag="kb16")
        nc.sync.dma_start(kb16, kb16_d.ap())
        ones_bf = constg.tile([1, 128], BF16, tag="ones_bf")
        nc.vector.memset(ones_bf, 1.0)

        # cross-stage tiles (stage-A output for stage B)
        ownT = outer.tile([128, C6, T], BF16, tag="ownT")    # 12K/part
        oT8 = outer.tile([128, C6, T], FP8, tag="oT8")       # 6K/part

        # DRAM bounce for the collectives (fp8, both layouts, split in
        # two token-halves so the first exchange overlaps the second
        # half's FFN)
        srcA = [dram.tile([T // 2, D], FP8, name=f"srcA{i}")
                for i in range(2)]
        dstA = [dram.tile([T, D], FP8, name=f"dstA{i}") for i in range(2)]
        srcB = [dram.tile([128, C6 * 512], FP8, name=f"srcB{i}")
                for i in range(2)]
        dstB = [dram.tile([256, C6 * 512], FP8, name=f"dstB{i}")
                for i in range(2)]

        # ============ STAGE A ============
        with tc.tile_pool(name="mid", bufs=1) as mid, \
             tc.tile_pool(name="sm", bufs=4) as sm, \
             tc.tile_pool(name="work", bufs=2) as work:

            with tc.tile_pool(name="inA", bufs=1) as inA, \
                 tc.tile_pool(name="attA2", bufs=1) as attA2, \
                 tc.tile_pool(name="lfw", bufs=2) as lfw:

                hx8 = inA.tile([128, C6, EXT], FP8, tag="hx8")
                nc.sync.dma_start(hx8, hx8_d.ap())
                hxT = inA.tile([128, C6, EXT], BF16, tag="hxT")
                aT8 = attA2.tile([128, C6, T], FP8, tag="aT8")

                with tc.tile_pool(name="attA1", bufs=1) as attA1:
                    # ---- q/k feature-major, v token-major (fp8 DoubleRow)
                    ps1 = tc.tile_pool(name="ps1", bufs=2, space="PSUM")
                    psG = ps1.__enter__()
                    wq_sb = lfw.tile([128, C6, D], FP8, tag="lfw")
                    nc.sync.dma_start(wq_sb, lfw_d["lfwq"].ap())
                    qT = attA1.tile([128, C6, T], BF16, tag="qT")
                    for f in range(C6):
                        for nch in range(2):
                            ps = psG.tile([128, 512], F32, tag="g")
                            for j in range(3):
                                nc.tensor.matmul(
                                    ps,
                                    wq_sb[:, 2 * j:2 * j + 2,
                                          f * 128:(f + 1) * 128],
                                    hx8[:, 2 * j:2 * j + 2,
                                        256 + nch * 512:
                                        256 + (nch + 1) * 512],
                                    start=(j == 0), stop=(j == 2),
                                    perf_mode=DR)
                            nc.scalar.activation(
                                out=qT[:, f, nch * 512:(nch + 1) * 512],
                                in_=ps, func=ACTF.Copy, scale=DQ_QKV)
                    wk_sb = lfw.tile([128, C6, D], FP8, tag="lfw")
                    nc.sync.dma_start(wk_sb, lfw_d["lfwk"].ap())
                    # hxT (bf16 residual) only needed at wo; load now
                    nc.sync.dma_start(hxT, hxT_d.ap())
                    kT = attA1.tile([128, C6, EXT], BF16, tag="kT")
                    for f in range(C6):
                        for nch in range(3):
                            ps = psG.tile([128, 512], F32, tag="g")
                            for j in range(3):
                                nc.tensor.matmul(
                                    ps,
                                    wk_sb[:, 2 * j:2 * j + 2,
                                          f * 128:(f + 1) * 128],
                                    hx8[:, 2 * j:2 * j + 2,
                                        nch * 512:(nch + 1) * 512],
                                    start=(j == 0), stop=(j == 2),
                                    perf_mode=DR)
                            nc.scalar.activation(
                                out=kT[:, f, nch * 512:(nch + 1) * 512],
                                in_=ps, func=ACTF.Copy, scale=DQ_QKV)
                    wv_sb = lfw.tile([128, C6, D], FP8, tag="lfw")
                    nc.sync.dma_start(wv_sb, lfw_d["lfwv"].ap())
                    vtok = attA1.tile([128, 12, D], BF16, tag="vtok")
                    for t in range(12):
                        for (n0, nn) in ((0, 512), (512, 256)):
                            ps = psG.tile([128, 512], F32, tag="g")
                            for j in range(3):
                                nc.tensor.matmul(
                                    ps[:, :nn],
                                    hx8[:, 2 * j:2 * j + 2,
                                        t * 128:(t + 1) * 128],
                                    wv_sb[:, 2 * j:2 * j + 2, n0:n0 + nn],
                                    start=(j == 0), stop=(j == 2),
                                    perf_mode=DR)
                            nc.scalar.activation(
                                out=vtok[:, t, n0:n0 + nn], in_=ps[:, :nn],
                                func=ACTF.Copy, scale=DQ_QKV)
                    ps1.__exit__(None, None, None)

                    # ---- sliding-window attention, software-pipelined
                    ps2 = tc.tile_pool(name="ps2", bufs=2, space="PSUM")
                    psS = ps2.__enter__()
                    ps2b = tc.tile_pool(name="ps2b", bufs=1, space="PSUM")
                    psT = ps2b.__enter__()
                    ps2c = tc.tile_pool(name="ps2c", bufs=2, space="PSUM")
                    psV = ps2c.__enter__()

                    m640_t = [None] * 8

                    def a_scores(qt, pair, h2):
                        if pair == 0 and h2 == 0:
                            m640_t[qt] = work.tile([128, 640], F32,
                                                   tag="m640", name="m640")
                            nc.sync.dma_start(m640_t[qt], m640_d.ap()[qt])
                        ps = psS.tile([128, 640], F32, tag="sc")
                        lhs = qT[h2 * 64:(h2 + 1) * 64, pair,
                                 qt * 128:(qt + 1) * 128]
                        nc.tensor.matmul(
                            ps[:, 0:512], lhs,
                            kT[h2 * 64:(h2 + 1) * 64, pair,
                               qt * 128: qt * 128 + 512],
                            start=True, stop=True,
                            tile_position=(h2 * 64, 0))
                        nc.tensor.matmul(
                            ps[:, 512:640], lhs,
                            kT[h2 * 64:(h2 + 1) * 64, pair,
                               qt * 128 + 512: qt * 128 + 640],
                            start=True, stop=True,
                            tile_position=(h2 * 64, 0))
                        return ps

                    def a_rest(qt, pair, h2, ps):
                        h = 2 * pair + h2
                        sb = work.tile([128, 640], F32, tag="sb")
                        nc.vector.tensor_tensor(sb, ps, m640_t[qt], ALU.add)
                        probs = work.tile([128, 640], BF16, tag="probs")
                        sme = sm.tile([128, 1], F32, tag="sme")
                        nc.scalar.activation(out=probs, in_=sb, func=ACTF.Exp,
                                             scale=ISQ_DH, accum_out=sme)
                        rs = sm.tile([128, 1], F32, tag="rs")
                        nc.vector.reciprocal(rs, sme)
                        dg = work.tile([128, 128], BF16, tag="dg")
                        nc.vector.tensor_scalar_mul(dg, id_bf, rs)
                        # scaled transpose: REGULAR matmul probs^T @ diag(rs)
                        # (is_transpose ignores rhs values, so can't be used)
                        tp = psT.tile([128, 5, 128], F32, tag="tp")
                        for dx in range(5):
                            nc.tensor.matmul(
                                tp[:, dx, :],
                                probs[:, dx * 128:(dx + 1) * 128], dg,
                                start=True, stop=True)
                        pt_sb = work.tile([128, 5, 128], BF16, tag="ptsb")
                        nc.any.tensor_copy(out=pt_sb, in_=tp)
                        pvt = psV.tile([128, 128], F32, tag="pv")
                        for dx in range(5):
                            nc.tensor.matmul(
                                pvt[h2 * 64:(h2 + 1) * 64, :],
                                vtok[:, qt + dx, h * 64:(h + 1) * 64],
                                pt_sb[:, dx, :], start=(dx == 0),
                                stop=(dx == 4),
                                tile_position=(0, h2 * 64))
                        nc.scalar.activation(
                            out=aT8[h2 * 64:(h2 + 1) * 64, pair,
                                    qt * 128:(qt + 1) * 128],
                            in_=pvt[h2 * 64:(h2 + 1) * 64, :],
                            func=ACTF.Copy, scale=SC_AT)

                    its = [(qt, pair, h2) for qt in range(8)
                           for pair in range(6) for h2 in range(2)]
                    prev = None
                    for it in its:
                        ps = a_scores(*it)
                        if prev is not None:
                            a_rest(prev[0][0], prev[0][1], prev[0][2],
                                   prev[1])
                        prev = (it, ps)
                    a_rest(prev[0][0], prev[0][1], prev[0][2], prev[1])

                    ps2c.__exit__(None, None, None)
                    ps2b.__exit__(None, None, None)
                    ps2.__exit__(None, None, None)

                # ---- wo + residual (feature-major, fp8 DoubleRow)
                ps3 = tc.tile_pool(name="ps3", bufs=2, space="PSUM")
                psG = ps3.__enter__()
                wo_sb = lfw.tile([128, C6, D], FP8, tag="lfw")
                nc.sync.dma_start(wo_sb, lfw_d["lfwo"].ap())
                r1T = mid.tile([128, C6, T], BF16, tag="resT")
                for f in range(C6):
                    for nch in range(2):
                        ps = psG.tile([128, 512], F32, tag="g")
                        for j in range(3):
                            nc.tensor.matmul(
                                ps,
                                wo_sb[:, 2 * j:2 * j + 2,
                                      f * 128:(f + 1) * 128],
                                aT8[:, 2 * j:2 * j + 2,
                                    nch * 512:(nch + 1) * 512],
                                start=(j == 0), stop=(j == 2), perf_mode=DR)
                        t0 = work.tile([128, 512], BF16, tag="t0")
                        nc.scalar.activation(out=t0, in_=ps, func=ACTF.Copy,
                                             scale=DQ_WO)
                        nc.vector.tensor_tensor(
                            r1T[:, f, nch * 512:(nch + 1) * 512], t0,
                            hxT[:, f, 256 + nch * 512: 256 + (nch + 1) * 512],
                            ALU.add)
                ps3.__exit__(None, None, None)

            # FFN weights: w1 stays resident across both FFN passes, in a
            # pool that reuses the space just freed by the attention pools.
            ffnp = tc.tile_pool(name="ffnp", bufs=1)
            ffnpo = ffnp.__enter__()
            w1sb = ffnpo.tile([128, KC, C6, 128], BF16, tag="w1sb")
            nc.sync.dma_start(w1sb, w1_d.ap())

            # ---- LN1 (transpose to token-major, LN, transpose back)
            # software-pipelined: forward transposes of t+1 issue before the
            # back transposes of t, so the PE isn't stalled by the LN chain
            ps3b = tc.tile_pool(name="ps3b", bufs=3, space="PSUM")
            psT = ps3b.__enter__()
            h1T = mid.tile([128, C6, T], BF16, tag="h1T")

            def ln1_fwd(t):
                rtok = work.tile([128, D], BF16, tag="rtok")
                tp = psT.tile([128, C6, 128], BF16, tag="tp3")
                for c in range(C6):
                    nc.tensor.transpose(tp[:, c, :],
                                        r1T[:, c, t * 128:(t + 1) * 128],
                                        id_bf)
                nc.any.tensor_copy(out=rtok, in_=tp)
                ltok = work.tile([128, D], BF16, tag="ltok")
                _ln_tile(nc, sm, rtok, ltok, eps_sb)
                return ltok

            def ln1_back(t, ltok):
                tp2 = psT.tile([128, C6, 128], BF16, tag="tp3")
                for c in range(C6):
                    nc.tensor.transpose(tp2[:, c, :],
                                        ltok[:, c * 128:(c + 1) * 128],
                                        id_bf)
                nc.any.tensor_copy(out=h1T[:, :, t * 128:(t + 1) * 128],
                                   in_=tp2)

            prevL = None
            for t in range(8):
                ltok = ln1_fwd(t)
                if prevL is not None:
                    ln1_back(*prevL)
                prevL = (t, ltok)
            ln1_back(*prevL)
            ps3b.__exit__(None, None, None)

            # ---- FFN + LN2, one token-half at a time; each half's fp8
            # payload is exchanged as soon as it is ready so the second
            # half's FFN overlaps the first collective.
            r2T = mid.tile([128, C6, T], BF16, tag="resT")
            g1all = ffnpo.tile([128, KC, 512], BF16, tag="g1all")
            for nch in range(2):
                # g1 = gelu(h1 @ w1), all 24 dff chunks
                ps4 = tc.tile_pool(name="ps4", bufs=2, space="PSUM")
                psG = ps4.__enter__()
                for kc in range(KC):
                    g1p = psG.tile([128, 512], F32, tag="g")
                    for k in range(C6):
                        nc.tensor.matmul(
                            g1p, w1sb[:, kc, k, :],
                            h1T[:, k, nch * 512:(nch + 1) * 512],
                            start=(k == 0), stop=(k == 5))
                    nc.scalar.activation(out=g1all[:, kc, :], in_=g1p,
                                         func=ACTF.Gelu_apprx_tanh)
                ps4.__exit__(None, None, None)
                # f2 = g1 @ w2 (+ residual)
                ps4b = tc.tile_pool(name="ps4b", bufs=1, space="PSUM")
                psF = ps4b.__enter__()
                with tc.tile_pool(name="bigw", bufs=4) as bigw:
                    f2ps = [psF.tile([128, 512], F32, tag=f"f2_{m}",
                                     name=f"f2_{m}") for m in range(C6)]
                    for kc in range(KC):
                        w2c = bigw.tile([128, D], BF16, tag="w2c")
                        nc.sync.dma_start(
                            w2c, w2_d.ap()[kc * 128:(kc + 1) * 128, :])
                        for m in range(C6):
                            nc.tensor.matmul(
                                f2ps[m], w2c[:, m * 128:(m + 1) * 128],
                                g1all[:, kc, :],
                                start=(kc == 0), stop=(kc == KC - 1))
                    for m in range(C6):
                        nc.vector.tensor_tensor(
                            r2T[:, m, nch * 512:(nch + 1) * 512], f2ps[m],
                            h1T[:, m, nch * 512:(nch + 1) * 512], ALU.add)
                ps4b.__exit__(None, None, None)

                # LN2 for this half (pipelined like LN1)
                ps5 = tc.tile_pool(name="ps5", bufs=3, space="PSUM")
                psT5 = ps5.__enter__()

                def ln2_fwd(t):
                    rtok = work.tile([128, D], BF16, tag="rtok")
                    tp = psT5.tile([128, C6, 128], BF16, tag="tp5")
                    for c in range(C6):
                        nc.tensor.transpose(tp[:, c, :],
                                            r2T[:, c, t * 128:(t + 1) * 128],
                                            id_bf)
                    nc.any.tensor_copy(out=rtok, in_=tp)
                    otok = work.tile([128, D], BF16, tag="ltok")
                    _ln_tile(nc, sm, rtok, otok, eps_sb)
                    return otok

                def ln2_back(t, otok):
                    otok8 = work.tile([128, D], FP8, tag="otok8")
                    nc.scalar.activation(out=otok8, in_=otok, func=ACTF.Copy,
                                         scale=SC_OLF)
                    nc.sync.dma_start(
                        srcA[t // 4][(t % 4) * 128:(t % 4 + 1) * 128, :],
                        otok8)
                    tp2 = psT5.tile([128, C6, 128], BF16, tag="tp5")
                    for c in range(C6):
                        nc.tensor.transpose(tp2[:, c, :],
                                            otok[:, c * 128:(c + 1) * 128],
                                            id_bf)
                    nc.any.tensor_copy(
                        out=ownT[:, :, t * 128:(t + 1) * 128], in_=tp2)
                    nc.scalar.activation(
                        out=oT8[:, :, t * 128:(t + 1) * 128], in_=tp2,
                        func=ACTF.Copy, scale=SC_OLF)
                    if "tap_olf" in taps:
                        of = work.tile([128, D], F32, tag="tapolf")
                        nc.vector.tensor_copy(out=of, in_=otok)
                        nc.sync.dma_start(
                            taps["tap_olf"].ap()[t * 128:(t + 1) * 128, :],
                            of)

                prevT = None
                for t in range(nch * 4, nch * 4 + 4):
                    otok = ln2_fwd(t)
                    if prevT is not None:
                        ln2_back(*prevT)
                    prevT = (t, otok)
                ln2_back(*prevT)
                nc.sync.dma_start(
                    srcB[nch], oT8[:, :, nch * 512:(nch + 1) * 512])
                ps5.__exit__(None, None, None)

                # exchange this half right away
                nc.gpsimd.collective_compute(
                    "AllGather", ALU.bypass,
                    replica_groups=[[0, 1], [2, 3], [4, 5], [6, 7]],
                    ins=[srcA[nch][:].opt()], outs=[dstA[nch][:].opt()])
                nc.gpsimd.collective_compute(
                    "AllGather", ALU.bypass,
                    replica_groups=[[0, 1], [2, 3], [4, 5], [6, 7]],
                    ins=[srcB[nch][:].opt()], outs=[dstB[nch][:].opt()])
            ffnp.__exit__(None, None, None)

        # ============ STAGE B ============
        with tc.tile_pool(name="resB", bufs=1) as resB, \
             tc.tile_pool(name="whead", bufs=2) as whead, \
             tc.tile_pool(name="hb", bufs=2) as hb, \
             tc.tile_pool(name="workB", bufs=2) as workB, \
             tc.tile_pool(name="smB", bufs=4) as smB:

            ps6 = tc.tile_pool(name="psG2", bufs=2, space="PSUM")
            psG2 = ps6.__enter__()

            # mwB needed only for fc at the end; start the DMA early
            mwB8_sb = resB.tile([128, NH * C6, D], FP8, tag="mwB8")
            nc.sync.dma_start(mwB8_sb, mwB_d.ap())

            # --- pre-collective: q2 for all 4 heads from local oT8
            q2T8 = []
            for h in range(NH):
                wh8 = whead.tile([128, C6, D], FP8, tag="wh")
                nc.sync.dma_start(
                    wh8, mwA_d.ap()[:, h * C6 * D:(h + 1) * C6 * D])
                q2 = resB.tile([128, C6, T], FP8, tag=f"q2T8_{h}")
                for f in range(C6):
                    for nch in range(2):
                        ps = psG2.tile([128, 512], F32, tag="g2")
                        for j in range(3):
                            nc.tensor.matmul(
                                ps,
                                wh8[:, 2 * j:2 * j + 2,
                                    f * 128:(f + 1) * 128],
                                oT8[:, 2 * j:2 * j + 2,
                                    nch * 512:(nch + 1) * 512],
                                start=(j == 0), stop=(j == 2), perf_mode=DR)
                        nc.scalar.activation(
                            out=q2[:, f, nch * 512:(nch + 1) * 512],
                            in_=ps, func=ACTF.Copy, scale=DQ_Q2)
                q2T8.append(q2)

            # --- land collective results (no transposes needed)
            # global token chunk tt: half g = tt//8, sub-half s = (tt%8)//4
            olfT8 = resB.tile([128, C6, 2 * T], FP8, tag="olfT8")
            olftok8 = resB.tile([128, 16, 784], FP8, tag="olftok8")
            nc.vector.memset(olftok8[:, :, 768:769], ZCOL)
            for g in range(2):
                for s in range(2):
                    nc.sync.dma_start(
                        olfT8[:, :, g * T + s * 512:g * T + (s + 1) * 512],
                        dstB[s][g * 128:(g + 1) * 128, :])
            for tt in range(16):
                g, s, r = tt // 8, (tt % 8) // 4, tt % 4
                nc.sync.dma_start(
                    olftok8[:, tt, 0:768],
                    dstA[s][g * 512 + r * 128:g * 512 + (r + 1) * 128, :])

            poT8 = resB.tile([128, NH * C6, T], FP8, tag="poT8")
            fcacc = resB.tile([128, C6, T], BF16, tag="fcacc")

            ps7z = tc.tile_pool(name="psZ", bufs=2, space="PSUM")
            psZ = ps7z.__enter__()
            ps7b = tc.tile_pool(name="psBC", bufs=2, space="PSUM")
            psBC = ps7b.__enter__()
            ps7 = tc.tile_pool(name="psPV", bufs=2, space="PSUM")
            psPV = ps7.__enter__()

            def sc_issue(h, qch):
                expT8 = hb.tile([128, 16, 512], FP8, tag="expT8")
                for kt in range(16):
                    ps = psG2.tile([128, 512], F32, tag="g2")
                    for j in range(3):
                        nc.tensor.matmul(
                            ps,
                            olfT8[:, 2 * j:2 * j + 2,
                                  kt * 128:(kt + 1) * 128],
                            q2T8[h][:, 2 * j:2 * j + 2,
                                    qch * 512:(qch + 1) * 512],
                            start=(j == 0), stop=(j == 2), perf_mode=DR)
                    nc.scalar.activation(out=expT8[:, kt, :], in_=ps,
                                         func=ACTF.Exp,
                                         bias=kb16[:, kt:kt + 1],
                                         scale=ISQ_DK / 256.0)
                return expT8

            def pv_issue(h, qch, expT8):
                zp = psZ.tile([1, 512], F32, tag="z")
                for i in range(8):
                    nc.tensor.matmul(
                        zp, olftok8[:, 2 * i:2 * i + 2, 768:769],
                        expT8[:, 2 * i:2 * i + 2, :],
                        start=(i == 0), stop=(i == 7), perf_mode=DR)
                rs = smB.tile([1, 512], BF16, tag="rs2")
                with nc.allow_low_precision(
                        reason="1/Z feeds fp8-precision normalization"):
                    nc.vector.reciprocal(rs, zp)
                bc = psBC.tile([128, 512], F32, tag="bc")
                bc_sb = workB.tile([128, 512], BF16, tag="bc_sb")
                for c in range(C6):
                    pp = psPV.tile([128, 512], F32, tag="pv")
                    for i in range(8):
                        nc.tensor.matmul(
                            pp,
                            olftok8[:, 2 * i:2 * i + 2,
                                    c * 128:(c + 1) * 128],
                            expT8[:, 2 * i:2 * i + 2, :],
                            start=(i == 0), stop=(i == 7), perf_mode=DR)
                    if c == 0:
                        nc.tensor.matmul(bc, ones_bf, rs,
                                         start=True, stop=True)
                        nc.scalar.activation(out=bc_sb, in_=bc,
                                             func=ACTF.Copy)
                    nc.vector.tensor_tensor(
                        poT8[:, h * C6 + c, qch * 512:(qch + 1) * 512],
                        pp, bc_sb, ALU.mult)

            # head loop: sc issued one step ahead of pv
            prev = None
            for h in range(NH):
                for qch in range(2):
                    expT8 = sc_issue(h, qch)
                    if prev is not None:
                        pv_issue(*prev)
                    prev = (h, qch, expT8)
            pv_issue(*prev)

            ps7.__exit__(None, None, None)
            ps7b.__exit__(None, None, None)
            ps7z.__exit__(None, None, None)

            # --- output projection over concatenated heads + residual,
            # interleaved with the tail LN/max per token-half so the
            # serial LN chain overlaps the second half's fc matmuls
            ps8 = tc.tile_pool(name="psT2", bufs=3, space="PSUM")
            psT2 = ps8.__enter__()
            ps9 = tc.tile_pool(name="psTail", bufs=2, space="PSUM")
            psTail = ps9.__enter__()
            maxacc = resB.tile([128, D], F32, tag="maxacc")

            def tail_t(t):
                rtok = workB.tile([128, D], BF16, tag="rtokB")
                tp = psT2.tile([128, C6, 128], BF16, tag="tpB")
                for c in range(C6):
                    nc.tensor.transpose(tp[:, c, :],
                                        fcacc[:, c, t * 128:(t + 1) * 128],
                                        id_bf)
                nc.any.tensor_copy(out=rtok, in_=tp)
                ltok = workB.tile([128, D], F32, tag="ltokB")
                _ln_tile(nc, smB, rtok, ltok, eps_sb)
                if "tap_attn" in taps:
                    nc.sync.dma_start(
                        taps["tap_attn"].ap()[t * 128:(t + 1) * 128, :], ltok)
                if t == 0:
                    nc.vector.tensor_copy(out=maxacc, in_=ltok)
                else:
                    nc.vector.tensor_tensor(maxacc, maxacc, ltok, ALU.max)

            for nch in range(2):
                for m in range(C6):
                    ps = psG2.tile([128, 512], F32, tag="g2")
                    for j in range(NH * C6 // 2):
                        nc.tensor.matmul(
                            ps,
                            mwB8_sb[:, 2 * j:2 * j + 2,
                                    m * 128:(m + 1) * 128],
                            poT8[:, 2 * j:2 * j + 2,
                                 nch * 512:(nch + 1) * 512],
                            start=(j == 0), stop=(j == NH * C6 // 2 - 1),
                            perf_mode=DR)
                    t1 = workB.tile([128, 512], BF16, tag="t1")
                    nc.scalar.activation(out=t1, in_=ps, func=ACTF.Copy,
                                         scale=DQ_FC)
                    nc.vector.tensor_tensor(
                        fcacc[:, m, nch * 512:(nch + 1) * 512], t1,
                        ownT[:, m, nch * 512:(nch + 1) * 512], ALU.add)
                for t in range(nch * 4, nch * 4 + 4):
                    tail_t(t)
            outsb = resB.tile([128, 6], F32, tag="outsb")
            for c in range(C6):
                pt = psTail.tile([128, 128], F32, tag="tpf")
                nc.tensor.transpose(pt, maxacc[:, c * 128:(c + 1) * 128],
                                    id_f32)
                nc.vector.tensor_reduce(out=outsb[:, c:c + 1], in_=pt,
                                        axis=AX.X, op=ALU.max)
            nc.sync.dma_start(out_d.ap(), outsb)
            ps9.__exit__(None, None, None)
            ps8.__exit__(None, None, None)
            ps6.__exit__(None, None, None)

    return


# ---------------- host side ----------------

_NC_CACHE = {}


def _get_nc(debug=False):
    key = bool(debug)
    if key not in _NC_CACHE:
        _NC_CACHE[key] = build(debug=debug)
    return _NC_CACHE[key]


def _prep_in_maps(inputs):
    bf = ml_dtypes.bfloat16
    f8 = ml_dtypes.float8_e4m3
    x = np.asarray(inputs["x"])
    emb = np.asarray(inputs["emb"], np.float32)
    pos = np.asarray(inputs["pos"], np.float32)
    g_e = np.asarray(inputs["ln_e_g"], np.float32)
    b_e = np.asarray(inputs["ln_e_b"], np.float32)

    def parr(w):
        w = np.ascontiguousarray(
            np.asarray(w, np.float32).reshape(C6, 128, -1)
            .transpose(1, 0, 2)).astype(bf)
        return w.reshape(128, -1)

    def parr8(w, scale):
        w = np.clip(np.asarray(w, np.float32) * scale, -240.0, 240.0)
        w = np.ascontiguousarray(
            w.reshape(-1, 128, w.shape[-1]).transpose(1, 0, 2)).astype(f8)
        return w.reshape(128, -1)

    wts = {
        "lfwq": parr8(inputs["lf_wq"], SC_LFW),
        "lfwk": parr8(inputs["lf_wk"], SC_LFW),
        "lfwv": parr8(inputs["lf_wv"], SC_LFW),
        "lfwo": parr8(inputs["lf_wo"], SC_LFW),
    }
    w1 = np.asarray(inputs["w1"], np.float32)
    wts["w1"] = np.ascontiguousarray(
        w1.reshape(C6, 128, KC, 128).transpose(1, 2, 0, 3)
    ).astype(bf).reshape(128, KC * D)
    wts["w2"] = np.asarray(inputs["w2"], np.float32).astype(bf)

    # folded stage-B matrices, fp8 x256
    wq = np.asarray(inputs["mha_wq"], np.float32).reshape(D, NH, DK)
    wk = np.asarray(inputs["mha_wk"], np.float32).reshape(D, NH, DK)
    wv = np.asarray(inputs["mha_wv"], np.float32).reshape(D, NH, DK)
    fc = np.asarray(inputs["mha_fc"], np.float32).reshape(NH, DK, D)
    mwA = np.concatenate(
        [parr8(wq[:, h, :] @ wk[:, h, :].T, SC_W) for h in range(NH)], axis=1)
    Bcat = np.concatenate([wv[:, h, :] @ fc[h] for h in range(NH)], axis=0)
    wts["mwA"] = np.ascontiguousarray(mwA)
    wts["mwB"] = np.ascontiguousarray(parr8(Bcat, SC_W))

    in_maps = []
    for b in range(B):
        h0 = emb[x[b]] + pos                        # [S, D] f32
        mu = h0.mean(-1, keepdims=True)
        var = h0.var(-1, keepdims=True)
        hn = (h0 - mu) / np.sqrt(var + EPS) * g_e + b_e
        kbias = np.where(x[b] != 0, 0.0, NEG).astype(np.float32)
        kb16 = np.ascontiguousarray(kbias.reshape(16, 128).T)
        for p in range(2):
            start = p * T - 256
            hxe = np.zeros((EXT, D), np.float32)
            lo, hi = max(0, start), min(S, start + EXT)
            hxe[lo - start: hi - start] = hn[lo:hi]
            hxf = np.ascontiguousarray(
                hxe.reshape(EXT, C6, 128).transpose(2, 1, 0))
            hxT = hxf.astype(bf).reshape(128, C6 * EXT)
            hx8 = np.clip(hxf * SC_HX, -240.0, 240.0).astype(f8).reshape(
                128, C6 * EXT)

            qi = np.arange(128)
            kj = np.arange(640)
            m640 = np.zeros((8, 128, 640), np.float32)
            for qt in range(8):
                qg = p * T + qt * 128 + qi[:, None]
                kg = start + qt * 128 + kj[None, :]
                ok = (np.abs(kg - qg) <= W) & (kg >= 0) & (kg < S)
                m640[qt] = np.where(ok, 0.0, NEG)

            m = {"hxT": hxT, "hx8": hx8, "m640": m640, "kb16": kb16}
            m.update(wts)
            in_maps.append(m)
    return in_maps


def _postprocess(results):
    out = np.zeros((B, D), np.float32)
    for b in range(B):
        m0 = np.asarray(results[2 * b]["out"]).T.reshape(D)
        m1 = np.asarray(results[2 * b + 1]["out"]).T.reshape(D)
        out[b] = np.maximum(m0, m1)
    return out


def run(inputs, debug=False, trace=False):
    nc = _get_nc(debug=debug)
    in_maps = _prep_in_maps(inputs)
    res = run_bass_kernel_spmd(nc, in_maps, core_ids=list(range(NCORES)),
                               trace=trace)
    return res


def kernel(**inputs):
    res = run(inputs, debug=False, trace=False)
    return _postprocess(res.results)


# revision 32
# speedup vs baseline: 1.3936x; 1.0264x over previous
"""Trainium2 Bass kernel for nn_LongformerEncoder (optimized v3).

Sharding: 8 cores = (batch b in 0..3, seq-half p in 0..1).
Stage A (longformer layer) runs on 1024 own tokens (+256-token halo).
A pairwise AllGather exchanges stage-A output; stage B (4-head/768-dim
MHA + max-pool) runs seq-split on queries with full keys, partial max
per core, final max across the pair on host.

v3 changes vs v2:
- Stage B entirely in fp8 (e4m3) with DoubleRow matmuls (2 contraction
  rows per PE pass): q2 projection, scores, PV, and the concatenated
  output projection. Attention contributes ~1.3% of the pre-LN signal,
  so fp8 error is negligible in the final output.
- The collective payload carries the stage-A output in fp8 in BOTH
  layouts (token-major for PV values, feature-major for score keys),
  eliminating all post-collective PE transposes in stage B.
- PV computed feature-major (lhsT = values chunk), so the attention
  output lands pre-transposed for the output projection; softmax
  denominator via a dedicated Z-column matmul, normalization via a
  ones-broadcast matmul + one DVE multiply per chunk.
- fc done once over the 4 heads' concatenated poT (single PSUM
  accumulation group; no inter-head DVE adds).
"""

import sys

sys.path.insert(0, "/opt/trn_rl_repo")

import numpy as np
import ml_dtypes

import concourse.bass as bass
import concourse.tile as tile
from concourse import bacc, mybir
from concourse.bass_utils import run_bass_kernel_spmd
from concourse.masks import make_identity

F32 = mybir.dt.float32
BF16 = mybir.dt.bfloat16
FP8 = mybir.dt.float8e4
AX = mybir.AxisListType
ALU = mybir.AluOpType
ACTF = mybir.ActivationFunctionType
DR = mybir.MatmulPerfMode.DoubleRow

B, S, D = 4, 2048, 768
W = 256
DFF = 3072
NH, DK = 4, 768
T = 1024            # own tokens per core
EXT = 1536          # own + 256 halo each side
NEG = -1e9
EPS = 1e-5
NCORES = 8
C6 = D // 128        # 6 feature chunks
KC = DFF // 128      # 24 dff chunks
ISQ_DH = 0.125       # 1/sqrt(64)
ISQ_DK = 1.0 / float(np.sqrt(DK))
SC_OLF = 16.0        # fp8 scale of stage-A output (both layouts)
SC_W = 256.0         # fp8 scale of folded stage-B weights
ZCOL = 1.0 / 16.0    # Z-helper column value so po lands at 256x true
DQ_Q2 = 1.0 / 256.0  # psum(16*256*q2) -> 16*q2
DQ_FC = 1.0 / 65536.0  # psum(256*256*fc) -> fc
SC_HX = 16.0         # fp8 scale of LN'd embeddings (stage-A input)
SC_LFW = 1024.0      # fp8 scale of longformer q/k/v/o weights
SC_AT = 32.0         # fp8 scale of stage-A attention output
DQ_QKV = 1.0 / (SC_HX * SC_LFW)
DQ_WO = 1.0 / (SC_AT * SC_LFW)


def build(debug=False):
    nc = bacc.Bacc("TRN2", target_bir_lowering=False, debug=False,
                   num_devices=NCORES)

    hxT_d = nc.dram_tensor("hxT", [128, C6 * EXT], BF16, kind="ExternalInput")
    hx8_d = nc.dram_tensor("hx8", [128, C6 * EXT], FP8, kind="ExternalInput")
    m640_d = nc.dram_tensor("m640", [8, 128, 640], F32, kind="ExternalInput")
    kb16_d = nc.dram_tensor("kb16", [128, 16], F32, kind="ExternalInput")
    lfw_d = {}
    for nm in ["lfwq", "lfwk", "lfwv", "lfwo"]:
        lfw_d[nm] = nc.dram_tensor(nm, [128, C6 * D], FP8,
                                   kind="ExternalInput")
    w1_d = nc.dram_tensor("w1", [128, KC * D], BF16, kind="ExternalInput")
    w2_d = nc.dram_tensor("w2", [DFF, D], BF16, kind="ExternalInput")
    mwA_d = nc.dram_tensor("mwA", [128, NH * C6 * D], FP8,
                           kind="ExternalInput")
    mwB_d = nc.dram_tensor("mwB", [128, NH * C6 * D], FP8,
                           kind="ExternalInput")
    out_d = nc.dram_tensor("out", [128, 6], F32, kind="ExternalOutput")
    taps = {}
    if debug:
        taps["tap_olf"] = nc.dram_tensor("tap_olf", [T, D], F32,
                                         kind="ExternalOutput")
        taps["tap_attn"] = nc.dram_tensor("tap_attn", [T, D], F32,
                                          kind="ExternalOutput")

    with tile.TileContext(nc) as tc:
        _body(nc, tc, hxT_d, hx8_d, m640_d, kb16_d, lfw_d, w1_d, w2_d,
              mwA_d, mwB_d, out_d, taps)
    nc.compile()
    return nc


def _ln_tile(nc, pool, x_ap, out_tile, eps_ap):
    """out = (x - mean)/sqrt(var+eps) over free dim (768). g==1, b==0."""
    stats = pool.tile([128, 2, 6], F32, tag="lnstats")
    nc.vector.bn_stats(out=stats[:, 0, :], in_=x_ap[:, 0:384])
    nc.vector.bn_stats(out=stats[:, 1, :], in_=x_ap[:, 384:768])
    mv = pool.tile([128, 2], F32, tag="lnmv")
    nc.vector.bn_aggr(out=mv, in_=stats)
    rstd = pool.tile([128, 1], F32, tag="lnrstd")
    nc.scalar.activation(out=rstd, in_=mv[:, 1:2], func=ACTF.Sqrt, bias=eps_ap)
    nc.vector.reciprocal(out=rstd, in_=rstd)
    nc.vector.tensor_scalar(out=out_tile, in0=x_ap, scalar1=mv[:, 0:1],
                            scalar2=rstd, op0=ALU.subtract, op1=ALU.mult)


def _body(nc, tc, hxT_d, hx8_d, m640_d, kb16_d, lfw_d, w1_d, w2_d,
          mwA_d, mwB_d, out_d, taps):
    import contextlib
    ctx = contextlib.ExitStack()
    with ctx:
        constg = ctx.enter_context(tc.tile_pool(name="constg", bufs=1))
        outer = ctx.enter_context(tc.tile_pool(name="outer", bufs=1))
        dram = ctx.enter_context(tc.tile_pool(name="dram", bufs=1,
                                              space="DRAM"))

        id_bf = constg.tile([128, 128], BF16, tag="id_bf")
        make_identity(nc, id_bf)
        id_f32 = constg.tile([128, 128], F32, tag="id_f32")
        make_identity(nc, id_f32)
        eps_sb = constg.tile([128, 1], F32, tag="eps")
        nc.vector.memset(eps_sb, EPS)
        kb16 = constg.tile([128, 16], F32, tag="kb16")
        nc.sync.dma_start(kb16, kb16_d.ap())
        ones_bf = constg.tile([1, 128], BF16, tag="ones_bf")
        nc.vector.memset(ones_bf, 1.0)

        # cross-stage tiles (stage-A output for stage B)
        ownT = outer.tile([128, C6, T], BF16, tag="ownT")    # 12K/part
        oT8 = outer.tile([128, C6, T], FP8, tag="oT8")       # 6K/part

        # DRAM bounce for the collectives (fp8, both layouts, split in
        # two token-halves so the first exchange overlaps the second
        # half's FFN)
        srcA = [dram.tile([T // 2, D], FP8, name=f"srcA{i}")
                for i in range(2)]
        dstA = [dram.tile([T, D], FP8, name=f"dstA{i}") for i in range(2)]
        srcB = [dram.tile([128, C6 * 512], FP8, name=f"srcB{i}")
                for i in range(2)]
        dstB = [dram.tile([256, C6 * 512], FP8, name=f"dstB{i}")
                for i in range(2)]

        # ============ STAGE A ============
        with tc.tile_pool(name="mid", bufs=1) as mid, \
             tc.tile_pool(name="sm", bufs=4) as sm, \
             tc.tile_pool(name="work", bufs=2) as work:

            with tc.tile_pool(name="inA", bufs=1) as inA, \
                 tc.tile_pool(name="attA2", bufs=1) as attA2, \
                 tc.tile_pool(name="lfw", bufs=2) as lfw:

                hx8 = inA.tile([128, C6, EXT], FP8, tag="hx8")
                nc.sync.dma_start(hx8, hx8_d.ap())
                hxT = inA.tile([128, C6, EXT], BF16, tag="hxT")
                aT8 = attA2.tile([128, C6, T], FP8, tag="aT8")

                with tc.tile_pool(name="attA1", bufs=1) as attA1:
                    # ---- q/k feature-major, v token-major (fp8 DoubleRow)
                    ps1 = tc.tile_pool(name="ps1", bufs=2, space="PSUM")
                    psG = ps1.__enter__()
                    wq_sb = lfw.tile([128, C6, D], FP8, tag="lfw")
                    nc.sync.dma_start(wq_sb, lfw_d["lfwq"].ap())
                    qT = attA1.tile([128, C6, T], BF16, tag="qT")
                    for f in range(C6):
                        for nch in range(2):
                            ps = psG.tile([128, 512], F32, tag="g")
                            for j in range(3):
                                nc.tensor.matmul(
                                    ps,
                                    wq_sb[:, 2 * j:2 * j + 2,
                                          f * 128:(f + 1) * 128],
                                    hx8[:, 2 * j:2 * j + 2,
                                        256 + nch * 512:
                                        256 + (nch + 1) * 512],
                                    start=(j == 0), stop=(j == 2),
                                    perf_mode=DR)
                            nc.scalar.activation(
                                out=qT[:, f, nch * 512:(nch + 1) * 512],
                                in_=ps, func=ACTF.Copy, scale=DQ_QKV)
                    wk_sb = lfw.tile([128, C6, D], FP8, tag="lfw")
                    nc.sync.dma_start(wk_sb, lfw_d["lfwk"].ap())
                    # hxT (bf16 residual) only needed at wo; load now
                    nc.sync.dma_start(hxT, hxT_d.ap())
                    kT = attA1.tile([128, C6, EXT], BF16, tag="kT")
                    for f in range(C6):
                        for nch in range(3):
                            ps = psG.tile([128, 512], F32, tag="g")
                            for j in range(3):
                                nc.tensor.matmul(
                                    ps,
                                    wk_sb[:, 2 * j:2 * j + 2,
                                          f * 128:(f + 1) * 128],
                                    hx8[:, 2 * j:2 * j + 2,
                                        nch * 512:(nch + 1) * 512],
                                    start=(j == 0), stop=(j == 2),
                                    perf_mode=DR)
                            nc.scalar.activation(
                                out=kT[:, f, nch * 512:(nch + 1) * 512],
                                in_=ps, func=ACTF.Copy, scale=DQ_QKV)
                    wv_sb = lfw.tile([128, C6, D], FP8, tag="lfw")
                    nc.sync.dma_start(wv_sb, lfw_d["lfwv"].ap())
                    # values token-major in fp8 at x32 (PV runs in fp8)
                    vtok8 = attA1.tile([128, 12, D], FP8, tag="vtok8")
                    for t in range(12):
                        for (n0, nn) in ((0, 512), (512, 256)):
                            ps = psG.tile([128, 512], F32, tag="g")
                            for j in range(3):
                                nc.tensor.matmul(
                                    ps[:, :nn],
                                    hx8[:, 2 * j:2 * j + 2,
                                        t * 128:(t + 1) * 128],
                                    wv_sb[:, 2 * j:2 * j + 2, n0:n0 + nn],
                                    start=(j == 0), stop=(j == 2),
                                    perf_mode=DR)
                            nc.scalar.activation(
                                out=vtok8[:, t, n0:n0 + nn], in_=ps[:, :nn],
                                func=ACTF.Copy, scale=DQ_QKV * SC_AT)
                    ps1.__exit__(None, None, None)

                    # ---- sliding-window attention, k-major scores so the
                    # probs land contraction-ready (no transpose matmuls);
                    # PV in fp8 DoubleRow; softmax Z via a ones-row matmul,
                    # normalization via GPSIMD partition-broadcast + DVE.
                    ps2 = tc.tile_pool(name="ps2", bufs=2, space="PSUM")
                    psS = ps2.__enter__()
                    ps2c = tc.tile_pool(name="ps2c", bufs=2, space="PSUM")
                    psV = ps2c.__enter__()
                    ps2z = tc.tile_pool(name="ps2z", bufs=2, space="PSUM")
                    psZ1 = ps2z.__enter__()

                    ones8 = constg.tile([128, 2, 16], FP8, tag="ones8")
                    nc.vector.memset(ones8, 1.0)

                    m640_t = [None] * 8

                    def a_scores(qt, pair, h2):
                        if pair == 0 and h2 == 0:
                            m640_t[qt] = work.tile([128, 640], F32,
                                                   tag="m640", name="m640")
                            nc.sync.dma_start(m640_t[qt], m640_d.ap()[qt])
                        ps = psS.tile([128, 640], F32, tag="sc")
                        rhsq = qT[h2 * 64:(h2 + 1) * 64, pair,
                                  qt * 128:(qt + 1) * 128]
                        for dx in range(5):
                            nc.tensor.matmul(
                                ps[:, dx * 128:(dx + 1) * 128],
                                kT[h2 * 64:(h2 + 1) * 64, pair,
                                   qt * 128 + dx * 128:
                                   qt * 128 + (dx + 1) * 128],
                                rhsq, start=True, stop=True,
                                tile_position=(h2 * 64, 0))
                        sb = work.tile([128, 640], F32, tag="sb")
                        nc.vector.tensor_tensor(sb, ps, m640_t[qt], ALU.add)
                        probs8 = work.tile([128, 5, 128], FP8, tag="probs8")
                        nc.scalar.activation(out=probs8, in_=sb,
                                             func=ACTF.Exp, scale=ISQ_DH)
                        return probs8

                    def a_rest(qt, pair, h2, probs8):
                        h = 2 * pair + h2
                        zq = psZ1.tile([1, 128], F32, tag="zq")
                        pvt = psV.tile([128, 128], F32, tag="pv")
                        for i in range(2):
                            nc.tensor.matmul(
                                zq, ones8[:, :, 0:1],
                                probs8[:, 2 * i:2 * i + 2, :],
                                start=(i == 0), stop=False, perf_mode=DR)
                        nc.tensor.matmul(zq, ones8[:, 0, 0:1],
                                         probs8[:, 4, :],
                                         start=False, stop=True)
                        if h2 == 0:
                            # DoubleRow requires dst partition offset 0
                            for i in range(2):
                                nc.tensor.matmul(
                                    pvt[0:64, :],
                                    vtok8[:, qt + 2 * i:qt + 2 * i + 2,
                                          h * 64:(h + 1) * 64],
                                    probs8[:, 2 * i:2 * i + 2, :],
                                    start=(i == 0), stop=False, perf_mode=DR,
                                    tile_position=(0, 0))
                            nc.tensor.matmul(
                                pvt[0:64, :],
                                vtok8[:, qt + 4, h * 64:(h + 1) * 64],
                                probs8[:, 4, :], start=False, stop=True,
                                tile_position=(0, 0))
                        else:
                            for dx in range(5):
                                nc.tensor.matmul(
                                    pvt[64:128, :],
                                    vtok8[:, qt + dx, h * 64:(h + 1) * 64],
                                    probs8[:, dx, :], start=(dx == 0),
                                    stop=(dx == 4),
                                    tile_position=(0, 64))
                        rs = sm.tile([1, 128], F32, tag="rs")
                        nc.vector.reciprocal(rs, zq)
                        rs_bc = work.tile([128, 128], F32, tag="rs_bc")
                        nc.gpsimd.partition_broadcast(rs_bc, rs)
                        nc.vector.tensor_tensor(
                            aT8[h2 * 64:(h2 + 1) * 64, pair,
                                qt * 128:(qt + 1) * 128],
                            pvt[h2 * 64:(h2 + 1) * 64, :],
                            rs_bc[h2 * 64:(h2 + 1) * 64, :], ALU.mult)

                    its = [(qt, pair, h2) for qt in range(8)
                           for pair in range(6) for h2 in range(2)]
                    prev = None
                    for it in its:
                        probs8 = a_scores(*it)
                        if prev is not None:
                            a_rest(prev[0][0], prev[0][1], prev[0][2],
                                   prev[1])
                        prev = (it, probs8)
                    a_rest(prev[0][0], prev[0][1], prev[0][2], prev[1])

                    ps2z.__exit__(None, None, None)
                    ps2c.__exit__(None, None, None)
                    ps2.__exit__(None, None, None)

                # ---- wo + residual (feature-major, fp8 DoubleRow)
                ps3 = tc.tile_pool(name="ps3", bufs=2, space="PSUM")
                psG = ps3.__enter__()
                wo_sb = lfw.tile([128, C6, D], FP8, tag="lfw")
                nc.sync.dma_start(wo_sb, lfw_d["lfwo"].ap())
                r1T = mid.tile([128, C6, T], BF16, tag="resT")
                for f in range(C6):
                    for nch in range(2):
                        ps = psG.tile([128, 512], F32, tag="g")
                        for j in range(3):
                            nc.tensor.matmul(
                                ps,
                                wo_sb[:, 2 * j:2 * j + 2,
                                      f * 128:(f + 1) * 128],
                                aT8[:, 2 * j:2 * j + 2,
                                    nch * 512:(nch + 1) * 512],
                                start=(j == 0), stop=(j == 2), perf_mode=DR)
                        t0 = work.tile([128, 512], BF16, tag="t0")
                        nc.scalar.activation(out=t0, in_=ps, func=ACTF.Copy,
                                             scale=DQ_WO)
                        nc.vector.tensor_tensor(
                            r1T[:, f, nch * 512:(nch + 1) * 512], t0,
                            hxT[:, f, 256 + nch * 512: 256 + (nch + 1) * 512],
                            ALU.add)
                ps3.__exit__(None, None, None)

            # FFN weights: w1 stays resident across both FFN passes, in a
            # pool that reuses the space just freed by the attention pools.
            ffnp = tc.tile_pool(name="ffnp", bufs=1)
            ffnpo = ffnp.__enter__()
            w1sb = ffnpo.tile([128, KC, C6, 128], BF16, tag="w1sb")
            nc.sync.dma_start(w1sb, w1_d.ap())

            # ---- LN1 (transpose to token-major, LN, transpose back)
            # software-pipelined: forward transposes of t+1 issue before the
            # back transposes of t, so the PE isn't stalled by the LN chain
            ps3b = tc.tile_pool(name="ps3b", bufs=3, space="PSUM")
            psT = ps3b.__enter__()
            h1T = mid.tile([128, C6, T], BF16, tag="h1T")

            def ln1_fwd(t):
                rtok = work.tile([128, D], BF16, tag="rtok")
                tp = psT.tile([128, C6, 128], BF16, tag="tp3")
                for c in range(C6):
                    nc.tensor.transpose(tp[:, c, :],
                                        r1T[:, c, t * 128:(t + 1) * 128],
                                        id_bf)
                nc.any.tensor_copy(out=rtok, in_=tp)
                ltok = work.tile([128, D], BF16, tag="ltok")
                _ln_tile(nc, sm, rtok, ltok, eps_sb)
                return ltok

            def ln1_back(t, ltok):
                tp2 = psT.tile([128, C6, 128], BF16, tag="tp3")
                for c in range(C6):
                    nc.tensor.transpose(tp2[:, c, :],
                                        ltok[:, c * 128:(c + 1) * 128],
                                        id_bf)
                nc.any.tensor_copy(out=h1T[:, :, t * 128:(t + 1) * 128],
                                   in_=tp2)

            prevL = None
            for t in range(8):
                ltok = ln1_fwd(t)
                if prevL is not None:
                    ln1_back(*prevL)
                prevL = (t, ltok)
            ln1_back(*prevL)
            ps3b.__exit__(None, None, None)

            # ---- FFN + LN2, one token-half at a time; each half's fp8
            # payload is exchanged as soon as it is ready so the second
            # half's FFN overlaps the first collective.
            r2T = mid.tile([128, C6, T], BF16, tag="resT")
            g1all = ffnpo.tile([128, KC, 512], BF16, tag="g1all")
            for nch in range(2):
                # g1 = gelu(h1 @ w1), all 24 dff chunks
                ps4 = tc.tile_pool(name="ps4", bufs=2, space="PSUM")
                psG = ps4.__enter__()
                for kc in range(KC):
                    g1p = psG.tile([128, 512], F32, tag="g")
                    for k in range(C6):
                        nc.tensor.matmul(
                            g1p, w1sb[:, kc, k, :],
                            h1T[:, k, nch * 512:(nch + 1) * 512],
                            start=(k == 0), stop=(k == 5))
                    nc.scalar.activation(out=g1all[:, kc, :], in_=g1p,
                                         func=ACTF.Gelu_apprx_tanh)
                ps4.__exit__(None, None, None)
                # f2 = g1 @ w2 (+ residual)
                ps4b = tc.tile_pool(name="ps4b", bufs=1, space="PSUM")
                psF = ps4b.__enter__()
                with tc.tile_pool(name="bigw", bufs=4) as bigw:
                    f2ps = [psF.tile([128, 512], F32, tag=f"f2_{m}",
                                     name=f"f2_{m}") for m in range(C6)]
                    for kc in range(KC):
                        w2c = bigw.tile([128, D], BF16, tag="w2c")
                        nc.sync.dma_start(
                            w2c, w2_d.ap()[kc * 128:(kc + 1) * 128, :])
                        for m in range(C6):
                            nc.tensor.matmul(
                                f2ps[m], w2c[:, m * 128:(m + 1) * 128],
                                g1all[:, kc, :],
                                start=(kc == 0), stop=(kc == KC - 1))
                    for m in range(C6):
                        nc.vector.tensor_tensor(
                            r2T[:, m, nch * 512:(nch + 1) * 512], f2ps[m],
                            h1T[:, m, nch * 512:(nch + 1) * 512], ALU.add)
                ps4b.__exit__(None, None, None)

                # LN2 for this half (pipelined like LN1)
                ps5 = tc.tile_pool(name="ps5", bufs=3, space="PSUM")
                psT5 = ps5.__enter__()

                def ln2_fwd(t):
                    rtok = work.tile([128, D], BF16, tag="rtok")
                    tp = psT5.tile([128, C6, 128], BF16, tag="tp5")
                    for c in range(C6):
                        nc.tensor.transpose(tp[:, c, :],
                                            r2T[:, c, t * 128:(t + 1) * 128],
                                            id_bf)
                    nc.any.tensor_copy(out=rtok, in_=tp)
                    otok = work.tile([128, D], BF16, tag="ltok")
                    _ln_tile(nc, sm, rtok, otok, eps_sb)
                    return otok

                def ln2_back(t, otok):
                    otok8 = work.tile([128, D], FP8, tag="otok8")
                    nc.scalar.activation(out=otok8, in_=otok, func=ACTF.Copy,
                                         scale=SC_OLF)
                    nc.sync.dma_start(
                        srcA[t // 4][(t % 4) * 128:(t % 4 + 1) * 128, :],
                        otok8)
                    tp2 = psT5.tile([128, C6, 128], BF16, tag="tp5")
                    for c in range(C6):
                        nc.tensor.transpose(tp2[:, c, :],
                                            otok[:, c * 128:(c + 1) * 128],
                                            id_bf)
                    nc.any.tensor_copy(
                        out=ownT[:, :, t * 128:(t + 1) * 128], in_=tp2)
                    nc.scalar.activation(
                        out=oT8[:, :, t * 128:(t + 1) * 128], in_=tp2,
                        func=ACTF.Copy, scale=SC_OLF)
                    if "tap_olf" in taps:
                        of = work.tile([128, D], F32, tag="tapolf")
                        nc.vector.tensor_copy(out=of, in_=otok)
                        nc.sync.dma_start(
                            taps["tap_olf"].ap()[t * 128:(t + 1) * 128, :],
                            of)

                prevT = None
                for t in range(nch * 4, nch * 4 + 4):
                    otok = ln2_fwd(t)
                    if prevT is not None:
                        ln2_back(*prevT)
                    prevT = (t, otok)
                ln2_back(*prevT)
                nc.sync.dma_start(
                    srcB[nch], oT8[:, :, nch * 512:(nch + 1) * 512])
                ps5.__exit__(None, None, None)

                # exchange this half right away
                nc.gpsimd.collective_compute(
                    "AllGather", ALU.bypass,
                    replica_groups=[[0, 1], [2, 3], [4, 5], [6, 7]],
                    ins=[srcA[nch][:].opt()], outs=[dstA[nch][:].opt()])
                nc.gpsimd.collective_compute(
                    "AllGather", ALU.bypass,
                    replica_groups=[[0, 1], [2, 3], [4, 5], [6, 7]],
                    ins=[srcB[nch][:].opt()], outs=[dstB[nch][:].opt()])
            ffnp.__exit__(None, None, None)

        # ============ STAGE B ============
        with tc.tile_pool(name="resB", bufs=1) as resB, \
             tc.tile_pool(name="whead", bufs=2) as whead, \
             tc.tile_pool(name="hb", bufs=2) as hb, \
             tc.tile_pool(name="workB", bufs=2) as workB, \
             tc.tile_pool(name="smB", bufs=4) as smB:

            ps6 = tc.tile_pool(name="psG2", bufs=2, space="PSUM")
            psG2 = ps6.__enter__()

            # mwB needed only for fc at the end; start the DMA early
            mwB8_sb = resB.tile([128, NH * C6, D], FP8, tag="mwB8")
            nc.sync.dma_start(mwB8_sb, mwB_d.ap())

            # --- pre-collective: q2 for all 4 heads from local oT8
            q2T8 = []
            for h in range(NH):
                wh8 = whead.tile([128, C6, D], FP8, tag="wh")
                nc.sync.dma_start(
                    wh8, mwA_d.ap()[:, h * C6 * D:(h + 1) * C6 * D])
                q2 = resB.tile([128, C6, T], FP8, tag=f"q2T8_{h}")
                for f in range(C6):
                    for nch in range(2):
                        ps = psG2.tile([128, 512], F32, tag="g2")
                        for j in range(3):
                            nc.tensor.matmul(
                                ps,
                                wh8[:, 2 * j:2 * j + 2,
                                    f * 128:(f + 1) * 128],
                                oT8[:, 2 * j:2 * j + 2,
                                    nch * 512:(nch + 1) * 512],
                                start=(j == 0), stop=(j == 2), perf_mode=DR)
                        nc.scalar.activation(
                            out=q2[:, f, nch * 512:(nch + 1) * 512],
                            in_=ps, func=ACTF.Copy, scale=DQ_Q2)
                q2T8.append(q2)

            # --- land collective results (no transposes needed)
            # global token chunk tt: half g = tt//8, sub-half s = (tt%8)//4
            olfT8 = resB.tile([128, C6, 2 * T], FP8, tag="olfT8")
            olftok8 = resB.tile([128, 16, 784], FP8, tag="olftok8")
            nc.vector.memset(olftok8[:, :, 768:769], ZCOL)
            for g in range(2):
                for s in range(2):
                    nc.sync.dma_start(
                        olfT8[:, :, g * T + s * 512:g * T + (s + 1) * 512],
                        dstB[s][g * 128:(g + 1) * 128, :])
            for tt in range(16):
                g, s, r = tt // 8, (tt % 8) // 4, tt % 4
                nc.sync.dma_start(
                    olftok8[:, tt, 0:768],
                    dstA[s][g * 512 + r * 128:g * 512 + (r + 1) * 128, :])

            poT8 = resB.tile([128, NH * C6, T], FP8, tag="poT8")
            fcacc = resB.tile([128, C6, T], BF16, tag="fcacc")

            ps7z = tc.tile_pool(name="psZ", bufs=2, space="PSUM")
            psZ = ps7z.__enter__()
            ps7b = tc.tile_pool(name="psBC", bufs=2, space="PSUM")
            psBC = ps7b.__enter__()
            ps7 = tc.tile_pool(name="psPV", bufs=2, space="PSUM")
            psPV = ps7.__enter__()

            def sc_issue(h, qch):
                expT8 = hb.tile([128, 16, 512], FP8, tag="expT8")
                for kt in range(16):
                    ps = psG2.tile([128, 512], F32, tag="g2")
                    for j in range(3):
                        nc.tensor.matmul(
                            ps,
                            olfT8[:, 2 * j:2 * j + 2,
                                  kt * 128:(kt + 1) * 128],
                            q2T8[h][:, 2 * j:2 * j + 2,
                                    qch * 512:(qch + 1) * 512],
                            start=(j == 0), stop=(j == 2), perf_mode=DR)
                    nc.scalar.activation(out=expT8[:, kt, :], in_=ps,
                                         func=ACTF.Exp,
                                         bias=kb16[:, kt:kt + 1],
                                         scale=ISQ_DK / 256.0)
                return expT8

            def pv_issue(h, qch, expT8):
                zp = psZ.tile([1, 512], F32, tag="z")
                for i in range(8):
                    nc.tensor.matmul(
                        zp, olftok8[:, 2 * i:2 * i + 2, 768:769],
                        expT8[:, 2 * i:2 * i + 2, :],
                        start=(i == 0), stop=(i == 7), perf_mode=DR)
                rs = smB.tile([1, 512], BF16, tag="rs2")
                with nc.allow_low_precision(
                        reason="1/Z feeds fp8-precision normalization"):
                    nc.vector.reciprocal(rs, zp)
                bc = psBC.tile([128, 512], F32, tag="bc")
                bc_sb = workB.tile([128, 512], BF16, tag="bc_sb")
                for c in range(C6):
                    pp = psPV.tile([128, 512], F32, tag="pv")
                    for i in range(8):
                        nc.tensor.matmul(
                            pp,
                            olftok8[:, 2 * i:2 * i + 2,
                                    c * 128:(c + 1) * 128],
                            expT8[:, 2 * i:2 * i + 2, :],
                            start=(i == 0), stop=(i == 7), perf_mode=DR)
                    if c == 0:
                        nc.tensor.matmul(bc, ones_bf, rs,
                                         start=True, stop=True)
                        nc.scalar.activation(out=bc_sb, in_=bc,
                                             func=ACTF.Copy)
                    nc.vector.tensor_tensor(
                        poT8[:, h * C6 + c, qch * 512:(qch + 1) * 512],
                        pp, bc_sb, ALU.mult)

            # head loop: sc issued one step ahead of pv
            prev = None
            for h in range(NH):
                for qch in range(2):
                    expT8 = sc_issue(h, qch)
                    if prev is not None:
                        pv_issue(*prev)
                    prev = (h, qch, expT8)
            pv_issue(*prev)

            ps7.__exit__(None, None, None)
            ps7b.__exit__(None, None, None)
            ps7z.__exit__(None, None, None)

            # --- output projection over concatenated heads + residual,
            # interleaved with the tail LN/max per token-half so the
            # serial LN chain overlaps the second half's fc matmuls
            ps8 = tc.tile_pool(name="psT2", bufs=3, space="PSUM")
            psT2 = ps8.__enter__()
            ps9 = tc.tile_pool(name="psTail", bufs=2, space="PSUM")
            psTail = ps9.__enter__()
            maxacc = resB.tile([128, D], F32, tag="maxacc")

            def tail_t(t):
                rtok = workB.tile([128, D], BF16, tag="rtokB")
                tp = psT2.tile([128, C6, 128], BF16, tag="tpB")
                for c in range(C6):
                    nc.tensor.transpose(tp[:, c, :],
                                        fcacc[:, c, t * 128:(t + 1) * 128],
                                        id_bf)
                nc.any.tensor_copy(out=rtok, in_=tp)
                ltok = workB.tile([128, D], F32, tag="ltokB")
                _ln_tile(nc, smB, rtok, ltok, eps_sb)
                if "tap_attn" in taps:
                    nc.sync.dma_start(
                        taps["tap_attn"].ap()[t * 128:(t + 1) * 128, :], ltok)
                if t == 0:
                    nc.vector.tensor_copy(out=maxacc, in_=ltok)
                else:
                    nc.vector.tensor_tensor(maxacc, maxacc, ltok, ALU.max)

            for nch in range(2):
                for m in range(C6):
                    ps = psG2.tile([128, 512], F32, tag="g2")
                    for j in range(NH * C6 // 2):
                        nc.tensor.matmul(
                            ps,
                            mwB8_sb[:, 2 * j:2 * j + 2,
                                    m * 128:(m + 1) * 128],
                            poT8[:, 2 * j:2 * j + 2,
                                 nch * 512:(nch + 1) * 512],
                            start=(j == 0), stop=(j == NH * C6 // 2 - 1),
                            perf_mode=DR)
                    t1 = workB.tile([128, 512], BF16, tag="t1")
                    nc.scalar.activation(out=t1, in_=ps, func=ACTF.Copy,
                                         scale=DQ_FC)
                    nc.vector.tensor_tensor(
                        fcacc[:, m, nch * 512:(nch + 1) * 512], t1,
                        ownT[:, m, nch * 512:(nch + 1) * 512], ALU.add)
                for t in range(nch * 4, nch * 4 + 4):
                    tail_t(t)
            outsb = resB.tile([128, 6], F32, tag="outsb")
            for c in range(C6):
                pt = psTail.tile([128, 128], F32, tag="tpf")
                nc.tensor.transpose(pt, maxacc[:, c * 128:(c + 1) * 128],
                                    id_f32)
                nc.vector.tensor_reduce(out=outsb[:, c:c + 1], in_=pt,
                                        axis=AX.X, op=ALU.max)
            nc.sync.dma_start(out_d.ap(), outsb)
            ps9.__exit__(None, None, None)
            ps8.__exit__(None, None, None)
            ps6.__exit__(None, None, None)

    return


# ---------------- host side ----------------

_NC_CACHE = {}


def _get_nc(debug=False):
    key = bool(debug)
    if key not in _NC_CACHE:
        _NC_CACHE[key] = build(debug=debug)
    return _NC_CACHE[key]


def _prep_in_maps(inputs):
    bf = ml_dtypes.bfloat16
    f8 = ml_dtypes.float8_e4m3
    x = np.asarray(inputs["x"])
    emb = np.asarray(inputs["emb"], np.float32)
    pos = np.asarray(inputs["pos"], np.float32)
    g_e = np.asarray(inputs["ln_e_g"], np.float32)
    b_e = np.asarray(inputs["ln_e_b"], np.float32)

    def parr(w):
        w = np.ascontiguousarray(
            np.asarray(w, np.float32).reshape(C6, 128, -1)
            .transpose(1, 0, 2)).astype(bf)
        return w.reshape(128, -1)

    def parr8(w, scale):
        w = np.clip(np.asarray(w, np.float32) * scale, -240.0, 240.0)
        w = np.ascontiguousarray(
            w.reshape(-1, 128, w.shape[-1]).transpose(1, 0, 2)).astype(f8)
        return w.reshape(128, -1)

    wts = {
        "lfwq": parr8(inputs["lf_wq"], SC_LFW),
        "lfwk": parr8(inputs["lf_wk"], SC_LFW),
        "lfwv": parr8(inputs["lf_wv"], SC_LFW),
        "lfwo": parr8(inputs["lf_wo"], SC_LFW),
    }
    w1 = np.asarray(inputs["w1"], np.float32)
    wts["w1"] = np.ascontiguousarray(
        w1.reshape(C6, 128, KC, 128).transpose(1, 2, 0, 3)
    ).astype(bf).reshape(128, KC * D)
    wts["w2"] = np.asarray(inputs["w2"], np.float32).astype(bf)

    # folded stage-B matrices, fp8 x256
    wq = np.asarray(inputs["mha_wq"], np.float32).reshape(D, NH, DK)
    wk = np.asarray(inputs["mha_wk"], np.float32).reshape(D, NH, DK)
    wv = np.asarray(inputs["mha_wv"], np.float32).reshape(D, NH, DK)
    fc = np.asarray(inputs["mha_fc"], np.float32).reshape(NH, DK, D)
    mwA = np.concatenate(
        [parr8(wq[:, h, :] @ wk[:, h, :].T, SC_W) for h in range(NH)], axis=1)
    Bcat = np.concatenate([wv[:, h, :] @ fc[h] for h in range(NH)], axis=0)
    wts["mwA"] = np.ascontiguousarray(mwA)
    wts["mwB"] = np.ascontiguousarray(parr8(Bcat, SC_W))

    in_maps = []
    for b in range(B):
        h0 = emb[x[b]] + pos                        # [S, D] f32
        mu = h0.mean(-1, keepdims=True)
        var = h0.var(-1, keepdims=True)
        hn = (h0 - mu) / np.sqrt(var + EPS) * g_e + b_e
        kbias = np.where(x[b] != 0, 0.0, NEG).astype(np.float32)
        kb16 = np.ascontiguousarray(kbias.reshape(16, 128).T)
        for p in range(2):
            start = p * T - 256
            hxe = np.zeros((EXT, D), np.float32)
            lo, hi = max(0, start), min(S, start + EXT)
            hxe[lo - start: hi - start] = hn[lo:hi]
            hxf = np.ascontiguousarray(
                hxe.reshape(EXT, C6, 128).transpose(2, 1, 0))
            hxT = hxf.astype(bf).reshape(128, C6 * EXT)
            hx8 = np.clip(hxf * SC_HX, -240.0, 240.0).astype(f8).reshape(
                128, C6 * EXT)

            qi = np.arange(128)
            kj = np.arange(640)
            m640 = np.zeros((8, 128, 640), np.float32)
            for qt in range(8):
                qg = p * T + qt * 128 + qi[:, None]
                kg = start + qt * 128 + kj[None, :]
                ok = (np.abs(kg - qg) <= W) & (kg >= 0) & (kg < S)
                # k-major: [key-in-chunk, dx-chunk, query]
                m640[qt] = np.ascontiguousarray(
                    np.where(ok, 0.0, NEG).T.reshape(5, 128, 128)
                    .transpose(1, 0, 2)).reshape(128, 640)

            m = {"hxT": hxT, "hx8": hx8, "m640": m640, "kb16": kb16}
            m.update(wts)
            in_maps.append(m)
    return in_maps


def _postprocess(results):
    out = np.zeros((B, D), np.float32)
    for b in range(B):
        m0 = np.asarray(results[2 * b]["out"]).T.reshape(D)
        m1 = np.asarray(results[2 * b + 1]["out"]).T.reshape(D)
        out[b] = np.maximum(m0, m1)
    return out


def run(inputs, debug=False, trace=False):
    nc = _get_nc(debug=debug)
    in_maps = _prep_in_maps(inputs)
    res = run_bass_kernel_spmd(nc, in_maps, core_ids=list(range(NCORES)),
                               trace=trace)
    return res


def kernel(**inputs):
    res = run(inputs, debug=False, trace=False)
    return _postprocess(res.results)


# revision 34
# speedup vs baseline: 1.3984x; 1.0035x over previous
"""Trainium2 Bass kernel for nn_LongformerEncoder (optimized v3).

Sharding: 8 cores = (batch b in 0..3, seq-half p in 0..1).
Stage A (longformer layer) runs on 1024 own tokens (+256-token halo).
A pairwise AllGather exchanges stage-A output; stage B (4-head/768-dim
MHA + max-pool) runs seq-split on queries with full keys, partial max
per core, final max across the pair on host.

v3 changes vs v2:
- Stage B entirely in fp8 (e4m3) with DoubleRow matmuls (2 contraction
  rows per PE pass): q2 projection, scores, PV, and the concatenated
  output projection. Attention contributes ~1.3% of the pre-LN signal,
  so fp8 error is negligible in the final output.
- The collective payload carries the stage-A output in fp8 in BOTH
  layouts (token-major for PV values, feature-major for score keys),
  eliminating all post-collective PE transposes in stage B.
- PV computed feature-major (lhsT = values chunk), so the attention
  output lands pre-transposed for the output projection; softmax
  denominator via a dedicated Z-column matmul, normalization via a
  ones-broadcast matmul + one DVE multiply per chunk.
- fc done once over the 4 heads' concatenated poT (single PSUM
  accumulation group; no inter-head DVE adds).
"""

import sys

sys.path.insert(0, "/opt/trn_rl_repo")

import numpy as np
import ml_dtypes

import concourse.bass as bass
import concourse.tile as tile
from concourse import bacc, mybir
from concourse.bass_utils import run_bass_kernel_spmd
from concourse.masks import make_identity

F32 = mybir.dt.float32
BF16 = mybir.dt.bfloat16
FP8 = mybir.dt.float8e4
AX = mybir.AxisListType
ALU = mybir.AluOpType
ACTF = mybir.ActivationFunctionType
DR = mybir.MatmulPerfMode.DoubleRow

B, S, D = 4, 2048, 768
W = 256
DFF = 3072
NH, DK = 4, 768
T = 1024            # own tokens per core
EXT = 1536          # own + 256 halo each side
NEG = -1e9
EPS = 1e-5
NCORES = 8
C6 = D // 128        # 6 feature chunks
KC = DFF // 128      # 24 dff chunks
ISQ_DH = 0.125       # 1/sqrt(64)
ISQ_DK = 1.0 / float(np.sqrt(DK))
SC_OLF = 16.0        # fp8 scale of stage-A output (both layouts)
SC_W = 256.0         # fp8 scale of folded stage-B weights
ZCOL = 1.0 / 16.0    # Z-helper column value so po lands at 256x true
DQ_Q2 = 1.0 / 256.0  # psum(16*256*q2) -> 16*q2
DQ_FC = 1.0 / 65536.0  # psum(256*256*fc) -> fc
SC_HX = 16.0         # fp8 scale of LN'd embeddings (stage-A input)
SC_LFW = 1024.0      # fp8 scale of longformer q/k/v/o weights
SC_AT = 32.0         # fp8 scale of stage-A attention output
DQ_QKV = 1.0 / (SC_HX * SC_LFW)
DQ_WO = 1.0 / (SC_AT * SC_LFW)


def build(debug=False):
    nc = bacc.Bacc("TRN2", target_bir_lowering=False, debug=False,
                   num_devices=NCORES)

    hxT_d = nc.dram_tensor("hxT", [128, C6 * EXT], BF16, kind="ExternalInput")
    hx8_d = nc.dram_tensor("hx8", [128, C6 * EXT], FP8, kind="ExternalInput")
    m640_d = nc.dram_tensor("m640", [8, 128, 640], F32, kind="ExternalInput")
    kb16_d = nc.dram_tensor("kb16", [128, 16], F32, kind="ExternalInput")
    lfw_d = {}
    for nm in ["lfwq", "lfwk", "lfwv", "lfwo"]:
        lfw_d[nm] = nc.dram_tensor(nm, [128, C6 * D], FP8,
                                   kind="ExternalInput")
    w1_d = nc.dram_tensor("w1", [128, KC * D], BF16, kind="ExternalInput")
    w2_d = nc.dram_tensor("w2", [DFF, D], BF16, kind="ExternalInput")
    mwA_d = nc.dram_tensor("mwA", [128, NH * C6 * D], FP8,
                           kind="ExternalInput")
    mwB_d = nc.dram_tensor("mwB", [128, NH * C6 * D], FP8,
                           kind="ExternalInput")
    out_d = nc.dram_tensor("out", [128, 6], F32, kind="ExternalOutput")
    taps = {}
    if debug:
        taps["tap_olf"] = nc.dram_tensor("tap_olf", [T, D], F32,
                                         kind="ExternalOutput")
        taps["tap_attn"] = nc.dram_tensor("tap_attn", [T, D], F32,
                                          kind="ExternalOutput")

    with tile.TileContext(nc) as tc:
        _body(nc, tc, hxT_d, hx8_d, m640_d, kb16_d, lfw_d, w1_d, w2_d,
              mwA_d, mwB_d, out_d, taps)
    nc.compile()
    return nc


def _ln_tile(nc, pool, x_ap, out_tile, eps_ap):
    """out = (x - mean)/sqrt(var+eps) over free dim (768). g==1, b==0.

    The wide normalize runs on the scalar engine (out = x*rstd - mu*rstd)
    so the DVE only carries the stats chain.
    """
    stats = pool.tile([128, 2, 6], F32, tag="lnstats")
    nc.vector.bn_stats(out=stats[:, 0, :], in_=x_ap[:, 0:384])
    nc.vector.bn_stats(out=stats[:, 1, :], in_=x_ap[:, 384:768])
    mv = pool.tile([128, 2], F32, tag="lnmv")
    nc.vector.bn_aggr(out=mv, in_=stats)
    rstd = pool.tile([128, 1], F32, tag="lnrstd")
    nc.scalar.activation(out=rstd, in_=mv[:, 1:2], func=ACTF.Sqrt, bias=eps_ap)
    nc.vector.reciprocal(out=rstd, in_=rstd)
    nb = pool.tile([128, 1], F32, tag="lnnb")
    nc.vector.tensor_scalar(out=nb, in0=mv[:, 0:1], scalar1=rstd,
                            scalar2=-1.0, op0=ALU.mult, op1=ALU.mult)
    nc.scalar.activation(out=out_tile, in_=x_ap, func=ACTF.Identity,
                         scale=rstd, bias=nb)


def _body(nc, tc, hxT_d, hx8_d, m640_d, kb16_d, lfw_d, w1_d, w2_d,
          mwA_d, mwB_d, out_d, taps):
    import contextlib
    ctx = contextlib.ExitStack()
    with ctx:
        constg = ctx.enter_context(tc.tile_pool(name="constg", bufs=1))
        outer = ctx.enter_context(tc.tile_pool(name="outer", bufs=1))
        dram = ctx.enter_context(tc.tile_pool(name="dram", bufs=1,
                                              space="DRAM"))

        id_bf = constg.tile([128, 128], BF16, tag="id_bf")
        make_identity(nc, id_bf)
        id_f32 = constg.tile([128, 128], F32, tag="id_f32")
        make_identity(nc, id_f32)
        eps_sb = constg.tile([128, 1], F32, tag="eps")
        nc.vector.memset(eps_sb, EPS)
        kb16 = constg.tile([128, 16], F32, tag="kb16")
        nc.sync.dma_start(kb16, kb16_d.ap())
        ones_bf = constg.tile([1, 128], BF16, tag="ones_bf")
        nc.vector.memset(ones_bf, 1.0)

        # cross-stage tiles (stage-A output for stage B)
        ownT = outer.tile([128, C6, T], BF16, tag="ownT")    # 12K/part
        oT8 = outer.tile([128, C6, T], FP8, tag="oT8")       # 6K/part

        # DRAM bounce for the collectives (fp8, both layouts, split in
        # two token-halves so the first exchange overlaps the second
        # half's FFN)
        srcA = [dram.tile([T // 2, D], FP8, name=f"srcA{i}")
                for i in range(2)]
        dstA = [dram.tile([T, D], FP8, name=f"dstA{i}") for i in range(2)]
        srcB = [dram.tile([128, C6 * 512], FP8, name=f"srcB{i}")
                for i in range(2)]
        dstB = [dram.tile([256, C6 * 512], FP8, name=f"dstB{i}")
                for i in range(2)]

        # ============ STAGE A ============
        with tc.tile_pool(name="mid", bufs=1) as mid, \
             tc.tile_pool(name="sm", bufs=4) as sm, \
             tc.tile_pool(name="work", bufs=2) as work:

            with tc.tile_pool(name="inA", bufs=1) as inA, \
                 tc.tile_pool(name="attA2", bufs=1) as attA2, \
                 tc.tile_pool(name="lfw", bufs=2) as lfw:

                hx8 = inA.tile([128, C6, EXT], FP8, tag="hx8")
                for j in range(3):
                    nc.sync.dma_start(
                        hx8[:, 2 * j:2 * j + 2, :],
                        hx8_d.ap()[:, 2 * j * EXT:(2 * j + 2) * EXT])
                hxT = inA.tile([128, C6, EXT], BF16, tag="hxT")
                aT8 = attA2.tile([128, C6, T], FP8, tag="aT8")

                with tc.tile_pool(name="attA1", bufs=1) as attA1:
                    # ---- q/k feature-major, v token-major (fp8 DoubleRow)
                    ps1 = tc.tile_pool(name="ps1", bufs=2, space="PSUM")
                    psG = ps1.__enter__()
                    wq_sb = lfw.tile([128, C6, D], FP8, tag="lfw")
                    nc.sync.dma_start(wq_sb, lfw_d["lfwq"].ap())
                    qT = attA1.tile([128, C6, T], BF16, tag="qT")
                    for f in range(C6):
                        for nch in range(2):
                            ps = psG.tile([128, 512], F32, tag="g")
                            for j in range(3):
                                nc.tensor.matmul(
                                    ps,
                                    wq_sb[:, 2 * j:2 * j + 2,
                                          f * 128:(f + 1) * 128],
                                    hx8[:, 2 * j:2 * j + 2,
                                        256 + nch * 512:
                                        256 + (nch + 1) * 512],
                                    start=(j == 0), stop=(j == 2),
                                    perf_mode=DR)
                            nc.scalar.activation(
                                out=qT[:, f, nch * 512:(nch + 1) * 512],
                                in_=ps, func=ACTF.Copy, scale=DQ_QKV)
                    wk_sb = lfw.tile([128, C6, D], FP8, tag="lfw")
                    nc.sync.dma_start(wk_sb, lfw_d["lfwk"].ap())
                    # hxT (bf16 residual) only needed at wo; load now
                    nc.sync.dma_start(hxT, hxT_d.ap())
                    kT = attA1.tile([128, C6, EXT], BF16, tag="kT")
                    for f in range(C6):
                        for nch in range(3):
                            ps = psG.tile([128, 512], F32, tag="g")
                            for j in range(3):
                                nc.tensor.matmul(
                                    ps,
                                    wk_sb[:, 2 * j:2 * j + 2,
                                          f * 128:(f + 1) * 128],
                                    hx8[:, 2 * j:2 * j + 2,
                                        nch * 512:(nch + 1) * 512],
                                    start=(j == 0), stop=(j == 2),
                                    perf_mode=DR)
                            nc.scalar.activation(
                                out=kT[:, f, nch * 512:(nch + 1) * 512],
                                in_=ps, func=ACTF.Copy, scale=DQ_QKV)
                    wv_sb = lfw.tile([128, C6, D], FP8, tag="lfw")
                    nc.sync.dma_start(wv_sb, lfw_d["lfwv"].ap())
                    # values token-major in fp8 at x32 (PV runs in fp8)
                    vtok8 = attA1.tile([128, 12, D], FP8, tag="vtok8")
                    for t in range(12):
                        for (n0, nn) in ((0, 512), (512, 256)):
                            ps = psG.tile([128, 512], F32, tag="g")
                            for j in range(3):
                                nc.tensor.matmul(
                                    ps[:, :nn],
                                    hx8[:, 2 * j:2 * j + 2,
                                        t * 128:(t + 1) * 128],
                                    wv_sb[:, 2 * j:2 * j + 2, n0:n0 + nn],
                                    start=(j == 0), stop=(j == 2),
                                    perf_mode=DR)
                            nc.scalar.activation(
                                out=vtok8[:, t, n0:n0 + nn], in_=ps[:, :nn],
                                func=ACTF.Copy, scale=DQ_QKV * SC_AT)
                    ps1.__exit__(None, None, None)

                    # ---- sliding-window attention, k-major scores so the
                    # probs land contraction-ready (no transpose matmuls);
                    # PV in fp8 DoubleRow; softmax Z via a ones-row matmul,
                    # normalization via GPSIMD partition-broadcast + DVE.
                    ps2 = tc.tile_pool(name="ps2", bufs=2, space="PSUM")
                    psS = ps2.__enter__()
                    ps2c = tc.tile_pool(name="ps2c", bufs=2, space="PSUM")
                    psV = ps2c.__enter__()
                    ps2z = tc.tile_pool(name="ps2z", bufs=2, space="PSUM")
                    psZ1 = ps2z.__enter__()

                    ones8 = constg.tile([128, 2, 16], FP8, tag="ones8")
                    nc.vector.memset(ones8, 1.0)

                    m640_t = [None] * 8

                    def a_scores(qt, pair, h2):
                        if pair == 0 and h2 == 0:
                            m640_t[qt] = work.tile([128, 640], F32,
                                                   tag="m640", name="m640")
                            nc.sync.dma_start(m640_t[qt], m640_d.ap()[qt])
                        ps = psS.tile([128, 640], F32, tag="sc")
                        rhsq = qT[h2 * 64:(h2 + 1) * 64, pair,
                                  qt * 128:(qt + 1) * 128]
                        for dx in range(5):
                            nc.tensor.matmul(
                                ps[:, dx * 128:(dx + 1) * 128],
                                kT[h2 * 64:(h2 + 1) * 64, pair,
                                   qt * 128 + dx * 128:
                                   qt * 128 + (dx + 1) * 128],
                                rhsq, start=True, stop=True,
                                tile_position=(h2 * 64, 0))
                        sb = work.tile([128, 640], F32, tag="sb")
                        nc.vector.tensor_tensor(sb, ps, m640_t[qt], ALU.add)
                        probs8 = work.tile([128, 5, 128], FP8, tag="probs8")
                        nc.scalar.activation(out=probs8, in_=sb,
                                             func=ACTF.Exp, scale=ISQ_DH)
                        return probs8

                    def a_rest(qt, pair, h2, probs8):
                        h = 2 * pair + h2
                        zq = psZ1.tile([1, 128], F32, tag="zq")
                        pvt = psV.tile([128, 128], F32, tag="pv")
                        for i in range(2):
                            nc.tensor.matmul(
                                zq, ones8[:, :, 0:1],
                                probs8[:, 2 * i:2 * i + 2, :],
                                start=(i == 0), stop=False, perf_mode=DR)
                        nc.tensor.matmul(zq, ones8[:, 0, 0:1],
                                         probs8[:, 4, :],
                                         start=False, stop=True)
                        if h2 == 0:
                            # DoubleRow requires dst partition offset 0
                            for i in range(2):
                                nc.tensor.matmul(
                                    pvt[0:64, :],
                                    vtok8[:, qt + 2 * i:qt + 2 * i + 2,
                                          h * 64:(h + 1) * 64],
                                    probs8[:, 2 * i:2 * i + 2, :],
                                    start=(i == 0), stop=False, perf_mode=DR,
                                    tile_position=(0, 0))
                            nc.tensor.matmul(
                                pvt[0:64, :],
                                vtok8[:, qt + 4, h * 64:(h + 1) * 64],
                                probs8[:, 4, :], start=False, stop=True,
                                tile_position=(0, 0))
                        else:
                            for dx in range(5):
                                nc.tensor.matmul(
                                    pvt[64:128, :],
                                    vtok8[:, qt + dx, h * 64:(h + 1) * 64],
                                    probs8[:, dx, :], start=(dx == 0),
                                    stop=(dx == 4),
                                    tile_position=(0, 64))
                        rs = sm.tile([1, 128], F32, tag="rs")
                        nc.vector.reciprocal(rs, zq)
                        rs_bc = work.tile([128, 128], F32, tag="rs_bc")
                        nc.gpsimd.partition_broadcast(rs_bc, rs)
                        nc.vector.tensor_tensor(
                            aT8[h2 * 64:(h2 + 1) * 64, pair,
                                qt * 128:(qt + 1) * 128],
                            pvt[h2 * 64:(h2 + 1) * 64, :],
                            rs_bc[h2 * 64:(h2 + 1) * 64, :], ALU.mult)

                    its = [(qt, pair, h2) for qt in range(8)
                           for pair in range(6) for h2 in range(2)]
                    prev = None
                    for it in its:
                        probs8 = a_scores(*it)
                        if prev is not None:
                            a_rest(prev[0][0], prev[0][1], prev[0][2],
                                   prev[1])
                        prev = (it, probs8)
                    a_rest(prev[0][0], prev[0][1], prev[0][2], prev[1])

                    ps2z.__exit__(None, None, None)
                    ps2c.__exit__(None, None, None)
                    ps2.__exit__(None, None, None)

                # ---- wo + residual (feature-major, fp8 DoubleRow)
                ps3 = tc.tile_pool(name="ps3", bufs=2, space="PSUM")
                psG = ps3.__enter__()
                wo_sb = lfw.tile([128, C6, D], FP8, tag="lfw")
                nc.sync.dma_start(wo_sb, lfw_d["lfwo"].ap())
                r1T = mid.tile([128, C6, T], BF16, tag="resT")
                for f in range(C6):
                    for nch in range(2):
                        ps = psG.tile([128, 512], F32, tag="g")
                        for j in range(3):
                            nc.tensor.matmul(
                                ps,
                                wo_sb[:, 2 * j:2 * j + 2,
                                      f * 128:(f + 1) * 128],
                                aT8[:, 2 * j:2 * j + 2,
                                    nch * 512:(nch + 1) * 512],
                                start=(j == 0), stop=(j == 2), perf_mode=DR)
                        t0 = work.tile([128, 512], BF16, tag="t0")
                        nc.scalar.activation(out=t0, in_=ps, func=ACTF.Copy,
                                             scale=DQ_WO)
                        nc.vector.tensor_tensor(
                            r1T[:, f, nch * 512:(nch + 1) * 512], t0,
                            hxT[:, f, 256 + nch * 512: 256 + (nch + 1) * 512],
                            ALU.add)
                ps3.__exit__(None, None, None)

            # FFN weights: w1 stays resident across both FFN passes, in a
            # pool that reuses the space just freed by the attention pools.
            ffnp = tc.tile_pool(name="ffnp", bufs=1)
            ffnpo = ffnp.__enter__()
            w1sb = ffnpo.tile([128, KC, C6, 128], BF16, tag="w1sb")
            nc.sync.dma_start(w1sb, w1_d.ap())

            # ---- LN1 (transpose to token-major, LN, transpose back)
            # software-pipelined: forward transposes of t+1 issue before the
            # back transposes of t, so the PE isn't stalled by the LN chain
            ps3b = tc.tile_pool(name="ps3b", bufs=3, space="PSUM")
            psT = ps3b.__enter__()
            h1T = mid.tile([128, C6, T], BF16, tag="h1T")

            def ln1_fwd(t):
                rtok = work.tile([128, D], BF16, tag="rtok")
                tp = psT.tile([128, C6, 128], BF16, tag="tp3")
                for c in range(C6):
                    nc.tensor.transpose(tp[:, c, :],
                                        r1T[:, c, t * 128:(t + 1) * 128],
                                        id_bf)
                nc.any.tensor_copy(out=rtok, in_=tp)
                ltok = work.tile([128, D], BF16, tag="ltok")
                _ln_tile(nc, sm, rtok, ltok, eps_sb)
                return ltok

            def ln1_back(t, ltok):
                tp2 = psT.tile([128, C6, 128], BF16, tag="tp3")
                for c in range(C6):
                    nc.tensor.transpose(tp2[:, c, :],
                                        ltok[:, c * 128:(c + 1) * 128],
                                        id_bf)
                nc.any.tensor_copy(out=h1T[:, :, t * 128:(t + 1) * 128],
                                   in_=tp2)

            prevL = None
            for t in range(8):
                ltok = ln1_fwd(t)
                if prevL is not None:
                    ln1_back(*prevL)
                prevL = (t, ltok)
            ln1_back(*prevL)
            ps3b.__exit__(None, None, None)

            # ---- FFN + LN2, one token-half at a time; each half's fp8
            # payload is exchanged as soon as it is ready so the second
            # half's FFN overlaps the first collective.
            r2T = mid.tile([128, C6, T], BF16, tag="resT")
            g1all = ffnpo.tile([128, KC, 512], BF16, tag="g1all")
            for nch in range(2):
                # g1 = gelu(h1 @ w1), all 24 dff chunks
                ps4 = tc.tile_pool(name="ps4", bufs=2, space="PSUM")
                psG = ps4.__enter__()
                for kc in range(KC):
                    g1p = psG.tile([128, 512], F32, tag="g")
                    for k in range(C6):
                        nc.tensor.matmul(
                            g1p, w1sb[:, kc, k, :],
                            h1T[:, k, nch * 512:(nch + 1) * 512],
                            start=(k == 0), stop=(k == 5))
                    nc.scalar.activation(out=g1all[:, kc, :], in_=g1p,
                                         func=ACTF.Gelu_apprx_tanh)
                ps4.__exit__(None, None, None)
                # f2 = g1 @ w2 (+ residual)
                ps4b = tc.tile_pool(name="ps4b", bufs=1, space="PSUM")
                psF = ps4b.__enter__()
                with tc.tile_pool(name="bigw", bufs=4) as bigw:
                    f2ps = [psF.tile([128, 512], F32, tag=f"f2_{m}",
                                     name=f"f2_{m}") for m in range(C6)]
                    for kc in range(KC):
                        w2c = bigw.tile([128, D], BF16, tag="w2c")
                        nc.sync.dma_start(
                            w2c, w2_d.ap()[kc * 128:(kc + 1) * 128, :])
                        for m in range(C6):
                            nc.tensor.matmul(
                                f2ps[m], w2c[:, m * 128:(m + 1) * 128],
                                g1all[:, kc, :],
                                start=(kc == 0), stop=(kc == KC - 1))
                    for m in range(C6):
                        nc.vector.tensor_tensor(
                            r2T[:, m, nch * 512:(nch + 1) * 512], f2ps[m],
                            h1T[:, m, nch * 512:(nch + 1) * 512], ALU.add)
                ps4b.__exit__(None, None, None)

                # LN2 for this half (pipelined like LN1)
                ps5 = tc.tile_pool(name="ps5", bufs=3, space="PSUM")
                psT5 = ps5.__enter__()

                def ln2_fwd(t):
                    rtok = work.tile([128, D], BF16, tag="rtok")
                    tp = psT5.tile([128, C6, 128], BF16, tag="tp5")
                    for c in range(C6):
                        nc.tensor.transpose(tp[:, c, :],
                                            r2T[:, c, t * 128:(t + 1) * 128],
                                            id_bf)
                    nc.any.tensor_copy(out=rtok, in_=tp)
                    otok = work.tile([128, D], BF16, tag="ltok")
                    _ln_tile(nc, sm, rtok, otok, eps_sb)
                    return otok

                def ln2_back(t, otok):
                    otok8 = work.tile([128, D], FP8, tag="otok8")
                    nc.scalar.activation(out=otok8, in_=otok, func=ACTF.Copy,
                                         scale=SC_OLF)
                    nc.sync.dma_start(
                        srcA[t // 4][(t % 4) * 128:(t % 4 + 1) * 128, :],
                        otok8)
                    tp2 = psT5.tile([128, C6, 128], BF16, tag="tp5")
                    for c in range(C6):
                        nc.tensor.transpose(tp2[:, c, :],
                                            otok[:, c * 128:(c + 1) * 128],
                                            id_bf)
                    nc.any.tensor_copy(
                        out=ownT[:, :, t * 128:(t + 1) * 128], in_=tp2)
                    nc.scalar.activation(
                        out=oT8[:, :, t * 128:(t + 1) * 128], in_=tp2,
                        func=ACTF.Copy, scale=SC_OLF)
                    if "tap_olf" in taps:
                        of = work.tile([128, D], F32, tag="tapolf")
                        nc.vector.tensor_copy(out=of, in_=otok)
                        nc.sync.dma_start(
                            taps["tap_olf"].ap()[t * 128:(t + 1) * 128, :],
                            of)

                prevT = None
                for t in range(nch * 4, nch * 4 + 4):
                    otok = ln2_fwd(t)
                    if prevT is not None:
                        ln2_back(*prevT)
                    prevT = (t, otok)
                ln2_back(*prevT)
                nc.sync.dma_start(
                    srcB[nch], oT8[:, :, nch * 512:(nch + 1) * 512])
                ps5.__exit__(None, None, None)

                # exchange this half right away
                nc.gpsimd.collective_compute(
                    "AllGather", ALU.bypass,
                    replica_groups=[[0, 1], [2, 3], [4, 5], [6, 7]],
                    ins=[srcA[nch][:].opt()], outs=[dstA[nch][:].opt()])
                nc.gpsimd.collective_compute(
                    "AllGather", ALU.bypass,
                    replica_groups=[[0, 1], [2, 3], [4, 5], [6, 7]],
                    ins=[srcB[nch][:].opt()], outs=[dstB[nch][:].opt()])
            ffnp.__exit__(None, None, None)

        # ============ STAGE B ============
        with tc.tile_pool(name="resB", bufs=1) as resB, \
             tc.tile_pool(name="whead", bufs=2) as whead, \
             tc.tile_pool(name="hb", bufs=2) as hb, \
             tc.tile_pool(name="workB", bufs=2) as workB, \
             tc.tile_pool(name="smB", bufs=4) as smB:

            ps6 = tc.tile_pool(name="psG2", bufs=2, space="PSUM")
            psG2 = ps6.__enter__()

            # mwB needed only for fc at the end; start the DMA early
            mwB8_sb = resB.tile([128, NH * C6, D], FP8, tag="mwB8")
            nc.sync.dma_start(mwB8_sb, mwB_d.ap())

            # --- pre-collective: q2 for all 4 heads from local oT8
            q2T8 = []
            for h in range(NH):
                wh8 = whead.tile([128, C6, D], FP8, tag="wh")
                nc.sync.dma_start(
                    wh8, mwA_d.ap()[:, h * C6 * D:(h + 1) * C6 * D])
                q2 = resB.tile([128, C6, T], FP8, tag=f"q2T8_{h}")
                for f in range(C6):
                    for nch in range(2):
                        ps = psG2.tile([128, 512], F32, tag="g2")
                        for j in range(3):
                            nc.tensor.matmul(
                                ps,
                                wh8[:, 2 * j:2 * j + 2,
                                    f * 128:(f + 1) * 128],
                                oT8[:, 2 * j:2 * j + 2,
                                    nch * 512:(nch + 1) * 512],
                                start=(j == 0), stop=(j == 2), perf_mode=DR)
                        nc.scalar.activation(
                            out=q2[:, f, nch * 512:(nch + 1) * 512],
                            in_=ps, func=ACTF.Copy, scale=DQ_Q2)
                q2T8.append(q2)

            # --- land collective results (no transposes needed)
            # global token chunk tt: half g = tt//8, sub-half s = (tt%8)//4
            olfT8 = resB.tile([128, C6, 2 * T], FP8, tag="olfT8")
            olftok8 = resB.tile([128, 16, 784], FP8, tag="olftok8")
            nc.vector.memset(olftok8[:, :, 768:769], ZCOL)
            for g in range(2):
                for s in range(2):
                    nc.sync.dma_start(
                        olfT8[:, :, g * T + s * 512:g * T + (s + 1) * 512],
                        dstB[s][g * 128:(g + 1) * 128, :])
            for tt in range(16):
                g, s, r = tt // 8, (tt % 8) // 4, tt % 4
                nc.sync.dma_start(
                    olftok8[:, tt, 0:768],
                    dstA[s][g * 512 + r * 128:g * 512 + (r + 1) * 128, :])

            poT8 = resB.tile([128, NH * C6, T], FP8, tag="poT8")
            fcacc = resB.tile([128, C6, T], BF16, tag="fcacc")

            ps7z = tc.tile_pool(name="psZ", bufs=2, space="PSUM")
            psZ = ps7z.__enter__()
            ps7b = tc.tile_pool(name="psBC", bufs=2, space="PSUM")
            psBC = ps7b.__enter__()
            ps7 = tc.tile_pool(name="psPV", bufs=2, space="PSUM")
            psPV = ps7.__enter__()

            def sc_issue(h, qch):
                expT8 = hb.tile([128, 16, 512], FP8, tag="expT8")
                for kt in range(16):
                    ps = psG2.tile([128, 512], F32, tag="g2")
                    for j in range(3):
                        nc.tensor.matmul(
                            ps,
                            olfT8[:, 2 * j:2 * j + 2,
                                  kt * 128:(kt + 1) * 128],
                            q2T8[h][:, 2 * j:2 * j + 2,
                                    qch * 512:(qch + 1) * 512],
                            start=(j == 0), stop=(j == 2), perf_mode=DR)
                    nc.scalar.activation(out=expT8[:, kt, :], in_=ps,
                                         func=ACTF.Exp,
                                         bias=kb16[:, kt:kt + 1],
                                         scale=ISQ_DK / 256.0)
                return expT8

            def pv_issue(h, qch, expT8):
                zp = psZ.tile([1, 512], F32, tag="z")
                for i in range(8):
                    nc.tensor.matmul(
                        zp, olftok8[:, 2 * i:2 * i + 2, 768:769],
                        expT8[:, 2 * i:2 * i + 2, :],
                        start=(i == 0), stop=(i == 7), perf_mode=DR)
                rs = smB.tile([1, 512], BF16, tag="rs2")
                with nc.allow_low_precision(
                        reason="1/Z feeds fp8-precision normalization"):
                    nc.vector.reciprocal(rs, zp)
                bc = psBC.tile([128, 512], F32, tag="bc")
                bc_sb = workB.tile([128, 512], BF16, tag="bc_sb")
                for c in range(C6):
                    pp = psPV.tile([128, 512], F32, tag="pv")
                    for i in range(8):
                        nc.tensor.matmul(
                            pp,
                            olftok8[:, 2 * i:2 * i + 2,
                                    c * 128:(c + 1) * 128],
                            expT8[:, 2 * i:2 * i + 2, :],
                            start=(i == 0), stop=(i == 7), perf_mode=DR)
                    if c == 0:
                        nc.tensor.matmul(bc, ones_bf, rs,
                                         start=True, stop=True)
                        nc.scalar.activation(out=bc_sb, in_=bc,
                                             func=ACTF.Copy)
                    nc.vector.tensor_tensor(
                        poT8[:, h * C6 + c, qch * 512:(qch + 1) * 512],
                        pp, bc_sb, ALU.mult)

            # head loop: sc issued one step ahead of pv
            prev = None
            for h in range(NH):
                for qch in range(2):
                    expT8 = sc_issue(h, qch)
                    if prev is not None:
                        pv_issue(*prev)
                    prev = (h, qch, expT8)
            pv_issue(*prev)

            ps7.__exit__(None, None, None)
            ps7b.__exit__(None, None, None)
            ps7z.__exit__(None, None, None)

            # --- output projection over concatenated heads + residual,
            # interleaved with the tail LN/max per token-half so the
            # serial LN chain overlaps the second half's fc matmuls
            ps8 = tc.tile_pool(name="psT2", bufs=3, space="PSUM")
            psT2 = ps8.__enter__()
            ps9 = tc.tile_pool(name="psTail", bufs=2, space="PSUM")
            psTail = ps9.__enter__()
            maxacc = resB.tile([128, D], F32, tag="maxacc")

            def tail_t(t):
                rtok = workB.tile([128, D], BF16, tag="rtokB")
                tp = psT2.tile([128, C6, 128], BF16, tag="tpB")
                for c in range(C6):
                    nc.tensor.transpose(tp[:, c, :],
                                        fcacc[:, c, t * 128:(t + 1) * 128],
                                        id_bf)
                nc.any.tensor_copy(out=rtok, in_=tp)
                ltok = workB.tile([128, D], F32, tag="ltokB")
                _ln_tile(nc, smB, rtok, ltok, eps_sb)
                if "tap_attn" in taps:
                    nc.sync.dma_start(
                        taps["tap_attn"].ap()[t * 128:(t + 1) * 128, :], ltok)
                if t == 0:
                    nc.vector.tensor_copy(out=maxacc, in_=ltok)
                else:
                    nc.vector.tensor_tensor(maxacc, maxacc, ltok, ALU.max)

            for nch in range(2):
                for m in range(C6):
                    ps = psG2.tile([128, 512], F32, tag="g2")
                    for j in range(NH * C6 // 2):
                        nc.tensor.matmul(
                            ps,
                            mwB8_sb[:, 2 * j:2 * j + 2,
                                    m * 128:(m + 1) * 128],
                            poT8[:, 2 * j:2 * j + 2,
                                 nch * 512:(nch + 1) * 512],
                            start=(j == 0), stop=(j == NH * C6 // 2 - 1),
                            perf_mode=DR)
                    t1 = workB.tile([128, 512], BF16, tag="t1")
                    nc.scalar.activation(out=t1, in_=ps, func=ACTF.Copy,
                                         scale=DQ_FC)
                    nc.vector.tensor_tensor(
                        fcacc[:, m, nch * 512:(nch + 1) * 512], t1,
                        ownT[:, m, nch * 512:(nch + 1) * 512], ALU.add)
                for t in range(nch * 4, nch * 4 + 4):
                    tail_t(t)
            outsb = resB.tile([128, 6], F32, tag="outsb")
            for c in range(C6):
                pt = psTail.tile([128, 128], F32, tag="tpf")
                nc.tensor.transpose(pt, maxacc[:, c * 128:(c + 1) * 128],
                                    id_f32)
                nc.vector.tensor_reduce(out=outsb[:, c:c + 1], in_=pt,
                                        axis=AX.X, op=ALU.max)
            nc.sync.dma_start(out_d.ap(), outsb)
            ps9.__exit__(None, None, None)
            ps8.__exit__(None, None, None)
            ps6.__exit__(None, None, None)

    return


# ---------------- host side ----------------

_NC_CACHE = {}


def _get_nc(debug=False):
    key = bool(debug)
    if key not in _NC_CACHE:
        _NC_CACHE[key] = build(debug=debug)
    return _NC_CACHE[key]


def _prep_in_maps(inputs):
    bf = ml_dtypes.bfloat16
    f8 = ml_dtypes.float8_e4m3
    x = np.asarray(inputs["x"])
    emb = np.asarray(inputs["emb"], np.float32)
    pos = np.asarray(inputs["pos"], np.float32)
    g_e = np.asarray(inputs["ln_e_g"], np.float32)
    b_e = np.asarray(inputs["ln_e_b"], np.float32)

    def parr(w):
        w = np.ascontiguousarray(
            np.asarray(w, np.float32).reshape(C6, 128, -1)
            .transpose(1, 0, 2)).astype(bf)
        return w.reshape(128, -1)

    def parr8(w, scale):
        w = np.clip(np.asarray(w, np.float32) * scale, -240.0, 240.0)
        w = np.ascontiguousarray(
            w.reshape(-1, 128, w.shape[-1]).transpose(1, 0, 2)).astype(f8)
        return w.reshape(128, -1)

    wts = {
        "lfwq": parr8(inputs["lf_wq"], SC_LFW),
        "lfwk": parr8(inputs["lf_wk"], SC_LFW),
        "lfwv": parr8(inputs["lf_wv"], SC_LFW),
        "lfwo": parr8(inputs["lf_wo"], SC_LFW),
    }
    w1 = np.asarray(inputs["w1"], np.float32)
    wts["w1"] = np.ascontiguousarray(
        w1.reshape(C6, 128, KC, 128).transpose(1, 2, 0, 3)
    ).astype(bf).reshape(128, KC * D)
    wts["w2"] = np.asarray(inputs["w2"], np.float32).astype(bf)

    # folded stage-B matrices, fp8 x256
    wq = np.asarray(inputs["mha_wq"], np.float32).reshape(D, NH, DK)
    wk = np.asarray(inputs["mha_wk"], np.float32).reshape(D, NH, DK)
    wv = np.asarray(inputs["mha_wv"], np.float32).reshape(D, NH, DK)
    fc = np.asarray(inputs["mha_fc"], np.float32).reshape(NH, DK, D)
    mwA = np.concatenate(
        [parr8(wq[:, h, :] @ wk[:, h, :].T, SC_W) for h in range(NH)], axis=1)
    Bcat = np.concatenate([wv[:, h, :] @ fc[h] for h in range(NH)], axis=0)
    wts["mwA"] = np.ascontiguousarray(mwA)
    wts["mwB"] = np.ascontiguousarray(parr8(Bcat, SC_W))

    in_maps = []
    for b in range(B):
        h0 = emb[x[b]] + pos                        # [S, D] f32
        mu = h0.mean(-1, keepdims=True)
        var = h0.var(-1, keepdims=True)
        hn = (h0 - mu) / np.sqrt(var + EPS) * g_e + b_e
        kbias = np.where(x[b] != 0, 0.0, NEG).astype(np.float32)
        kb16 = np.ascontiguousarray(kbias.reshape(16, 128).T)
        for p in range(2):
            start = p * T - 256
            hxe = np.zeros((EXT, D), np.float32)
            lo, hi = max(0, start), min(S, start + EXT)
            hxe[lo - start: hi - start] = hn[lo:hi]
            hxf = np.ascontiguousarray(
                hxe.reshape(EXT, C6, 128).transpose(2, 1, 0))
            hxT = hxf.astype(bf).reshape(128, C6 * EXT)
            hx8 = np.clip(hxf * SC_HX, -240.0, 240.0).astype(f8).reshape(
                128, C6 * EXT)

            qi = np.arange(128)
            kj = np.arange(640)
            m640 = np.zeros((8, 128, 640), np.float32)
            for qt in range(8):
                qg = p * T + qt * 128 + qi[:, None]
                kg = start + qt * 128 + kj[None, :]
                ok = (np.abs(kg - qg) <= W) & (kg >= 0) & (kg < S)
                # k-major: [key-in-chunk, dx-chunk, query]
                m640[qt] = np.ascontiguousarray(
                    np.where(ok, 0.0, NEG).T.reshape(5, 128, 128)
                    .transpose(1, 0, 2)).reshape(128, 640)

            m = {"hxT": hxT, "hx8": hx8, "m640": m640, "kb16": kb16}
            m.update(wts)
            in_maps.append(m)
    return in_maps


def _postprocess(results):
    out = np.zeros((B, D), np.float32)
    for b in range(B):
        m0 = np.asarray(results[2 * b]["out"]).T.reshape(D)
        m1 = np.asarray(results[2 * b + 1]["out"]).T.reshape(D)
        out[b] = np.maximum(m0, m1)
    return out


def run(inputs, debug=False, trace=False):
    nc = _get_nc(debug=debug)
    in_maps = _prep_in_maps(inputs)
    res = run_bass_kernel_spmd(nc, in_maps, core_ids=list(range(NCORES)),
                               trace=trace)
    return res


def kernel(**inputs):
    res = run(inputs, debug=False, trace=False)
    return _postprocess(res.results)


# revision 36
# speedup vs baseline: 1.4175x; 1.0136x over previous
"""Trainium2 Bass kernel for nn_LongformerEncoder (optimized v3).

Sharding: 8 cores = (batch b in 0..3, seq-half p in 0..1).
Stage A (longformer layer) runs on 1024 own tokens (+256-token halo).
A pairwise AllGather exchanges stage-A output; stage B (4-head/768-dim
MHA + max-pool) runs seq-split on queries with full keys, partial max
per core, final max across the pair on host.

v3 changes vs v2:
- Stage B entirely in fp8 (e4m3) with DoubleRow matmuls (2 contraction
  rows per PE pass): q2 projection, scores, PV, and the concatenated
  output projection. Attention contributes ~1.3% of the pre-LN signal,
  so fp8 error is negligible in the final output.
- The collective payload carries the stage-A output in fp8 in BOTH
  layouts (token-major for PV values, feature-major for score keys),
  eliminating all post-collective PE transposes in stage B.
- PV computed feature-major (lhsT = values chunk), so the attention
  output lands pre-transposed for the output projection; softmax
  denominator via a dedicated Z-column matmul, normalization via a
  ones-broadcast matmul + one DVE multiply per chunk.
- fc done once over the 4 heads' concatenated poT (single PSUM
  accumulation group; no inter-head DVE adds).
"""

import sys

sys.path.insert(0, "/opt/trn_rl_repo")

import numpy as np
import ml_dtypes

import concourse.bass as bass
import concourse.tile as tile
from concourse import bacc, mybir
from concourse.bass_utils import run_bass_kernel_spmd
from concourse.masks import make_identity

F32 = mybir.dt.float32
BF16 = mybir.dt.bfloat16
FP8 = mybir.dt.float8e4
AX = mybir.AxisListType
ALU = mybir.AluOpType
ACTF = mybir.ActivationFunctionType
DR = mybir.MatmulPerfMode.DoubleRow

B, S, D = 4, 2048, 768
W = 256
DFF = 3072
NH, DK = 4, 768
T = 1024            # own tokens per core
EXT = 1536          # own + 256 halo each side
NEG = -1e9
EPS = 1e-5
NCORES = 8
C6 = D // 128        # 6 feature chunks
KC = DFF // 128      # 24 dff chunks
ISQ_DH = 0.125       # 1/sqrt(64)
ISQ_DK = 1.0 / float(np.sqrt(DK))
SC_OLF = 16.0        # fp8 scale of stage-A output (both layouts)
SC_W = 256.0         # fp8 scale of folded stage-B weights
ZCOL = 1.0 / 16.0    # Z-helper column value so po lands at 256x true
DQ_Q2 = 1.0 / 256.0  # psum(16*256*q2) -> 16*q2
DQ_FC = 1.0 / 65536.0  # psum(256*256*fc) -> fc
SC_HX = 16.0         # fp8 scale of LN'd embeddings (stage-A input)
SC_LFW = 1024.0      # fp8 scale of longformer q/k/v/o weights
SC_AT = 32.0         # fp8 scale of stage-A attention output
DQ_QKV = 1.0 / (SC_HX * SC_LFW)
DQ_WO = 1.0 / (SC_AT * SC_LFW)


def build(debug=False):
    nc = bacc.Bacc("TRN2", target_bir_lowering=False, debug=False,
                   num_devices=NCORES)

    hxT_d = nc.dram_tensor("hxT", [128, C6 * EXT], BF16, kind="ExternalInput")
    hx8_d = nc.dram_tensor("hx8", [128, C6 * EXT], FP8, kind="ExternalInput")
    m640_d = nc.dram_tensor("m640", [8, 128, 640], F32, kind="ExternalInput")
    kb16_d = nc.dram_tensor("kb16", [128, 16], F32, kind="ExternalInput")
    lfw_d = {}
    for nm in ["lfwq", "lfwk", "lfwv", "lfwo"]:
        lfw_d[nm] = nc.dram_tensor(nm, [128, C6 * D], FP8,
                                   kind="ExternalInput")
    w1_d = nc.dram_tensor("w1", [128, KC * D], BF16, kind="ExternalInput")
    w2_d = nc.dram_tensor("w2", [DFF, D], BF16, kind="ExternalInput")
    mwA_d = nc.dram_tensor("mwA", [128, NH * C6 * D], FP8,
                           kind="ExternalInput")
    mwB_d = nc.dram_tensor("mwB", [128, NH * C6 * D], FP8,
                           kind="ExternalInput")
    out_d = nc.dram_tensor("out", [128, 6], F32, kind="ExternalOutput")
    taps = {}
    if debug:
        taps["tap_olf"] = nc.dram_tensor("tap_olf", [T, D], F32,
                                         kind="ExternalOutput")
        taps["tap_attn"] = nc.dram_tensor("tap_attn", [T, D], F32,
                                          kind="ExternalOutput")

    with tile.TileContext(nc) as tc:
        _body(nc, tc, hxT_d, hx8_d, m640_d, kb16_d, lfw_d, w1_d, w2_d,
              mwA_d, mwB_d, out_d, taps)
    nc.compile()
    return nc


def _ln_tile(nc, pool, x_ap, out_tile, eps_ap):
    """out = (x - mean)/sqrt(var+eps) over free dim (768). g==1, b==0.

    The wide normalize runs on the scalar engine (out = x*rstd - mu*rstd)
    so the DVE only carries the stats chain.
    """
    stats = pool.tile([128, 2, 6], F32, tag="lnstats")
    nc.vector.bn_stats(out=stats[:, 0, :], in_=x_ap[:, 0:384])
    nc.vector.bn_stats(out=stats[:, 1, :], in_=x_ap[:, 384:768])
    mv = pool.tile([128, 2], F32, tag="lnmv")
    nc.vector.bn_aggr(out=mv, in_=stats)
    rstd = pool.tile([128, 1], F32, tag="lnrstd")
    nc.scalar.activation(out=rstd, in_=mv[:, 1:2], func=ACTF.Sqrt, bias=eps_ap)
    nc.vector.reciprocal(out=rstd, in_=rstd)
    nb = pool.tile([128, 1], F32, tag="lnnb")
    nc.vector.tensor_scalar(out=nb, in0=mv[:, 0:1], scalar1=rstd,
                            scalar2=-1.0, op0=ALU.mult, op1=ALU.mult)
    nc.scalar.activation(out=out_tile, in_=x_ap, func=ACTF.Identity,
                         scale=rstd, bias=nb)


def _body(nc, tc, hxT_d, hx8_d, m640_d, kb16_d, lfw_d, w1_d, w2_d,
          mwA_d, mwB_d, out_d, taps):
    import contextlib
    ctx = contextlib.ExitStack()
    with ctx:
        constg = ctx.enter_context(tc.tile_pool(name="constg", bufs=1))
        outer = ctx.enter_context(tc.tile_pool(name="outer", bufs=1))
        dram = ctx.enter_context(tc.tile_pool(name="dram", bufs=1,
                                              space="DRAM"))

        id_bf = constg.tile([128, 128], BF16, tag="id_bf")
        make_identity(nc, id_bf)
        id_f32 = constg.tile([128, 128], F32, tag="id_f32")
        make_identity(nc, id_f32)
        eps_sb = constg.tile([128, 1], F32, tag="eps")
        nc.vector.memset(eps_sb, EPS)
        kb16 = constg.tile([128, 16], F32, tag="kb16")
        nc.sync.dma_start(kb16, kb16_d.ap())
        ones_bf = constg.tile([1, 128], BF16, tag="ones_bf")
        nc.vector.memset(ones_bf, 1.0)

        # cross-stage tiles (stage-A output for stage B)
        ownT = outer.tile([128, C6, T], BF16, tag="ownT")    # 12K/part
        oT8 = outer.tile([128, C6, T], FP8, tag="oT8")       # 6K/part

        # DRAM bounce for the collectives (fp8, both layouts, split in
        # two token-halves so the first exchange overlaps the second
        # half's FFN)
        srcA = [dram.tile([T // 2, D], FP8, name=f"srcA{i}")
                for i in range(2)]
        dstA = [dram.tile([T, D], FP8, name=f"dstA{i}") for i in range(2)]
        srcB = [dram.tile([128, C6 * 512], FP8, name=f"srcB{i}")
                for i in range(2)]
        dstB = [dram.tile([256, C6 * 512], FP8, name=f"dstB{i}")
                for i in range(2)]

        # ============ STAGE A ============
        with tc.tile_pool(name="mid", bufs=1) as mid, \
             tc.tile_pool(name="sm", bufs=4) as sm, \
             tc.tile_pool(name="work", bufs=4) as work:

            with tc.tile_pool(name="inA", bufs=1) as inA, \
                 tc.tile_pool(name="attA2", bufs=1) as attA2, \
                 tc.tile_pool(name="lfw", bufs=2) as lfw:

                hx8 = inA.tile([128, C6, EXT], FP8, tag="hx8")
                for j in range(3):
                    nc.sync.dma_start(
                        hx8[:, 2 * j:2 * j + 2, :],
                        hx8_d.ap()[:, 2 * j * EXT:(2 * j + 2) * EXT])
                hxT = inA.tile([128, C6, EXT], BF16, tag="hxT")
                aT8 = attA2.tile([128, C6, T], FP8, tag="aT8")

                with tc.tile_pool(name="attA1", bufs=1) as attA1:
                    # ---- q/k feature-major, v token-major (fp8 DoubleRow)
                    ps1 = tc.tile_pool(name="ps1", bufs=2, space="PSUM")
                    psG = ps1.__enter__()
                    wq_sb = lfw.tile([128, C6, D], FP8, tag="lfw")
                    nc.sync.dma_start(wq_sb, lfw_d["lfwq"].ap())
                    qT = attA1.tile([128, C6, T], BF16, tag="qT")
                    for f in range(C6):
                        for nch in range(2):
                            ps = psG.tile([128, 512], F32, tag="g")
                            for j in range(3):
                                nc.tensor.matmul(
                                    ps,
                                    wq_sb[:, 2 * j:2 * j + 2,
                                          f * 128:(f + 1) * 128],
                                    hx8[:, 2 * j:2 * j + 2,
                                        256 + nch * 512:
                                        256 + (nch + 1) * 512],
                                    start=(j == 0), stop=(j == 2),
                                    perf_mode=DR)
                            nc.scalar.activation(
                                out=qT[:, f, nch * 512:(nch + 1) * 512],
                                in_=ps, func=ACTF.Copy, scale=DQ_QKV)
                    wk_sb = lfw.tile([128, C6, D], FP8, tag="lfw")
                    nc.sync.dma_start(wk_sb, lfw_d["lfwk"].ap())
                    # hxT (bf16 residual) only needed at wo; load now
                    nc.sync.dma_start(hxT, hxT_d.ap())
                    kT = attA1.tile([128, C6, EXT], BF16, tag="kT")
                    for f in range(C6):
                        for nch in range(3):
                            ps = psG.tile([128, 512], F32, tag="g")
                            for j in range(3):
                                nc.tensor.matmul(
                                    ps,
                                    wk_sb[:, 2 * j:2 * j + 2,
                                          f * 128:(f + 1) * 128],
                                    hx8[:, 2 * j:2 * j + 2,
                                        nch * 512:(nch + 1) * 512],
                                    start=(j == 0), stop=(j == 2),
                                    perf_mode=DR)
                            nc.scalar.activation(
                                out=kT[:, f, nch * 512:(nch + 1) * 512],
                                in_=ps, func=ACTF.Copy, scale=DQ_QKV)
                    wv_sb = lfw.tile([128, C6, D], FP8, tag="lfw")
                    nc.sync.dma_start(wv_sb, lfw_d["lfwv"].ap())
                    # values token-major in fp8 at x32 (PV runs in fp8)
                    vtok8 = attA1.tile([128, 12, D], FP8, tag="vtok8")
                    for t in range(12):
                        for (n0, nn) in ((0, 512), (512, 256)):
                            ps = psG.tile([128, 512], F32, tag="g")
                            for j in range(3):
                                nc.tensor.matmul(
                                    ps[:, :nn],
                                    hx8[:, 2 * j:2 * j + 2,
                                        t * 128:(t + 1) * 128],
                                    wv_sb[:, 2 * j:2 * j + 2, n0:n0 + nn],
                                    start=(j == 0), stop=(j == 2),
                                    perf_mode=DR)
                            nc.scalar.activation(
                                out=vtok8[:, t, n0:n0 + nn], in_=ps[:, :nn],
                                func=ACTF.Copy, scale=DQ_QKV * SC_AT)
                    ps1.__exit__(None, None, None)

                    # ---- sliding-window attention, k-major scores so the
                    # probs land contraction-ready (no transpose matmuls);
                    # PV in fp8 DoubleRow; softmax Z via a ones-row matmul,
                    # normalization via GPSIMD partition-broadcast + DVE.
                    ps2 = tc.tile_pool(name="ps2", bufs=2, space="PSUM")
                    psS = ps2.__enter__()
                    ps2c = tc.tile_pool(name="ps2c", bufs=2, space="PSUM")
                    psV = ps2c.__enter__()
                    ps2z = tc.tile_pool(name="ps2z", bufs=2, space="PSUM")
                    psZ1 = ps2z.__enter__()

                    ones8 = constg.tile([128, 2, 16], FP8, tag="ones8")
                    nc.vector.memset(ones8, 1.0)

                    m640_t = [None] * 8

                    def a_scores(qt, pair, h2):
                        if pair == 0 and h2 == 0:
                            m640_t[qt] = work.tile([128, 640], F32,
                                                   tag="m640", name="m640")
                            nc.sync.dma_start(m640_t[qt], m640_d.ap()[qt])
                        ps = psS.tile([128, 640], F32, tag="sc")
                        rhsq = qT[h2 * 64:(h2 + 1) * 64, pair,
                                  qt * 128:(qt + 1) * 128]
                        for dx in range(5):
                            nc.tensor.matmul(
                                ps[:, dx * 128:(dx + 1) * 128],
                                kT[h2 * 64:(h2 + 1) * 64, pair,
                                   qt * 128 + dx * 128:
                                   qt * 128 + (dx + 1) * 128],
                                rhsq, start=True, stop=True,
                                tile_position=(h2 * 64, 0))
                        sb = work.tile([128, 640], F32, tag="sb")
                        nc.vector.tensor_tensor(sb, ps, m640_t[qt], ALU.add)
                        probs8 = work.tile([128, 5, 128], FP8, tag="probs8")
                        nc.scalar.activation(out=probs8, in_=sb,
                                             func=ACTF.Exp, scale=ISQ_DH)
                        return probs8

                    def a_rest(qt, pair, h2, probs8):
                        h = 2 * pair + h2
                        zq = psZ1.tile([1, 128], F32, tag="zq")
                        pvt = psV.tile([128, 128], F32, tag="pv")
                        for i in range(2):
                            nc.tensor.matmul(
                                zq, ones8[:, :, 0:1],
                                probs8[:, 2 * i:2 * i + 2, :],
                                start=(i == 0), stop=False, perf_mode=DR)
                        nc.tensor.matmul(zq, ones8[:, 0, 0:1],
                                         probs8[:, 4, :],
                                         start=False, stop=True)
                        if h2 == 0:
                            # DoubleRow requires dst partition offset 0
                            for i in range(2):
                                nc.tensor.matmul(
                                    pvt[0:64, :],
                                    vtok8[:, qt + 2 * i:qt + 2 * i + 2,
                                          h * 64:(h + 1) * 64],
                                    probs8[:, 2 * i:2 * i + 2, :],
                                    start=(i == 0), stop=False, perf_mode=DR,
                                    tile_position=(0, 0))
                            nc.tensor.matmul(
                                pvt[0:64, :],
                                vtok8[:, qt + 4, h * 64:(h + 1) * 64],
                                probs8[:, 4, :], start=False, stop=True,
                                tile_position=(0, 0))
                        else:
                            for dx in range(5):
                                nc.tensor.matmul(
                                    pvt[64:128, :],
                                    vtok8[:, qt + dx, h * 64:(h + 1) * 64],
                                    probs8[:, dx, :], start=(dx == 0),
                                    stop=(dx == 4),
                                    tile_position=(0, 64))
                        rs = sm.tile([1, 128], F32, tag="rs")
                        nc.vector.reciprocal(rs, zq)
                        rs_bc = work.tile([128, 128], F32, tag="rs_bc")
                        nc.gpsimd.partition_broadcast(rs_bc, rs)
                        nc.vector.tensor_tensor(
                            aT8[h2 * 64:(h2 + 1) * 64, pair,
                                qt * 128:(qt + 1) * 128],
                            pvt[h2 * 64:(h2 + 1) * 64, :],
                            rs_bc[h2 * 64:(h2 + 1) * 64, :], ALU.mult)

                    its = [(qt, pair, h2) for qt in range(8)
                           for pair in range(6) for h2 in range(2)]
                    prev = None
                    for it in its:
                        probs8 = a_scores(*it)
                        if prev is not None:
                            a_rest(prev[0][0], prev[0][1], prev[0][2],
                                   prev[1])
                        prev = (it, probs8)
                    a_rest(prev[0][0], prev[0][1], prev[0][2], prev[1])

                    ps2z.__exit__(None, None, None)
                    ps2c.__exit__(None, None, None)
                    ps2.__exit__(None, None, None)

                # ---- wo + residual (feature-major, fp8 DoubleRow)
                ps3 = tc.tile_pool(name="ps3", bufs=2, space="PSUM")
                psG = ps3.__enter__()
                wo_sb = lfw.tile([128, C6, D], FP8, tag="lfw")
                nc.sync.dma_start(wo_sb, lfw_d["lfwo"].ap())
                r1T = mid.tile([128, C6, T], BF16, tag="resT")
                for f in range(C6):
                    for nch in range(2):
                        ps = psG.tile([128, 512], F32, tag="g")
                        for j in range(3):
                            nc.tensor.matmul(
                                ps,
                                wo_sb[:, 2 * j:2 * j + 2,
                                      f * 128:(f + 1) * 128],
                                aT8[:, 2 * j:2 * j + 2,
                                    nch * 512:(nch + 1) * 512],
                                start=(j == 0), stop=(j == 2), perf_mode=DR)
                        t0 = work.tile([128, 512], BF16, tag="t0")
                        nc.scalar.activation(out=t0, in_=ps, func=ACTF.Copy,
                                             scale=DQ_WO)
                        nc.vector.tensor_tensor(
                            r1T[:, f, nch * 512:(nch + 1) * 512], t0,
                            hxT[:, f, 256 + nch * 512: 256 + (nch + 1) * 512],
                            ALU.add)
                ps3.__exit__(None, None, None)

            # FFN weights: w1 stays resident across both FFN passes, in a
            # pool that reuses the space just freed by the attention pools.
            ffnp = tc.tile_pool(name="ffnp", bufs=1)
            ffnpo = ffnp.__enter__()
            w1sb = ffnpo.tile([128, KC, C6, 128], BF16, tag="w1sb")
            nc.sync.dma_start(w1sb, w1_d.ap())

            # ---- LN1 (transpose to token-major, LN, transpose back)
            # software-pipelined: forward transposes of t+1 issue before the
            # back transposes of t, so the PE isn't stalled by the LN chain
            ps3b = tc.tile_pool(name="ps3b", bufs=4, space="PSUM")
            psT = ps3b.__enter__()
            h1T = mid.tile([128, C6, T], BF16, tag="h1T")

            def ln1_fwd(t):
                rtok = work.tile([128, D], BF16, tag="rtok")
                tp = psT.tile([128, C6, 128], BF16, tag="tp3")
                for c in range(C6):
                    nc.tensor.transpose(tp[:, c, :],
                                        r1T[:, c, t * 128:(t + 1) * 128],
                                        id_bf)
                nc.any.tensor_copy(out=rtok, in_=tp)
                ltok = work.tile([128, D], BF16, tag="ltok")
                _ln_tile(nc, sm, rtok, ltok, eps_sb)
                return ltok

            def ln1_back(t, ltok):
                tp2 = psT.tile([128, C6, 128], BF16, tag="tp3")
                for c in range(C6):
                    nc.tensor.transpose(tp2[:, c, :],
                                        ltok[:, c * 128:(c + 1) * 128],
                                        id_bf)
                nc.any.tensor_copy(out=h1T[:, :, t * 128:(t + 1) * 128],
                                   in_=tp2)

            prevLs = []
            for t in range(8):
                ltok = ln1_fwd(t)
                prevLs.append((t, ltok))
                if len(prevLs) > 2:
                    ln1_back(*prevLs.pop(0))
            for pl in prevLs:
                ln1_back(*pl)
            ps3b.__exit__(None, None, None)

            # ---- FFN + LN2, one token-half at a time; each half's fp8
            # payload is exchanged as soon as it is ready so the second
            # half's FFN overlaps the first collective.
            r2T = mid.tile([128, C6, T], BF16, tag="resT")
            g1all = ffnpo.tile([128, KC, 512], BF16, tag="g1all")
            for nch in range(2):
                # g1 = gelu(h1 @ w1), all 24 dff chunks
                ps4 = tc.tile_pool(name="ps4", bufs=2, space="PSUM")
                psG = ps4.__enter__()
                for kc in range(KC):
                    g1p = psG.tile([128, 512], F32, tag="g")
                    for k in range(C6):
                        nc.tensor.matmul(
                            g1p, w1sb[:, kc, k, :],
                            h1T[:, k, nch * 512:(nch + 1) * 512],
                            start=(k == 0), stop=(k == 5))
                    nc.scalar.activation(out=g1all[:, kc, :], in_=g1p,
                                         func=ACTF.Gelu_apprx_tanh)
                ps4.__exit__(None, None, None)
                # f2 = g1 @ w2 (+ residual)
                ps4b = tc.tile_pool(name="ps4b", bufs=1, space="PSUM")
                psF = ps4b.__enter__()
                with tc.tile_pool(name="bigw", bufs=4) as bigw:
                    f2ps = [psF.tile([128, 512], F32, tag=f"f2_{m}",
                                     name=f"f2_{m}") for m in range(C6)]
                    for kc in range(KC):
                        w2c = bigw.tile([128, D], BF16, tag="w2c")
                        nc.sync.dma_start(
                            w2c, w2_d.ap()[kc * 128:(kc + 1) * 128, :])
                        for m in range(C6):
                            nc.tensor.matmul(
                                f2ps[m], w2c[:, m * 128:(m + 1) * 128],
                                g1all[:, kc, :],
                                start=(kc == 0), stop=(kc == KC - 1))
                    for m in range(C6):
                        nc.vector.tensor_tensor(
                            r2T[:, m, nch * 512:(nch + 1) * 512], f2ps[m],
                            h1T[:, m, nch * 512:(nch + 1) * 512], ALU.add)
                ps4b.__exit__(None, None, None)

                # LN2 for this half (pipelined like LN1)
                ps5 = tc.tile_pool(name="ps5", bufs=4, space="PSUM")
                psT5 = ps5.__enter__()

                def ln2_fwd(t):
                    rtok = work.tile([128, D], BF16, tag="rtok")
                    tp = psT5.tile([128, C6, 128], BF16, tag="tp5")
                    for c in range(C6):
                        nc.tensor.transpose(tp[:, c, :],
                                            r2T[:, c, t * 128:(t + 1) * 128],
                                            id_bf)
                    nc.any.tensor_copy(out=rtok, in_=tp)
                    otok = work.tile([128, D], BF16, tag="ltok")
                    _ln_tile(nc, sm, rtok, otok, eps_sb)
                    return otok

                def ln2_back(t, otok):
                    otok8 = work.tile([128, D], FP8, tag="otok8")
                    nc.scalar.activation(out=otok8, in_=otok, func=ACTF.Copy,
                                         scale=SC_OLF)
                    nc.sync.dma_start(
                        srcA[t // 4][(t % 4) * 128:(t % 4 + 1) * 128, :],
                        otok8)
                    tp2 = psT5.tile([128, C6, 128], BF16, tag="tp5")
                    for c in range(C6):
                        nc.tensor.transpose(tp2[:, c, :],
                                            otok[:, c * 128:(c + 1) * 128],
                                            id_bf)
                    nc.any.tensor_copy(
                        out=ownT[:, :, t * 128:(t + 1) * 128], in_=tp2)
                    nc.scalar.activation(
                        out=oT8[:, :, t * 128:(t + 1) * 128], in_=tp2,
                        func=ACTF.Copy, scale=SC_OLF)
                    if "tap_olf" in taps:
                        of = work.tile([128, D], F32, tag="tapolf")
                        nc.vector.tensor_copy(out=of, in_=otok)
                        nc.sync.dma_start(
                            taps["tap_olf"].ap()[t * 128:(t + 1) * 128, :],
                            of)

                prevTs = []
                for t in range(nch * 4, nch * 4 + 4):
                    otok = ln2_fwd(t)
                    prevTs.append((t, otok))
                    if len(prevTs) > 2:
                        ln2_back(*prevTs.pop(0))
                for pt in prevTs:
                    ln2_back(*pt)
                nc.sync.dma_start(
                    srcB[nch], oT8[:, :, nch * 512:(nch + 1) * 512])
                ps5.__exit__(None, None, None)

                # exchange this half right away
                nc.gpsimd.collective_compute(
                    "AllGather", ALU.bypass,
                    replica_groups=[[0, 1], [2, 3], [4, 5], [6, 7]],
                    ins=[srcA[nch][:].opt()], outs=[dstA[nch][:].opt()])
                nc.gpsimd.collective_compute(
                    "AllGather", ALU.bypass,
                    replica_groups=[[0, 1], [2, 3], [4, 5], [6, 7]],
                    ins=[srcB[nch][:].opt()], outs=[dstB[nch][:].opt()])
            ffnp.__exit__(None, None, None)

        # ============ STAGE B ============
        with tc.tile_pool(name="resB", bufs=1) as resB, \
             tc.tile_pool(name="whead", bufs=2) as whead, \
             tc.tile_pool(name="hb", bufs=2) as hb, \
             tc.tile_pool(name="workB", bufs=2) as workB, \
             tc.tile_pool(name="smB", bufs=4) as smB:

            ps6 = tc.tile_pool(name="psG2", bufs=2, space="PSUM")
            psG2 = ps6.__enter__()

            # mwB needed only for fc at the end; start the DMA early
            mwB8_sb = resB.tile([128, NH * C6, D], FP8, tag="mwB8")
            nc.sync.dma_start(mwB8_sb, mwB_d.ap())

            # --- pre-collective: q2 for all 4 heads from local oT8
            q2T8 = []
            for h in range(NH):
                wh8 = whead.tile([128, C6, D], FP8, tag="wh")
                nc.sync.dma_start(
                    wh8, mwA_d.ap()[:, h * C6 * D:(h + 1) * C6 * D])
                q2 = resB.tile([128, C6, T], FP8, tag=f"q2T8_{h}")
                for f in range(C6):
                    for nch in range(2):
                        ps = psG2.tile([128, 512], F32, tag="g2")
                        for j in range(3):
                            nc.tensor.matmul(
                                ps,
                                wh8[:, 2 * j:2 * j + 2,
                                    f * 128:(f + 1) * 128],
                                oT8[:, 2 * j:2 * j + 2,
                                    nch * 512:(nch + 1) * 512],
                                start=(j == 0), stop=(j == 2), perf_mode=DR)
                        nc.scalar.activation(
                            out=q2[:, f, nch * 512:(nch + 1) * 512],
                            in_=ps, func=ACTF.Copy, scale=DQ_Q2)
                q2T8.append(q2)

            # --- land collective results (no transposes needed)
            # global token chunk tt: half g = tt//8, sub-half s = (tt%8)//4
            olfT8 = resB.tile([128, C6, 2 * T], FP8, tag="olfT8")
            olftok8 = resB.tile([128, 16, 784], FP8, tag="olftok8")
            nc.vector.memset(olftok8[:, :, 768:769], ZCOL)
            for g in range(2):
                for s in range(2):
                    nc.sync.dma_start(
                        olfT8[:, :, g * T + s * 512:g * T + (s + 1) * 512],
                        dstB[s][g * 128:(g + 1) * 128, :])
            for tt in range(16):
                g, s, r = tt // 8, (tt % 8) // 4, tt % 4
                nc.sync.dma_start(
                    olftok8[:, tt, 0:768],
                    dstA[s][g * 512 + r * 128:g * 512 + (r + 1) * 128, :])

            poT8 = resB.tile([128, NH * C6, T], FP8, tag="poT8")
            fcacc = resB.tile([128, C6, T], BF16, tag="fcacc")

            ps7z = tc.tile_pool(name="psZ", bufs=2, space="PSUM")
            psZ = ps7z.__enter__()
            ps7b = tc.tile_pool(name="psBC", bufs=2, space="PSUM")
            psBC = ps7b.__enter__()
            ps7 = tc.tile_pool(name="psPV", bufs=2, space="PSUM")
            psPV = ps7.__enter__()

            def sc_issue(h, qch):
                expT8 = hb.tile([128, 16, 512], FP8, tag="expT8")
                for kt in range(16):
                    ps = psG2.tile([128, 512], F32, tag="g2")
                    for j in range(3):
                        nc.tensor.matmul(
                            ps,
                            olfT8[:, 2 * j:2 * j + 2,
                                  kt * 128:(kt + 1) * 128],
                            q2T8[h][:, 2 * j:2 * j + 2,
                                    qch * 512:(qch + 1) * 512],
                            start=(j == 0), stop=(j == 2), perf_mode=DR)
                    nc.scalar.activation(out=expT8[:, kt, :], in_=ps,
                                         func=ACTF.Exp,
                                         bias=kb16[:, kt:kt + 1],
                                         scale=ISQ_DK / 256.0)
                return expT8

            def pv_issue(h, qch, expT8):
                zp = psZ.tile([1, 512], F32, tag="z")
                for i in range(8):
                    nc.tensor.matmul(
                        zp, olftok8[:, 2 * i:2 * i + 2, 768:769],
                        expT8[:, 2 * i:2 * i + 2, :],
                        start=(i == 0), stop=(i == 7), perf_mode=DR)
                rs = smB.tile([1, 512], BF16, tag="rs2")
                with nc.allow_low_precision(
                        reason="1/Z feeds fp8-precision normalization"):
                    nc.vector.reciprocal(rs, zp)
                bc = psBC.tile([128, 512], F32, tag="bc")
                bc_sb = workB.tile([128, 512], BF16, tag="bc_sb")
                for c in range(C6):
                    pp = psPV.tile([128, 512], F32, tag="pv")
                    for i in range(8):
                        nc.tensor.matmul(
                            pp,
                            olftok8[:, 2 * i:2 * i + 2,
                                    c * 128:(c + 1) * 128],
                            expT8[:, 2 * i:2 * i + 2, :],
                            start=(i == 0), stop=(i == 7), perf_mode=DR)
                    if c == 0:
                        nc.tensor.matmul(bc, ones_bf, rs,
                                         start=True, stop=True)
                        nc.scalar.activation(out=bc_sb, in_=bc,
                                             func=ACTF.Copy)
                    nc.vector.tensor_tensor(
                        poT8[:, h * C6 + c, qch * 512:(qch + 1) * 512],
                        pp, bc_sb, ALU.mult)

            # head loop: sc issued one step ahead of pv
            prev = None
            for h in range(NH):
                for qch in range(2):
                    expT8 = sc_issue(h, qch)
                    if prev is not None:
                        pv_issue(*prev)
                    prev = (h, qch, expT8)
            pv_issue(*prev)

            ps7.__exit__(None, None, None)
            ps7b.__exit__(None, None, None)
            ps7z.__exit__(None, None, None)

            # --- output projection over concatenated heads + residual,
            # interleaved with the tail LN/max per token-half so the
            # serial LN chain overlaps the second half's fc matmuls
            ps8 = tc.tile_pool(name="psT2", bufs=3, space="PSUM")
            psT2 = ps8.__enter__()
            ps9 = tc.tile_pool(name="psTail", bufs=2, space="PSUM")
            psTail = ps9.__enter__()
            maxacc = resB.tile([128, D], F32, tag="maxacc")

            def tail_t(t):
                rtok = workB.tile([128, D], BF16, tag="rtokB")
                tp = psT2.tile([128, C6, 128], BF16, tag="tpB")
                for c in range(C6):
                    nc.tensor.transpose(tp[:, c, :],
                                        fcacc[:, c, t * 128:(t + 1) * 128],
                                        id_bf)
                nc.any.tensor_copy(out=rtok, in_=tp)
                ltok = workB.tile([128, D], F32, tag="ltokB")
                _ln_tile(nc, smB, rtok, ltok, eps_sb)
                if "tap_attn" in taps:
                    nc.sync.dma_start(
                        taps["tap_attn"].ap()[t * 128:(t + 1) * 128, :], ltok)
                if t == 0:
                    nc.vector.tensor_copy(out=maxacc, in_=ltok)
                else:
                    nc.vector.tensor_tensor(maxacc, maxacc, ltok, ALU.max)

            for nch in range(2):
                for m in range(C6):
                    ps = psG2.tile([128, 512], F32, tag="g2")
                    for j in range(NH * C6 // 2):
                        nc.tensor.matmul(
                            ps,
                            mwB8_sb[:, 2 * j:2 * j + 2,
                                    m * 128:(m + 1) * 128],
                            poT8[:, 2 * j:2 * j + 2,
                                 nch * 512:(nch + 1) * 512],
                            start=(j == 0), stop=(j == NH * C6 // 2 - 1),
                            perf_mode=DR)
                    t1 = workB.tile([128, 512], BF16, tag="t1")
                    nc.scalar.activation(out=t1, in_=ps, func=ACTF.Copy,
                                         scale=DQ_FC)
                    nc.vector.tensor_tensor(
                        fcacc[:, m, nch * 512:(nch + 1) * 512], t1,
                        ownT[:, m, nch * 512:(nch + 1) * 512], ALU.add)
                for t in range(nch * 4, nch * 4 + 4):
                    tail_t(t)
            outsb = resB.tile([128, 6], F32, tag="outsb")
            for c in range(C6):
                pt = psTail.tile([128, 128], F32, tag="tpf")
                nc.tensor.transpose(pt, maxacc[:, c * 128:(c + 1) * 128],
                                    id_f32)
                nc.vector.tensor_reduce(out=outsb[:, c:c + 1], in_=pt,
                                        axis=AX.X, op=ALU.max)
            nc.sync.dma_start(out_d.ap(), outsb)
            ps9.__exit__(None, None, None)
            ps8.__exit__(None, None, None)
            ps6.__exit__(None, None, None)

    return


# ---------------- host side ----------------

_NC_CACHE = {}


def _get_nc(debug=False):
    key = bool(debug)
    if key not in _NC_CACHE:
        _NC_CACHE[key] = build(debug=debug)
    return _NC_CACHE[key]


def _prep_in_maps(inputs):
    bf = ml_dtypes.bfloat16
    f8 = ml_dtypes.float8_e4m3
    x = np.asarray(inputs["x"])
    emb = np.asarray(inputs["emb"], np.float32)
    pos = np.asarray(inputs["pos"], np.float32)
    g_e = np.asarray(inputs["ln_e_g"], np.float32)
    b_e = np.asarray(inputs["ln_e_b"], np.float32)

    def parr(w):
        w = np.ascontiguousarray(
            np.asarray(w, np.float32).reshape(C6, 128, -1)
            .transpose(1, 0, 2)).astype(bf)
        return w.reshape(128, -1)

    def parr8(w, scale):
        w = np.clip(np.asarray(w, np.float32) * scale, -240.0, 240.0)
        w = np.ascontiguousarray(
            w.reshape(-1, 128, w.shape[-1]).transpose(1, 0, 2)).astype(f8)
        return w.reshape(128, -1)

    wts = {
        "lfwq": parr8(inputs["lf_wq"], SC_LFW),
        "lfwk": parr8(inputs["lf_wk"], SC_LFW),
        "lfwv": parr8(inputs["lf_wv"], SC_LFW),
        "lfwo": parr8(inputs["lf_wo"], SC_LFW),
    }
    w1 = np.asarray(inputs["w1"], np.float32)
    wts["w1"] = np.ascontiguousarray(
        w1.reshape(C6, 128, KC, 128).transpose(1, 2, 0, 3)
    ).astype(bf).reshape(128, KC * D)
    wts["w2"] = np.asarray(inputs["w2"], np.float32).astype(bf)

    # folded stage-B matrices, fp8 x256
    wq = np.asarray(inputs["mha_wq"], np.float32).reshape(D, NH, DK)
    wk = np.asarray(inputs["mha_wk"], np.float32).reshape(D, NH, DK)
    wv = np.asarray(inputs["mha_wv"], np.float32).reshape(D, NH, DK)
    fc = np.asarray(inputs["mha_fc"], np.float32).reshape(NH, DK, D)
    mwA = np.concatenate(
        [parr8(wq[:, h, :] @ wk[:, h, :].T, SC_W) for h in range(NH)], axis=1)
    Bcat = np.concatenate([wv[:, h, :] @ fc[h] for h in range(NH)], axis=0)
    wts["mwA"] = np.ascontiguousarray(mwA)
    wts["mwB"] = np.ascontiguousarray(parr8(Bcat, SC_W))

    in_maps = []
    for b in range(B):
        h0 = emb[x[b]] + pos                        # [S, D] f32
        mu = h0.mean(-1, keepdims=True)
        var = h0.var(-1, keepdims=True)
        hn = (h0 - mu) / np.sqrt(var + EPS) * g_e + b_e
        kbias = np.where(x[b] != 0, 0.0, NEG).astype(np.float32)
        kb16 = np.ascontiguousarray(kbias.reshape(16, 128).T)
        for p in range(2):
            start = p * T - 256
            hxe = np.zeros((EXT, D), np.float32)
            lo, hi = max(0, start), min(S, start + EXT)
            hxe[lo - start: hi - start] = hn[lo:hi]
            hxf = np.ascontiguousarray(
                hxe.reshape(EXT, C6, 128).transpose(2, 1, 0))
            hxT = hxf.astype(bf).reshape(128, C6 * EXT)
            hx8 = np.clip(hxf * SC_HX, -240.0, 240.0).astype(f8).reshape(
                128, C6 * EXT)

            qi = np.arange(128)
            kj = np.arange(640)
            m640 = np.zeros((8, 128, 640), np.float32)
            for qt in range(8):
                qg = p * T + qt * 128 + qi[:, None]
                kg = start + qt * 128 + kj[None, :]
                ok = (np.abs(kg - qg) <= W) & (kg >= 0) & (kg < S)
                # k-major: [key-in-chunk, dx-chunk, query]
                m640[qt] = np.ascontiguousarray(
                    np.where(ok, 0.0, NEG).T.reshape(5, 128, 128)
                    .transpose(1, 0, 2)).reshape(128, 640)

            m = {"hxT": hxT, "hx8": hx8, "m640": m640, "kb16": kb16}
            m.update(wts)
            in_maps.append(m)
    return in_maps


def _postprocess(results):
    out = np.zeros((B, D), np.float32)
    for b in range(B):
        m0 = np.asarray(results[2 * b]["out"]).T.reshape(D)
        m1 = np.asarray(results[2 * b + 1]["out"]).T.reshape(D)
        out[b] = np.maximum(m0, m1)
    return out


def run(inputs, debug=False, trace=False):
    nc = _get_nc(debug=debug)
    in_maps = _prep_in_maps(inputs)
    res = run_bass_kernel_spmd(nc, in_maps, core_ids=list(range(NCORES)),
                               trace=trace)
    return res


def kernel(**inputs):
    res = run(inputs, debug=False, trace=False)
    return _postprocess(res.results)


# revision 43
# speedup vs baseline: 1.4222x; 1.0033x over previous
"""Trainium2 Bass kernel for nn_LongformerEncoder (optimized v3).

Sharding: 8 cores = (batch b in 0..3, seq-half p in 0..1).
Stage A (longformer layer) runs on 1024 own tokens (+256-token halo).
A pairwise AllGather exchanges stage-A output; stage B (4-head/768-dim
MHA + max-pool) runs seq-split on queries with full keys, partial max
per core, final max across the pair on host.

v3 changes vs v2:
- Stage B entirely in fp8 (e4m3) with DoubleRow matmuls (2 contraction
  rows per PE pass): q2 projection, scores, PV, and the concatenated
  output projection. Attention contributes ~1.3% of the pre-LN signal,
  so fp8 error is negligible in the final output.
- The collective payload carries the stage-A output in fp8 in BOTH
  layouts (token-major for PV values, feature-major for score keys),
  eliminating all post-collective PE transposes in stage B.
- PV computed feature-major (lhsT = values chunk), so the attention
  output lands pre-transposed for the output projection; softmax
  denominator via a dedicated Z-column matmul, normalization via a
  ones-broadcast matmul + one DVE multiply per chunk.
- fc done once over the 4 heads' concatenated poT (single PSUM
  accumulation group; no inter-head DVE adds).
"""

import sys

sys.path.insert(0, "/opt/trn_rl_repo")

import numpy as np
import ml_dtypes

import concourse.bass as bass
import concourse.tile as tile
from concourse import bacc, mybir
from concourse.bass_utils import run_bass_kernel_spmd
from concourse.masks import make_identity

F32 = mybir.dt.float32
BF16 = mybir.dt.bfloat16
FP8 = mybir.dt.float8e4
AX = mybir.AxisListType
ALU = mybir.AluOpType
ACTF = mybir.ActivationFunctionType
DR = mybir.MatmulPerfMode.DoubleRow

B, S, D = 4, 2048, 768
W = 256
DFF = 3072
NH, DK = 4, 768
T = 1024            # own tokens per core
EXT = 1536          # own + 256 halo each side
NEG = -1e9
EPS = 1e-5
NCORES = 8
C6 = D // 128        # 6 feature chunks
KC = DFF // 128      # 24 dff chunks
ISQ_DH = 0.125       # 1/sqrt(64)
ISQ_DK = 1.0 / float(np.sqrt(DK))
SC_OLF = 16.0        # fp8 scale of stage-A output (both layouts)
SC_W = 256.0         # fp8 scale of folded stage-B weights
ZCOL = 1.0 / 16.0    # Z-helper column value so po lands at 256x true
DQ_Q2 = 1.0 / 256.0  # psum(16*256*q2) -> 16*q2
DQ_FC = 1.0 / 65536.0  # psum(256*256*fc) -> fc
SC_HX = 16.0         # fp8 scale of LN'd embeddings (stage-A input)
SC_LFW = 1024.0      # fp8 scale of longformer q/k/v/o weights
SC_AT = 32.0         # fp8 scale of stage-A attention output
DQ_QKV = 1.0 / (SC_HX * SC_LFW)
DQ_WO = 1.0 / (SC_AT * SC_LFW)


def build(debug=False):
    nc = bacc.Bacc("TRN2", target_bir_lowering=False, debug=False,
                   num_devices=NCORES)

    hxT_d = nc.dram_tensor("hxT", [128, C6 * EXT], BF16, kind="ExternalInput")
    hx8_d = nc.dram_tensor("hx8", [128, C6 * EXT], FP8, kind="ExternalInput")
    m640_d = nc.dram_tensor("m640", [8, 128, 640], F32, kind="ExternalInput")
    kb16_d = nc.dram_tensor("kb16", [128, 16], F32, kind="ExternalInput")
    lfw_d = {}
    for nm in ["lfwq", "lfwk", "lfwv", "lfwo"]:
        lfw_d[nm] = nc.dram_tensor(nm, [128, C6 * D], FP8,
                                   kind="ExternalInput")
    w1_d = nc.dram_tensor("w1", [128, KC * D], BF16, kind="ExternalInput")
    w2_d = nc.dram_tensor("w2", [DFF, D], BF16, kind="ExternalInput")
    mwA_d = nc.dram_tensor("mwA", [128, NH * C6 * D], FP8,
                           kind="ExternalInput")
    mwB_d = nc.dram_tensor("mwB", [128, NH * C6 * D], FP8,
                           kind="ExternalInput")
    out_d = nc.dram_tensor("out", [128, 6], F32, kind="ExternalOutput")
    taps = {}
    if debug:
        taps["tap_olf"] = nc.dram_tensor("tap_olf", [T, D], F32,
                                         kind="ExternalOutput")
        taps["tap_attn"] = nc.dram_tensor("tap_attn", [T, D], F32,
                                          kind="ExternalOutput")

    with tile.TileContext(nc) as tc:
        _body(nc, tc, hxT_d, hx8_d, m640_d, kb16_d, lfw_d, w1_d, w2_d,
              mwA_d, mwB_d, out_d, taps)
    nc.compile()
    return nc


def _ln_tile(nc, pool, xh0, xh1, xfull, out_tile, eps_ap):
    """out = (x - mean)/sqrt(var+eps) over 768 features. g==1, b==0.

    Reads the input (typically the PSUM transpose tile, no SBUF copy
    needed) in two halves for bn_stats; the wide normalize runs on the
    scalar engine (out = x*rstd - mu*rstd) so the DVE only carries the
    stats chain.
    """
    stats = pool.tile([128, 2, 6], F32, tag="lnstats")
    nc.vector.bn_stats(out=stats[:, 0, :], in_=xh0)
    nc.vector.bn_stats(out=stats[:, 1, :], in_=xh1)
    mv = pool.tile([128, 2], F32, tag="lnmv")
    nc.vector.bn_aggr(out=mv, in_=stats)
    rstd = pool.tile([128, 1], F32, tag="lnrstd")
    nc.scalar.activation(out=rstd, in_=mv[:, 1:2], func=ACTF.Sqrt, bias=eps_ap)
    nc.vector.reciprocal(out=rstd, in_=rstd)
    nb = pool.tile([128, 1], F32, tag="lnnb")
    nc.vector.tensor_scalar(out=nb, in0=mv[:, 0:1], scalar1=rstd,
                            scalar2=-1.0, op0=ALU.mult, op1=ALU.mult)
    nc.scalar.activation(out=out_tile, in_=xfull, func=ACTF.Identity,
                         scale=rstd, bias=nb)


def _body(nc, tc, hxT_d, hx8_d, m640_d, kb16_d, lfw_d, w1_d, w2_d,
          mwA_d, mwB_d, out_d, taps):
    import contextlib
    ctx = contextlib.ExitStack()
    with ctx:
        constg = ctx.enter_context(tc.tile_pool(name="constg", bufs=1))
        outer = ctx.enter_context(tc.tile_pool(name="outer", bufs=1))
        dram = ctx.enter_context(tc.tile_pool(name="dram", bufs=1,
                                              space="DRAM"))

        id_bf = constg.tile([128, 128], BF16, tag="id_bf")
        make_identity(nc, id_bf)
        id_f32 = constg.tile([128, 128], F32, tag="id_f32")
        make_identity(nc, id_f32)
        eps_sb = constg.tile([128, 1], F32, tag="eps")
        nc.vector.memset(eps_sb, EPS)
        kb16 = constg.tile([128, 16], F32, tag="kb16")
        nc.sync.dma_start(kb16, kb16_d.ap())
        ones_bf = constg.tile([1, 128], BF16, tag="ones_bf")
        nc.vector.memset(ones_bf, 1.0)

        # cross-stage tiles (stage-A output for stage B)
        ownT = outer.tile([128, C6, T], BF16, tag="ownT")    # 12K/part
        oT8 = outer.tile([128, C6, T], FP8, tag="oT8")       # 6K/part

        # DRAM bounce for the collectives (fp8, both layouts, split in
        # two token-halves so the first exchange overlaps the second
        # half's FFN)
        srcA = [dram.tile([T // 2, D], FP8, name=f"srcA{i}")
                for i in range(2)]
        dstA = [dram.tile([T, D], FP8, name=f"dstA{i}") for i in range(2)]
        srcB = [dram.tile([128, C6 * 512], FP8, name=f"srcB{i}")
                for i in range(2)]
        dstB = [dram.tile([256, C6 * 512], FP8, name=f"dstB{i}")
                for i in range(2)]

        # ============ STAGE A ============
        with tc.tile_pool(name="mid", bufs=1) as mid, \
             tc.tile_pool(name="sm", bufs=4) as sm, \
             tc.tile_pool(name="work", bufs=4) as work:

            with tc.tile_pool(name="inA", bufs=1) as inA, \
                 tc.tile_pool(name="attA2", bufs=1) as attA2, \
                 tc.tile_pool(name="lfw", bufs=2) as lfw:

                hx8 = inA.tile([128, C6, EXT], FP8, tag="hx8")
                hxT = inA.tile([128, C6, EXT], BF16, tag="hxT")
                aT8 = attA2.tile([128, C6, T], FP8, tag="aT8")

                with tc.tile_pool(name="attA1", bufs=1) as attA1:
                    # ---- q/k feature-major, v token-major (fp8 DoubleRow)
                    ps1 = tc.tile_pool(name="ps1", bufs=2, space="PSUM")
                    psG = ps1.__enter__()
                    # interleave the wq / hx8 loads pair-by-pair so the
                    # first accumulation group can start ~7us sooner
                    wq_sb = lfw.tile([128, C6, D], FP8, tag="lfw")
                    for j in range(3):
                        nc.sync.dma_start(
                            wq_sb[:, 2 * j:2 * j + 2, :],
                            lfw_d["lfwq"].ap()[:, 2 * j * D:(2 * j + 2) * D])
                        nc.sync.dma_start(
                            hx8[:, 2 * j:2 * j + 2, :],
                            hx8_d.ap()[:, 2 * j * EXT:(2 * j + 2) * EXT])
                    qT = attA1.tile([128, C6, T], BF16, tag="qT")
                    for f in range(C6):
                        for nch in range(2):
                            ps = psG.tile([128, 512], F32, tag="g")
                            for j in range(3):
                                nc.tensor.matmul(
                                    ps,
                                    wq_sb[:, 2 * j:2 * j + 2,
                                          f * 128:(f + 1) * 128],
                                    hx8[:, 2 * j:2 * j + 2,
                                        256 + nch * 512:
                                        256 + (nch + 1) * 512],
                                    start=(j == 0), stop=(j == 2),
                                    perf_mode=DR)
                            nc.scalar.activation(
                                out=qT[:, f, nch * 512:(nch + 1) * 512],
                                in_=ps, func=ACTF.Copy, scale=DQ_QKV)
                    wk_sb = lfw.tile([128, C6, D], FP8, tag="lfw")
                    nc.sync.dma_start(wk_sb, lfw_d["lfwk"].ap())
                    # hxT (bf16 residual) only needed at wo; load now
                    nc.sync.dma_start(hxT, hxT_d.ap())
                    kT = attA1.tile([128, C6, EXT], BF16, tag="kT")
                    for f in range(C6):
                        for nch in range(3):
                            ps = psG.tile([128, 512], F32, tag="g")
                            for j in range(3):
                                nc.tensor.matmul(
                                    ps,
                                    wk_sb[:, 2 * j:2 * j + 2,
                                          f * 128:(f + 1) * 128],
                                    hx8[:, 2 * j:2 * j + 2,
                                        nch * 512:(nch + 1) * 512],
                                    start=(j == 0), stop=(j == 2),
                                    perf_mode=DR)
                            nc.scalar.activation(
                                out=kT[:, f, nch * 512:(nch + 1) * 512],
                                in_=ps, func=ACTF.Copy, scale=DQ_QKV)
                    wv_sb = lfw.tile([128, C6, D], FP8, tag="lfw")
                    nc.sync.dma_start(wv_sb, lfw_d["lfwv"].ap())
                    # values token-major in fp8 at x32 (PV runs in fp8)
                    vtok8 = attA1.tile([128, 12, D], FP8, tag="vtok8")
                    for t in range(12):
                        for (n0, nn) in ((0, 512), (512, 256)):
                            ps = psG.tile([128, 512], F32, tag="g")
                            for j in range(3):
                                nc.tensor.matmul(
                                    ps[:, :nn],
                                    hx8[:, 2 * j:2 * j + 2,
                                        t * 128:(t + 1) * 128],
                                    wv_sb[:, 2 * j:2 * j + 2, n0:n0 + nn],
                                    start=(j == 0), stop=(j == 2),
                                    perf_mode=DR)
                            nc.scalar.activation(
                                out=vtok8[:, t, n0:n0 + nn], in_=ps[:, :nn],
                                func=ACTF.Copy, scale=DQ_QKV * SC_AT)
                    ps1.__exit__(None, None, None)

                    # ---- sliding-window attention, k-major scores so the
                    # probs land contraction-ready (no transpose matmuls);
                    # PV in fp8 DoubleRow; softmax Z via a ones-row matmul,
                    # normalization via GPSIMD partition-broadcast + DVE.
                    ps2 = tc.tile_pool(name="ps2", bufs=2, space="PSUM")
                    psS = ps2.__enter__()
                    ps2c = tc.tile_pool(name="ps2c", bufs=2, space="PSUM")
                    psV = ps2c.__enter__()
                    ps2z = tc.tile_pool(name="ps2z", bufs=2, space="PSUM")
                    psZ1 = ps2z.__enter__()

                    ones8 = constg.tile([128, 2, 16], FP8, tag="ones8")
                    nc.vector.memset(ones8, 1.0)

                    m640_t = [None] * 8

                    def a_scores(qt, pair, h2):
                        if pair == 0 and h2 == 0:
                            m640_t[qt] = work.tile([128, 640], F32,
                                                   tag="m640", name="m640")
                            nc.sync.dma_start(m640_t[qt], m640_d.ap()[qt])
                        ps = psS.tile([128, 640], F32, tag="sc")
                        rhsq = qT[h2 * 64:(h2 + 1) * 64, pair,
                                  qt * 128:(qt + 1) * 128]
                        for dx in range(5):
                            nc.tensor.matmul(
                                ps[:, dx * 128:(dx + 1) * 128],
                                kT[h2 * 64:(h2 + 1) * 64, pair,
                                   qt * 128 + dx * 128:
                                   qt * 128 + (dx + 1) * 128],
                                rhsq, start=True, stop=True,
                                tile_position=(h2 * 64, 0))
                        sb = work.tile([128, 640], F32, tag="sb")
                        nc.vector.tensor_tensor(sb, ps, m640_t[qt], ALU.add)
                        probs8 = work.tile([128, 5, 128], FP8, tag="probs8")
                        nc.scalar.activation(out=probs8, in_=sb,
                                             func=ACTF.Exp, scale=ISQ_DH)
                        return probs8

                    def a_rest(qt, pair, h2, probs8):
                        h = 2 * pair + h2
                        zq = psZ1.tile([1, 128], F32, tag="zq")
                        pvt = psV.tile([128, 128], F32, tag="pv")
                        for i in range(2):
                            nc.tensor.matmul(
                                zq, ones8[:, :, 0:1],
                                probs8[:, 2 * i:2 * i + 2, :],
                                start=(i == 0), stop=False, perf_mode=DR)
                        nc.tensor.matmul(zq, ones8[:, 0, 0:1],
                                         probs8[:, 4, :],
                                         start=False, stop=True)
                        if h2 == 0:
                            # DoubleRow requires dst partition offset 0
                            for i in range(2):
                                nc.tensor.matmul(
                                    pvt[0:64, :],
                                    vtok8[:, qt + 2 * i:qt + 2 * i + 2,
                                          h * 64:(h + 1) * 64],
                                    probs8[:, 2 * i:2 * i + 2, :],
                                    start=(i == 0), stop=False, perf_mode=DR,
                                    tile_position=(0, 0))
                            nc.tensor.matmul(
                                pvt[0:64, :],
                                vtok8[:, qt + 4, h * 64:(h + 1) * 64],
                                probs8[:, 4, :], start=False, stop=True,
                                tile_position=(0, 0))
                        else:
                            for dx in range(5):
                                nc.tensor.matmul(
                                    pvt[64:128, :],
                                    vtok8[:, qt + dx, h * 64:(h + 1) * 64],
                                    probs8[:, dx, :], start=(dx == 0),
                                    stop=(dx == 4),
                                    tile_position=(0, 64))
                        rs = sm.tile([1, 128], F32, tag="rs")
                        nc.vector.reciprocal(rs, zq)
                        rs_bc = work.tile([128, 128], F32, tag="rs_bc")
                        nc.gpsimd.partition_broadcast(rs_bc, rs)
                        nc.vector.tensor_tensor(
                            aT8[h2 * 64:(h2 + 1) * 64, pair,
                                qt * 128:(qt + 1) * 128],
                            pvt[h2 * 64:(h2 + 1) * 64, :],
                            rs_bc[h2 * 64:(h2 + 1) * 64, :], ALU.mult)

                    its = [(qt, pair, h2) for qt in range(8)
                           for pair in range(6) for h2 in range(2)]
                    prev = None
                    for it in its:
                        probs8 = a_scores(*it)
                        if prev is not None:
                            a_rest(prev[0][0], prev[0][1], prev[0][2],
                                   prev[1])
                        prev = (it, probs8)
                    a_rest(prev[0][0], prev[0][1], prev[0][2], prev[1])

                    ps2z.__exit__(None, None, None)
                    ps2c.__exit__(None, None, None)
                    ps2.__exit__(None, None, None)

                # ---- wo + residual (feature-major, fp8 DoubleRow)
                ps3 = tc.tile_pool(name="ps3", bufs=2, space="PSUM")
                psG = ps3.__enter__()
                wo_sb = lfw.tile([128, C6, D], FP8, tag="lfw")
                nc.sync.dma_start(wo_sb, lfw_d["lfwo"].ap())
                r1T = mid.tile([128, C6, T], BF16, tag="resT")
                for f in range(C6):
                    for nch in range(2):
                        ps = psG.tile([128, 512], F32, tag="g")
                        for j in range(3):
                            nc.tensor.matmul(
                                ps,
                                wo_sb[:, 2 * j:2 * j + 2,
                                      f * 128:(f + 1) * 128],
                                aT8[:, 2 * j:2 * j + 2,
                                    nch * 512:(nch + 1) * 512],
                                start=(j == 0), stop=(j == 2), perf_mode=DR)
                        t0 = work.tile([128, 512], BF16, tag="t0")
                        nc.scalar.activation(out=t0, in_=ps, func=ACTF.Copy,
                                             scale=DQ_WO)
                        nc.vector.tensor_tensor(
                            r1T[:, f, nch * 512:(nch + 1) * 512], t0,
                            hxT[:, f, 256 + nch * 512: 256 + (nch + 1) * 512],
                            ALU.add)
                ps3.__exit__(None, None, None)

            # FFN weights: w1 stays resident across both FFN passes, in a
            # pool that reuses the space just freed by the attention pools.
            ffnp = tc.tile_pool(name="ffnp", bufs=1)
            ffnpo = ffnp.__enter__()
            w1sb = ffnpo.tile([128, KC, C6, 128], BF16, tag="w1sb")
            nc.sync.dma_start(w1sb, w1_d.ap())

            # ---- LN1 (transpose to token-major, LN, transpose back)
            # software-pipelined: forward transposes of t+1 issue before the
            # back transposes of t, so the PE isn't stalled by the LN chain
            ps3b = tc.tile_pool(name="ps3b", bufs=4, space="PSUM")
            psT = ps3b.__enter__()
            h1T = mid.tile([128, C6, T], BF16, tag="h1T")

            def ln1_fwd(t):
                tp = psT.tile([128, C6 * 128], BF16, tag="tp3")
                for c in range(C6):
                    nc.tensor.transpose(tp[:, c * 128:(c + 1) * 128],
                                        r1T[:, c, t * 128:(t + 1) * 128],
                                        id_bf)
                ltok = work.tile([128, D], BF16, tag="ltok")
                _ln_tile(nc, sm, tp[:, 0:384], tp[:, 384:768], tp, ltok,
                         eps_sb)
                return ltok

            def ln1_back(t, ltok):
                tp2 = psT.tile([128, C6, 128], BF16, tag="tp3")
                for c in range(C6):
                    nc.tensor.transpose(tp2[:, c, :],
                                        ltok[:, c * 128:(c + 1) * 128],
                                        id_bf)
                nc.vector.tensor_copy(out=h1T[:, :, t * 128:(t + 1) * 128],
                                      in_=tp2)

            prevLs = []
            for t in range(8):
                ltok = ln1_fwd(t)
                prevLs.append((t, ltok))
                if len(prevLs) > 2:
                    ln1_back(*prevLs.pop(0))
            for pl in prevLs:
                ln1_back(*pl)
            ps3b.__exit__(None, None, None)

            # ---- FFN + LN2, one token-half at a time; each half's fp8
            # payload is exchanged as soon as it is ready so the second
            # half's FFN overlaps the first collective.
            r2T = mid.tile([128, C6, T], BF16, tag="resT")
            g1all = ffnpo.tile([128, KC, 512], BF16, tag="g1all")
            for nch in range(2):
                # g1 = gelu(h1 @ w1), all 24 dff chunks
                ps4 = tc.tile_pool(name="ps4", bufs=2, space="PSUM")
                psG = ps4.__enter__()
                for kc in range(KC):
                    g1p = psG.tile([128, 512], F32, tag="g")
                    for k in range(C6):
                        nc.tensor.matmul(
                            g1p, w1sb[:, kc, k, :],
                            h1T[:, k, nch * 512:(nch + 1) * 512],
                            start=(k == 0), stop=(k == 5))
                    nc.scalar.activation(out=g1all[:, kc, :], in_=g1p,
                                         func=ACTF.Gelu_apprx_tanh)
                ps4.__exit__(None, None, None)
                # f2 = g1 @ w2 (+ residual)
                ps4b = tc.tile_pool(name="ps4b", bufs=1, space="PSUM")
                psF = ps4b.__enter__()
                with tc.tile_pool(name="bigw", bufs=4) as bigw:
                    f2ps = [psF.tile([128, 512], F32, tag=f"f2_{m}",
                                     name=f"f2_{m}") for m in range(C6)]
                    for kc in range(KC):
                        w2c = bigw.tile([128, D], BF16, tag="w2c")
                        nc.sync.dma_start(
                            w2c, w2_d.ap()[kc * 128:(kc + 1) * 128, :])
                        for m in range(C6):
                            nc.tensor.matmul(
                                f2ps[m], w2c[:, m * 128:(m + 1) * 128],
                                g1all[:, kc, :],
                                start=(kc == 0), stop=(kc == KC - 1))
                    for m in range(C6):
                        nc.vector.tensor_tensor(
                            r2T[:, m, nch * 512:(nch + 1) * 512], f2ps[m],
                            h1T[:, m, nch * 512:(nch + 1) * 512], ALU.add)
                ps4b.__exit__(None, None, None)

                # LN2 for this half (pipelined like LN1)
                ps5 = tc.tile_pool(name="ps5", bufs=4, space="PSUM")
                psT5 = ps5.__enter__()

                def ln2_fwd(t):
                    tp = psT5.tile([128, C6 * 128], BF16, tag="tp5")
                    for c in range(C6):
                        nc.tensor.transpose(
                            tp[:, c * 128:(c + 1) * 128],
                            r2T[:, c, t * 128:(t + 1) * 128], id_bf)
                    otok = work.tile([128, D], BF16, tag="ltok")
                    _ln_tile(nc, sm, tp[:, 0:384], tp[:, 384:768], tp, otok,
                             eps_sb)
                    return otok

                def ln2_back(t, otok):
                    otok8 = work.tile([128, D], FP8, tag="otok8")
                    nc.scalar.activation(out=otok8, in_=otok, func=ACTF.Copy,
                                         scale=SC_OLF)
                    nc.sync.dma_start(
                        srcA[t // 4][(t % 4) * 128:(t % 4 + 1) * 128, :],
                        otok8)
                    tp2 = psT5.tile([128, C6, 128], BF16, tag="tp5")
                    for c in range(C6):
                        nc.tensor.transpose(tp2[:, c, :],
                                            otok[:, c * 128:(c + 1) * 128],
                                            id_bf)
                    nc.vector.tensor_copy(
                        out=ownT[:, :, t * 128:(t + 1) * 128], in_=tp2)
                    nc.scalar.activation(
                        out=oT8[:, :, t * 128:(t + 1) * 128], in_=tp2,
                        func=ACTF.Copy, scale=SC_OLF)
                    if "tap_olf" in taps:
                        of = work.tile([128, D], F32, tag="tapolf")
                        nc.vector.tensor_copy(out=of, in_=otok)
                        nc.sync.dma_start(
                            taps["tap_olf"].ap()[t * 128:(t + 1) * 128, :],
                            of)

                prevTs = []
                for t in range(nch * 4, nch * 4 + 4):
                    otok = ln2_fwd(t)
                    prevTs.append((t, otok))
                    if len(prevTs) > 2:
                        ln2_back(*prevTs.pop(0))
                for pt in prevTs:
                    ln2_back(*pt)
                nc.sync.dma_start(
                    srcB[nch], oT8[:, :, nch * 512:(nch + 1) * 512])
                ps5.__exit__(None, None, None)

                # exchange this half right away
                nc.gpsimd.collective_compute(
                    "AllGather", ALU.bypass,
                    replica_groups=[[0, 1], [2, 3], [4, 5], [6, 7]],
                    ins=[srcA[nch][:].opt()], outs=[dstA[nch][:].opt()])
                nc.gpsimd.collective_compute(
                    "AllGather", ALU.bypass,
                    replica_groups=[[0, 1], [2, 3], [4, 5], [6, 7]],
                    ins=[srcB[nch][:].opt()], outs=[dstB[nch][:].opt()])
            ffnp.__exit__(None, None, None)

        # ============ STAGE B ============
        with tc.tile_pool(name="resB", bufs=1) as resB, \
             tc.tile_pool(name="whead", bufs=2) as whead, \
             tc.tile_pool(name="hb", bufs=2) as hb, \
             tc.tile_pool(name="workB", bufs=2) as workB, \
             tc.tile_pool(name="smB", bufs=4) as smB:

            ps6 = tc.tile_pool(name="psG2", bufs=2, space="PSUM")
            psG2 = ps6.__enter__()

            # mwB needed only for fc at the end; start the DMA early
            mwB8_sb = resB.tile([128, NH * C6, D], FP8, tag="mwB8")
            nc.sync.dma_start(mwB8_sb, mwB_d.ap())

            # --- pre-collective: q2 for all 4 heads from local oT8
            q2T8 = []
            for h in range(NH):
                wh8 = whead.tile([128, C6, D], FP8, tag="wh")
                nc.sync.dma_start(
                    wh8, mwA_d.ap()[:, h * C6 * D:(h + 1) * C6 * D])
                q2 = resB.tile([128, C6, T], FP8, tag=f"q2T8_{h}")
                for f in range(C6):
                    for nch in range(2):
                        ps = psG2.tile([128, 512], F32, tag="g2")
                        for j in range(3):
                            nc.tensor.matmul(
                                ps,
                                wh8[:, 2 * j:2 * j + 2,
                                    f * 128:(f + 1) * 128],
                                oT8[:, 2 * j:2 * j + 2,
                                    nch * 512:(nch + 1) * 512],
                                start=(j == 0), stop=(j == 2), perf_mode=DR)
                        nc.scalar.activation(
                            out=q2[:, f, nch * 512:(nch + 1) * 512],
                            in_=ps, func=ACTF.Copy, scale=DQ_Q2)
                q2T8.append(q2)

            # --- land collective results (no transposes needed)
            # global token chunk tt: half g = tt//8, sub-half s = (tt%8)//4
            olfT8 = resB.tile([128, C6, 2 * T], FP8, tag="olfT8")
            olftok8 = resB.tile([128, 16, 784], FP8, tag="olftok8")
            nc.vector.memset(olftok8[:, :, 768:769], ZCOL)
            for g in range(2):
                for s in range(2):
                    nc.sync.dma_start(
                        olfT8[:, :, g * T + s * 512:g * T + (s + 1) * 512],
                        dstB[s][g * 128:(g + 1) * 128, :])
            for tt in range(16):
                g, s, r = tt // 8, (tt % 8) // 4, tt % 4
                nc.sync.dma_start(
                    olftok8[:, tt, 0:768],
                    dstA[s][g * 512 + r * 128:g * 512 + (r + 1) * 128, :])

            poT8 = resB.tile([128, NH * C6, T], FP8, tag="poT8")
            fcacc = resB.tile([128, C6, T], BF16, tag="fcacc")

            ps7z = tc.tile_pool(name="psZ", bufs=2, space="PSUM")
            psZ = ps7z.__enter__()
            ps7b = tc.tile_pool(name="psBC", bufs=2, space="PSUM")
            psBC = ps7b.__enter__()
            ps7 = tc.tile_pool(name="psPV", bufs=2, space="PSUM")
            psPV = ps7.__enter__()

            def sc_issue(h, qch):
                expT8 = hb.tile([128, 16, 512], FP8, tag="expT8")
                for kt in range(16):
                    ps = psG2.tile([128, 512], F32, tag="g2")
                    for j in range(3):
                        nc.tensor.matmul(
                            ps,
                            olfT8[:, 2 * j:2 * j + 2,
                                  kt * 128:(kt + 1) * 128],
                            q2T8[h][:, 2 * j:2 * j + 2,
                                    qch * 512:(qch + 1) * 512],
                            start=(j == 0), stop=(j == 2), perf_mode=DR)
                    nc.scalar.activation(out=expT8[:, kt, :], in_=ps,
                                         func=ACTF.Exp,
                                         bias=kb16[:, kt:kt + 1],
                                         scale=ISQ_DK / 256.0)
                return expT8

            def pv_issue(h, qch, expT8):
                zp = psZ.tile([1, 512], F32, tag="z")
                for i in range(8):
                    nc.tensor.matmul(
                        zp, olftok8[:, 2 * i:2 * i + 2, 768:769],
                        expT8[:, 2 * i:2 * i + 2, :],
                        start=(i == 0), stop=(i == 7), perf_mode=DR)
                rs = smB.tile([1, 512], BF16, tag="rs2")
                with nc.allow_low_precision(
                        reason="1/Z feeds fp8-precision normalization"):
                    nc.vector.reciprocal(rs, zp)
                bc = psBC.tile([128, 512], F32, tag="bc")
                bc_sb = workB.tile([128, 512], BF16, tag="bc_sb")
                for c in range(C6):
                    pp = psPV.tile([128, 512], F32, tag="pv")
                    for i in range(8):
                        nc.tensor.matmul(
                            pp,
                            olftok8[:, 2 * i:2 * i + 2,
                                    c * 128:(c + 1) * 128],
                            expT8[:, 2 * i:2 * i + 2, :],
                            start=(i == 0), stop=(i == 7), perf_mode=DR)
                    if c == 0:
                        nc.tensor.matmul(bc, ones_bf, rs,
                                         start=True, stop=True)
                        nc.scalar.activation(out=bc_sb, in_=bc,
                                             func=ACTF.Copy)
                    nc.vector.tensor_tensor(
                        poT8[:, h * C6 + c, qch * 512:(qch + 1) * 512],
                        pp, bc_sb, ALU.mult)

            # head loop: sc issued one step ahead of pv
            prev = None
            for h in range(NH):
                for qch in range(2):
                    expT8 = sc_issue(h, qch)
                    if prev is not None:
                        pv_issue(*prev)
                    prev = (h, qch, expT8)
            pv_issue(*prev)

            ps7.__exit__(None, None, None)
            ps7b.__exit__(None, None, None)
            ps7z.__exit__(None, None, None)

            # --- output projection over concatenated heads + residual,
            # interleaved with the tail LN/max per token-half so the
            # serial LN chain overlaps the second half's fc matmuls
            ps8 = tc.tile_pool(name="psT2", bufs=3, space="PSUM")
            psT2 = ps8.__enter__()
            ps9 = tc.tile_pool(name="psTail", bufs=2, space="PSUM")
            psTail = ps9.__enter__()
            maxacc = resB.tile([128, D], F32, tag="maxacc")

            def tail_t(t):
                tp = psT2.tile([128, C6 * 128], BF16, tag="tpB")
                for c in range(C6):
                    nc.tensor.transpose(tp[:, c * 128:(c + 1) * 128],
                                        fcacc[:, c, t * 128:(t + 1) * 128],
                                        id_bf)
                ltok = workB.tile([128, D], F32, tag="ltokB")
                _ln_tile(nc, smB, tp[:, 0:384], tp[:, 384:768], tp, ltok,
                         eps_sb)
                if "tap_attn" in taps:
                    nc.sync.dma_start(
                        taps["tap_attn"].ap()[t * 128:(t + 1) * 128, :], ltok)
                if t == 0:
                    nc.vector.tensor_copy(out=maxacc, in_=ltok)
                else:
                    nc.vector.tensor_tensor(maxacc, maxacc, ltok, ALU.max)

            for nch in range(2):
                for m in range(C6):
                    ps = psG2.tile([128, 512], F32, tag="g2")
                    for j in range(NH * C6 // 2):
                        nc.tensor.matmul(
                            ps,
                            mwB8_sb[:, 2 * j:2 * j + 2,
                                    m * 128:(m + 1) * 128],
                            poT8[:, 2 * j:2 * j + 2,
                                 nch * 512:(nch + 1) * 512],
                            start=(j == 0), stop=(j == NH * C6 // 2 - 1),
                            perf_mode=DR)
                    t1 = workB.tile([128, 512], BF16, tag="t1")
                    nc.scalar.activation(out=t1, in_=ps, func=ACTF.Copy,
                                         scale=DQ_FC)
                    nc.vector.tensor_tensor(
                        fcacc[:, m, nch * 512:(nch + 1) * 512], t1,
                        ownT[:, m, nch * 512:(nch + 1) * 512], ALU.add)
                for t in range(nch * 4, nch * 4 + 4):
                    tail_t(t)
            outsb = resB.tile([128, 6], F32, tag="outsb")
            for c in range(C6):
                pt = psTail.tile([128, 128], F32, tag="tpf")
                nc.tensor.transpose(pt, maxacc[:, c * 128:(c + 1) * 128],
                                    id_f32)
                nc.vector.tensor_reduce(out=outsb[:, c:c + 1], in_=pt,
                                        axis=AX.X, op=ALU.max)
            nc.sync.dma_start(out_d.ap(), outsb)
            ps9.__exit__(None, None, None)
            ps8.__exit__(None, None, None)
            ps6.__exit__(None, None, None)

    return


# ---------------- host side ----------------

_NC_CACHE = {}


def _get_nc(debug=False):
    key = bool(debug)
    if key not in _NC_CACHE:
        _NC_CACHE[key] = build(debug=debug)
    return _NC_CACHE[key]


def _prep_in_maps(inputs):
    bf = ml_dtypes.bfloat16
    f8 = ml_dtypes.float8_e4m3
    x = np.asarray(inputs["x"])
    emb = np.asarray(inputs["emb"], np.float32)
    pos = np.asarray(inputs["pos"], np.float32)
    g_e = np.asarray(inputs["ln_e_g"], np.float32)
    b_e = np.asarray(inputs["ln_e_b"], np.float32)

    def parr(w):
        w = np.ascontiguousarray(
            np.asarray(w, np.float32).reshape(C6, 128, -1)
            .transpose(1, 0, 2)).astype(bf)
        return w.reshape(128, -1)

    def parr8(w, scale):
        w = np.clip(np.asarray(w, np.float32) * scale, -240.0, 240.0)
        w = np.ascontiguousarray(
            w.reshape(-1, 128, w.shape[-1]).transpose(1, 0, 2)).astype(f8)
        return w.reshape(128, -1)

    wts = {
        "lfwq": parr8(inputs["lf_wq"], SC_LFW),
        "lfwk": parr8(inputs["lf_wk"], SC_LFW),
        "lfwv": parr8(inputs["lf_wv"], SC_LFW),
        "lfwo": parr8(inputs["lf_wo"], SC_LFW),
    }
    w1 = np.asarray(inputs["w1"], np.float32)
    wts["w1"] = np.ascontiguousarray(
        w1.reshape(C6, 128, KC, 128).transpose(1, 2, 0, 3)
    ).astype(bf).reshape(128, KC * D)
    wts["w2"] = np.asarray(inputs["w2"], np.float32).astype(bf)

    # folded stage-B matrices, fp8 x256
    wq = np.asarray(inputs["mha_wq"], np.float32).reshape(D, NH, DK)
    wk = np.asarray(inputs["mha_wk"], np.float32).reshape(D, NH, DK)
    wv = np.asarray(inputs["mha_wv"], np.float32).reshape(D, NH, DK)
    fc = np.asarray(inputs["mha_fc"], np.float32).reshape(NH, DK, D)
    mwA = np.concatenate(
        [parr8(wq[:, h, :] @ wk[:, h, :].T, SC_W) for h in range(NH)], axis=1)
    Bcat = np.concatenate([wv[:, h, :] @ fc[h] for h in range(NH)], axis=0)
    wts["mwA"] = np.ascontiguousarray(mwA)
    wts["mwB"] = np.ascontiguousarray(parr8(Bcat, SC_W))

    in_maps = []
    for b in range(B):
        h0 = emb[x[b]] + pos                        # [S, D] f32
        mu = h0.mean(-1, keepdims=True)
        var = h0.var(-1, keepdims=True)
        hn = (h0 - mu) / np.sqrt(var + EPS) * g_e + b_e
        kbias = np.where(x[b] != 0, 0.0, NEG).astype(np.float32)
        kb16 = np.ascontiguousarray(kbias.reshape(16, 128).T)
        for p in range(2):
            start = p * T - 256
            hxe = np.zeros((EXT, D), np.float32)
            lo, hi = max(0, start), min(S, start + EXT)
            hxe[lo - start: hi - start] = hn[lo:hi]
            hxf = np.ascontiguousarray(
                hxe.reshape(EXT, C6, 128).transpose(2, 1, 0))
            hxT = hxf.astype(bf).reshape(128, C6 * EXT)
            hx8 = np.clip(hxf * SC_HX, -240.0, 240.0).astype(f8).reshape(
                128, C6 * EXT)

            qi = np.arange(128)
            kj = np.arange(640)
            m640 = np.zeros((8, 128, 640), np.float32)
            for qt in range(8):
                qg = p * T + qt * 128 + qi[:, None]
                kg = start + qt * 128 + kj[None, :]
                ok = (np.abs(kg - qg) <= W) & (kg >= 0) & (kg < S)
                # k-major: [key-in-chunk, dx-chunk, query]
                m640[qt] = np.ascontiguousarray(
                    np.where(ok, 0.0, NEG).T.reshape(5, 128, 128)
                    .transpose(1, 0, 2)).reshape(128, 640)

            m = {"hxT": hxT, "hx8": hx8, "m640": m640, "kb16": kb16}
            m.update(wts)
            in_maps.append(m)
    return in_maps


def _postprocess(results):
    out = np.zeros((B, D), np.float32)
    for b in range(B):
        m0 = np.asarray(results[2 * b]["out"]).T.reshape(D)
        m1 = np.asarray(results[2 * b + 1]["out"]).T.reshape(D)
        out[b] = np.maximum(m0, m1)
    return out


def run(inputs, debug=False, trace=False):
    nc = _get_nc(debug=debug)
    in_maps = _prep_in_maps(inputs)
    res = run_bass_kernel_spmd(nc, in_maps, core_ids=list(range(NCORES)),
                               trace=trace)
    return res


def kernel(**inputs):
    res = run(inputs, debug=False, trace=False)
    return _postprocess(res.results)


# revision 44
# speedup vs baseline: 1.4294x; 1.0051x over previous
"""Trainium2 Bass kernel for nn_LongformerEncoder (optimized v3).

Sharding: 8 cores = (batch b in 0..3, seq-half p in 0..1).
Stage A (longformer layer) runs on 1024 own tokens (+256-token halo).
A pairwise AllGather exchanges stage-A output; stage B (4-head/768-dim
MHA + max-pool) runs seq-split on queries with full keys, partial max
per core, final max across the pair on host.

v3 changes vs v2:
- Stage B entirely in fp8 (e4m3) with DoubleRow matmuls (2 contraction
  rows per PE pass): q2 projection, scores, PV, and the concatenated
  output projection. Attention contributes ~1.3% of the pre-LN signal,
  so fp8 error is negligible in the final output.
- The collective payload carries the stage-A output in fp8 in BOTH
  layouts (token-major for PV values, feature-major for score keys),
  eliminating all post-collective PE transposes in stage B.
- PV computed feature-major (lhsT = values chunk), so the attention
  output lands pre-transposed for the output projection; softmax
  denominator via a dedicated Z-column matmul, normalization via a
  ones-broadcast matmul + one DVE multiply per chunk.
- fc done once over the 4 heads' concatenated poT (single PSUM
  accumulation group; no inter-head DVE adds).
"""

import sys

sys.path.insert(0, "/opt/trn_rl_repo")

import numpy as np
import ml_dtypes

import concourse.bass as bass
import concourse.tile as tile
from concourse import bacc, mybir
from concourse.bass_utils import run_bass_kernel_spmd
from concourse.masks import make_identity

F32 = mybir.dt.float32
BF16 = mybir.dt.bfloat16
FP8 = mybir.dt.float8e4
AX = mybir.AxisListType
ALU = mybir.AluOpType
ACTF = mybir.ActivationFunctionType
DR = mybir.MatmulPerfMode.DoubleRow

B, S, D = 4, 2048, 768
W = 256
DFF = 3072
NH, DK = 4, 768
T = 1024            # own tokens per core
EXT = 1536          # own + 256 halo each side
NEG = -1e9
EPS = 1e-5
NCORES = 8
C6 = D // 128        # 6 feature chunks
KC = DFF // 128      # 24 dff chunks
ISQ_DH = 0.125       # 1/sqrt(64)
ISQ_DK = 1.0 / float(np.sqrt(DK))
SC_OLF = 16.0        # fp8 scale of stage-A output (both layouts)
SC_W = 256.0         # fp8 scale of folded stage-B weights
ZCOL = 1.0 / 16.0    # Z-helper column value so po lands at 256x true
DQ_Q2 = 1.0 / 256.0  # psum(16*256*q2) -> 16*q2
DQ_FC = 1.0 / 65536.0  # psum(256*256*fc) -> fc
SC_HX = 16.0         # fp8 scale of LN'd embeddings (stage-A input)
SC_LFW = 1024.0      # fp8 scale of longformer q/k/v/o weights
SC_AT = 32.0         # fp8 scale of stage-A attention output
DQ_QKV = 1.0 / (SC_HX * SC_LFW)
DQ_WO = 1.0 / (SC_AT * SC_LFW)


def build(debug=False):
    nc = bacc.Bacc("TRN2", target_bir_lowering=False, debug=False,
                   num_devices=NCORES)

    hxT_d = nc.dram_tensor("hxT", [128, C6 * EXT], BF16, kind="ExternalInput")
    hx8_d = nc.dram_tensor("hx8", [128, C6 * EXT], FP8, kind="ExternalInput")
    m640_d = nc.dram_tensor("m640", [8, 128, 640], F32, kind="ExternalInput")
    kb16_d = nc.dram_tensor("kb16", [128, 16], F32, kind="ExternalInput")
    lfw_d = {}
    for nm in ["lfwq", "lfwk", "lfwv", "lfwo"]:
        lfw_d[nm] = nc.dram_tensor(nm, [128, C6 * D], FP8,
                                   kind="ExternalInput")
    w1_d = nc.dram_tensor("w1", [128, KC * D], BF16, kind="ExternalInput")
    w2_d = nc.dram_tensor("w2", [DFF, D], BF16, kind="ExternalInput")
    mwA_d = nc.dram_tensor("mwA", [128, NH * C6 * D], FP8,
                           kind="ExternalInput")
    mwB_d = nc.dram_tensor("mwB", [128, NH * C6 * D], FP8,
                           kind="ExternalInput")
    out_d = nc.dram_tensor("out", [128, 6], F32, kind="ExternalOutput")
    taps = {}
    if debug:
        taps["tap_olf"] = nc.dram_tensor("tap_olf", [T, D], F32,
                                         kind="ExternalOutput")
        taps["tap_attn"] = nc.dram_tensor("tap_attn", [T, D], F32,
                                          kind="ExternalOutput")

    with tile.TileContext(nc) as tc:
        _body(nc, tc, hxT_d, hx8_d, m640_d, kb16_d, lfw_d, w1_d, w2_d,
              mwA_d, mwB_d, out_d, taps)
    nc.compile()
    return nc


def _ln_tile(nc, pool, xh0, xh1, xfull, out_tile, eps_ap):
    """out = (x - mean)/sqrt(var+eps) over 768 features. g==1, b==0.

    Reads the input (typically the PSUM transpose tile, no SBUF copy
    needed) in two halves for bn_stats; the wide normalize runs on the
    scalar engine (out = x*rstd - mu*rstd) so the DVE only carries the
    stats chain.
    """
    stats = pool.tile([128, 2, 6], F32, tag="lnstats")
    nc.vector.bn_stats(out=stats[:, 0, :], in_=xh0)
    nc.vector.bn_stats(out=stats[:, 1, :], in_=xh1)
    mv = pool.tile([128, 2], F32, tag="lnmv")
    nc.vector.bn_aggr(out=mv, in_=stats)
    rstd = pool.tile([128, 1], F32, tag="lnrstd")
    nc.scalar.activation(out=rstd, in_=mv[:, 1:2], func=ACTF.Sqrt, bias=eps_ap)
    nc.vector.reciprocal(out=rstd, in_=rstd)
    nb = pool.tile([128, 1], F32, tag="lnnb")
    nc.vector.tensor_scalar(out=nb, in0=mv[:, 0:1], scalar1=rstd,
                            scalar2=-1.0, op0=ALU.mult, op1=ALU.mult)
    nc.scalar.activation(out=out_tile, in_=xfull, func=ACTF.Identity,
                         scale=rstd, bias=nb)


def _body(nc, tc, hxT_d, hx8_d, m640_d, kb16_d, lfw_d, w1_d, w2_d,
          mwA_d, mwB_d, out_d, taps):
    import contextlib
    ctx = contextlib.ExitStack()
    with ctx:
        constg = ctx.enter_context(tc.tile_pool(name="constg", bufs=1))
        outer = ctx.enter_context(tc.tile_pool(name="outer", bufs=1))
        dram = ctx.enter_context(tc.tile_pool(name="dram", bufs=1,
                                              space="DRAM"))

        id_bf = constg.tile([128, 128], BF16, tag="id_bf")
        make_identity(nc, id_bf)
        id_f32 = constg.tile([128, 128], F32, tag="id_f32")
        make_identity(nc, id_f32)
        eps_sb = constg.tile([128, 1], F32, tag="eps")
        nc.vector.memset(eps_sb, EPS)
        kb16 = constg.tile([128, 16], F32, tag="kb16")
        nc.sync.dma_start(kb16, kb16_d.ap())
        ones_bf = constg.tile([1, 128], BF16, tag="ones_bf")
        nc.vector.memset(ones_bf, 1.0)

        # cross-stage tiles (stage-A output for stage B)
        ownT = outer.tile([128, C6, T], BF16, tag="ownT")    # 12K/part
        oT8 = outer.tile([128, C6, T], FP8, tag="oT8")       # 6K/part

        # DRAM bounce for the collectives (fp8, both layouts, split in
        # two token-halves so the first exchange overlaps the second
        # half's FFN)
        srcA = [dram.tile([T // 2, D], FP8, name=f"srcA{i}")
                for i in range(2)]
        dstA = [dram.tile([T, D], FP8, name=f"dstA{i}") for i in range(2)]
        srcB = [dram.tile([128, C6 * 512], FP8, name=f"srcB{i}")
                for i in range(2)]
        dstB = [dram.tile([256, C6 * 512], FP8, name=f"dstB{i}")
                for i in range(2)]

        # ============ STAGE A ============
        with tc.tile_pool(name="mid", bufs=1) as mid, \
             tc.tile_pool(name="sm", bufs=4) as sm, \
             tc.tile_pool(name="work", bufs=4) as work:

            with tc.tile_pool(name="inA", bufs=1) as inA, \
                 tc.tile_pool(name="attA2", bufs=1) as attA2, \
                 tc.tile_pool(name="lfw", bufs=2) as lfw:

                hx8 = inA.tile([128, C6, EXT], FP8, tag="hx8")
                hxT = inA.tile([128, C6, EXT], BF16, tag="hxT")
                aT8 = attA2.tile([128, C6, T], FP8, tag="aT8")

                with tc.tile_pool(name="attA1", bufs=1) as attA1:
                    # ---- q/k feature-major, v token-major (fp8 DoubleRow)
                    ps1 = tc.tile_pool(name="ps1", bufs=2, space="PSUM")
                    psG = ps1.__enter__()
                    # interleave the wq / hx8 loads pair-by-pair so the
                    # first accumulation group can start ~7us sooner
                    wq_sb = lfw.tile([128, C6, D], FP8, tag="lfw")
                    for j in range(3):
                        nc.sync.dma_start(
                            wq_sb[:, 2 * j:2 * j + 2, :],
                            lfw_d["lfwq"].ap()[:, 2 * j * D:(2 * j + 2) * D])
                        nc.sync.dma_start(
                            hx8[:, 2 * j:2 * j + 2, :],
                            hx8_d.ap()[:, 2 * j * EXT:(2 * j + 2) * EXT])
                    qT = attA1.tile([128, C6, T], BF16, tag="qT")
                    for f in range(C6):
                        for nch in range(2):
                            ps = psG.tile([128, 512], F32, tag="g")
                            for j in range(3):
                                nc.tensor.matmul(
                                    ps,
                                    wq_sb[:, 2 * j:2 * j + 2,
                                          f * 128:(f + 1) * 128],
                                    hx8[:, 2 * j:2 * j + 2,
                                        256 + nch * 512:
                                        256 + (nch + 1) * 512],
                                    start=(j == 0), stop=(j == 2),
                                    perf_mode=DR)
                            nc.scalar.activation(
                                out=qT[:, f, nch * 512:(nch + 1) * 512],
                                in_=ps, func=ACTF.Copy, scale=DQ_QKV)
                    wk_sb = lfw.tile([128, C6, D], FP8, tag="lfw")
                    nc.sync.dma_start(wk_sb, lfw_d["lfwk"].ap())
                    # hxT (bf16 residual) only needed at wo; load now
                    nc.sync.dma_start(hxT, hxT_d.ap())
                    kT = attA1.tile([128, C6, EXT], BF16, tag="kT")
                    for f in range(C6):
                        for nch in range(3):
                            ps = psG.tile([128, 512], F32, tag="g")
                            for j in range(3):
                                nc.tensor.matmul(
                                    ps,
                                    wk_sb[:, 2 * j:2 * j + 2,
                                          f * 128:(f + 1) * 128],
                                    hx8[:, 2 * j:2 * j + 2,
                                        nch * 512:(nch + 1) * 512],
                                    start=(j == 0), stop=(j == 2),
                                    perf_mode=DR)
                            nc.scalar.activation(
                                out=kT[:, f, nch * 512:(nch + 1) * 512],
                                in_=ps, func=ACTF.Copy, scale=DQ_QKV)
                    wv_sb = lfw.tile([128, C6, D], FP8, tag="lfw")
                    nc.sync.dma_start(wv_sb, lfw_d["lfwv"].ap())
                    # values token-major in fp8 at x32 (PV runs in fp8)
                    vtok8 = attA1.tile([128, 12, D], FP8, tag="vtok8")
                    for t in range(12):
                        for (n0, nn) in ((0, 512), (512, 256)):
                            ps = psG.tile([128, 512], F32, tag="g")
                            for j in range(3):
                                nc.tensor.matmul(
                                    ps[:, :nn],
                                    hx8[:, 2 * j:2 * j + 2,
                                        t * 128:(t + 1) * 128],
                                    wv_sb[:, 2 * j:2 * j + 2, n0:n0 + nn],
                                    start=(j == 0), stop=(j == 2),
                                    perf_mode=DR)
                            nc.scalar.activation(
                                out=vtok8[:, t, n0:n0 + nn], in_=ps[:, :nn],
                                func=ACTF.Copy, scale=DQ_QKV * SC_AT)
                    ps1.__exit__(None, None, None)

                    # ---- sliding-window attention, k-major scores so the
                    # probs land contraction-ready (no transpose matmuls);
                    # PV in fp8 DoubleRow; softmax Z via a ones-row matmul,
                    # normalization via GPSIMD partition-broadcast + DVE.
                    ps2 = tc.tile_pool(name="ps2", bufs=2, space="PSUM")
                    psS = ps2.__enter__()
                    ps2c = tc.tile_pool(name="ps2c", bufs=2, space="PSUM")
                    psV = ps2c.__enter__()
                    ps2z = tc.tile_pool(name="ps2z", bufs=2, space="PSUM")
                    psZ1 = ps2z.__enter__()

                    ones8 = constg.tile([128, 2, 16], FP8, tag="ones8")
                    nc.vector.memset(ones8, 1.0)

                    m640_t = [None] * 8

                    def a_scores(qt, pair, h2):
                        if pair == 0 and h2 == 0:
                            m640_t[qt] = work.tile([128, 640], F32,
                                                   tag="m640", name="m640")
                            nc.sync.dma_start(m640_t[qt], m640_d.ap()[qt])
                        ps = psS.tile([128, 640], F32, tag="sc")
                        rhsq = qT[h2 * 64:(h2 + 1) * 64, pair,
                                  qt * 128:(qt + 1) * 128]
                        for dx in range(5):
                            nc.tensor.matmul(
                                ps[:, dx * 128:(dx + 1) * 128],
                                kT[h2 * 64:(h2 + 1) * 64, pair,
                                   qt * 128 + dx * 128:
                                   qt * 128 + (dx + 1) * 128],
                                rhsq, start=True, stop=True,
                                tile_position=(h2 * 64, 0))
                        sb = work.tile([128, 640], F32, tag="sb")
                        nc.vector.tensor_tensor(sb, ps, m640_t[qt], ALU.add)
                        probs8 = work.tile([128, 5, 128], FP8, tag="probs8")
                        nc.scalar.activation(out=probs8, in_=sb,
                                             func=ACTF.Exp, scale=ISQ_DH)
                        return probs8

                    def a_rest(qt, pair, h2, probs8):
                        h = 2 * pair + h2
                        zq = psZ1.tile([1, 128], F32, tag="zq")
                        pvt = psV.tile([128, 128], F32, tag="pv")
                        for i in range(2):
                            nc.tensor.matmul(
                                zq, ones8[:, :, 0:1],
                                probs8[:, 2 * i:2 * i + 2, :],
                                start=(i == 0), stop=False, perf_mode=DR)
                        nc.tensor.matmul(zq, ones8[:, 0, 0:1],
                                         probs8[:, 4, :],
                                         start=False, stop=True)
                        if h2 == 0:
                            # DoubleRow requires dst partition offset 0
                            for i in range(2):
                                nc.tensor.matmul(
                                    pvt[0:64, :],
                                    vtok8[:, qt + 2 * i:qt + 2 * i + 2,
                                          h * 64:(h + 1) * 64],
                                    probs8[:, 2 * i:2 * i + 2, :],
                                    start=(i == 0), stop=False, perf_mode=DR,
                                    tile_position=(0, 0))
                            nc.tensor.matmul(
                                pvt[0:64, :],
                                vtok8[:, qt + 4, h * 64:(h + 1) * 64],
                                probs8[:, 4, :], start=False, stop=True,
                                tile_position=(0, 0))
                        else:
                            for dx in range(5):
                                nc.tensor.matmul(
                                    pvt[64:128, :],
                                    vtok8[:, qt + dx, h * 64:(h + 1) * 64],
                                    probs8[:, dx, :], start=(dx == 0),
                                    stop=(dx == 4),
                                    tile_position=(0, 64))
                        rs = sm.tile([1, 128], F32, tag="rs")
                        nc.vector.reciprocal(rs, zq)
                        rs_bc = work.tile([128, 128], F32, tag="rs_bc")
                        nc.gpsimd.partition_broadcast(rs_bc, rs)
                        nc.vector.tensor_tensor(
                            aT8[h2 * 64:(h2 + 1) * 64, pair,
                                qt * 128:(qt + 1) * 128],
                            pvt[h2 * 64:(h2 + 1) * 64, :],
                            rs_bc[h2 * 64:(h2 + 1) * 64, :], ALU.mult)

                    its = [(qt, pair, h2) for qt in range(8)
                           for pair in range(6) for h2 in range(2)]
                    prev = None
                    for it in its:
                        probs8 = a_scores(*it)
                        if prev is not None:
                            a_rest(prev[0][0], prev[0][1], prev[0][2],
                                   prev[1])
                        prev = (it, probs8)
                    a_rest(prev[0][0], prev[0][1], prev[0][2], prev[1])

                    ps2z.__exit__(None, None, None)
                    ps2c.__exit__(None, None, None)
                    ps2.__exit__(None, None, None)

                # ---- wo + residual (feature-major, fp8 DoubleRow)
                ps3 = tc.tile_pool(name="ps3", bufs=2, space="PSUM")
                psG = ps3.__enter__()
                wo_sb = lfw.tile([128, C6, D], FP8, tag="lfw")
                nc.sync.dma_start(wo_sb, lfw_d["lfwo"].ap())
                r1T = mid.tile([128, C6, T], BF16, tag="resT")
                for f in range(C6):
                    for nch in range(2):
                        ps = psG.tile([128, 512], F32, tag="g")
                        for j in range(3):
                            nc.tensor.matmul(
                                ps,
                                wo_sb[:, 2 * j:2 * j + 2,
                                      f * 128:(f + 1) * 128],
                                aT8[:, 2 * j:2 * j + 2,
                                    nch * 512:(nch + 1) * 512],
                                start=(j == 0), stop=(j == 2), perf_mode=DR)
                        t0 = work.tile([128, 512], BF16, tag="t0")
                        nc.scalar.activation(out=t0, in_=ps, func=ACTF.Copy,
                                             scale=DQ_WO)
                        nc.vector.tensor_tensor(
                            r1T[:, f, nch * 512:(nch + 1) * 512], t0,
                            hxT[:, f, 256 + nch * 512: 256 + (nch + 1) * 512],
                            ALU.add)
                ps3.__exit__(None, None, None)

            # FFN weights: w1 stays resident across both FFN passes, in a
            # pool that reuses the space just freed by the attention pools.
            ffnp = tc.tile_pool(name="ffnp", bufs=1)
            ffnpo = ffnp.__enter__()
            w1sb = ffnpo.tile([128, KC, C6, 128], BF16, tag="w1sb")
            nc.sync.dma_start(w1sb, w1_d.ap())

            # ---- LN1 (transpose to token-major, LN, transpose back)
            # software-pipelined: forward transposes of t+1 issue before the
            # back transposes of t, so the PE isn't stalled by the LN chain
            ps3b = tc.tile_pool(name="ps3b", bufs=3, space="PSUM")
            psT = ps3b.__enter__()
            h1T = mid.tile([128, C6, T], BF16, tag="h1T")

            def ln1_fwd(t):
                tp = psT.tile([128, C6 * 128], BF16, tag="tp3")
                for c in range(C6):
                    nc.tensor.transpose(tp[:, c * 128:(c + 1) * 128],
                                        r1T[:, c, t * 128:(t + 1) * 128],
                                        id_bf)
                ltok = work.tile([128, D], BF16, tag="ltok")
                _ln_tile(nc, sm, tp[:, 0:384], tp[:, 384:768], tp, ltok,
                         eps_sb)
                return ltok

            def ln1_back(t, ltok):
                tp2 = psT.tile([128, C6, 128], BF16, tag="tp3b")
                for c in range(C6):
                    nc.tensor.transpose(tp2[:, c, :],
                                        ltok[:, c * 128:(c + 1) * 128],
                                        id_bf)
                nc.vector.tensor_copy(out=h1T[:, :, t * 128:(t + 1) * 128],
                                      in_=tp2)

            prevLs = []
            for t in range(8):
                ltok = ln1_fwd(t)
                prevLs.append((t, ltok))
                if len(prevLs) > 3:
                    ln1_back(*prevLs.pop(0))
            for pl in prevLs:
                ln1_back(*pl)
            ps3b.__exit__(None, None, None)

            # ---- FFN + LN2, one token-half at a time; each half's fp8
            # payload is exchanged as soon as it is ready so the second
            # half's FFN overlaps the first collective.
            r2T = mid.tile([128, C6, T], BF16, tag="resT")
            g1all = ffnpo.tile([128, KC, 512], BF16, tag="g1all")
            for nch in range(2):
                # g1 = gelu(h1 @ w1), all 24 dff chunks
                ps4 = tc.tile_pool(name="ps4", bufs=2, space="PSUM")
                psG = ps4.__enter__()
                for kc in range(KC):
                    g1p = psG.tile([128, 512], F32, tag="g")
                    for k in range(C6):
                        nc.tensor.matmul(
                            g1p, w1sb[:, kc, k, :],
                            h1T[:, k, nch * 512:(nch + 1) * 512],
                            start=(k == 0), stop=(k == 5))
                    nc.scalar.activation(out=g1all[:, kc, :], in_=g1p,
                                         func=ACTF.Gelu_apprx_tanh)
                ps4.__exit__(None, None, None)
                # f2 = g1 @ w2 (+ residual)
                ps4b = tc.tile_pool(name="ps4b", bufs=1, space="PSUM")
                psF = ps4b.__enter__()
                with tc.tile_pool(name="bigw", bufs=4) as bigw:
                    f2ps = [psF.tile([128, 512], F32, tag=f"f2_{m}",
                                     name=f"f2_{m}") for m in range(C6)]
                    for kc in range(KC):
                        w2c = bigw.tile([128, D], BF16, tag="w2c")
                        nc.sync.dma_start(
                            w2c, w2_d.ap()[kc * 128:(kc + 1) * 128, :])
                        for m in range(C6):
                            nc.tensor.matmul(
                                f2ps[m], w2c[:, m * 128:(m + 1) * 128],
                                g1all[:, kc, :],
                                start=(kc == 0), stop=(kc == KC - 1))
                    for m in range(C6):
                        nc.vector.tensor_tensor(
                            r2T[:, m, nch * 512:(nch + 1) * 512], f2ps[m],
                            h1T[:, m, nch * 512:(nch + 1) * 512], ALU.add)
                ps4b.__exit__(None, None, None)

                # LN2 for this half (pipelined like LN1)
                ps5 = tc.tile_pool(name="ps5", bufs=3, space="PSUM")
                psT5 = ps5.__enter__()

                def ln2_fwd(t):
                    tp = psT5.tile([128, C6 * 128], BF16, tag="tp5")
                    for c in range(C6):
                        nc.tensor.transpose(
                            tp[:, c * 128:(c + 1) * 128],
                            r2T[:, c, t * 128:(t + 1) * 128], id_bf)
                    otok = work.tile([128, D], BF16, tag="ltok")
                    _ln_tile(nc, sm, tp[:, 0:384], tp[:, 384:768], tp, otok,
                             eps_sb)
                    return otok

                def ln2_back(t, otok):
                    otok8 = work.tile([128, D], FP8, tag="otok8")
                    nc.scalar.activation(out=otok8, in_=otok, func=ACTF.Copy,
                                         scale=SC_OLF)
                    nc.sync.dma_start(
                        srcA[t // 4][(t % 4) * 128:(t % 4 + 1) * 128, :],
                        otok8)
                    tp2 = psT5.tile([128, C6, 128], BF16, tag="tp5b")
                    for c in range(C6):
                        nc.tensor.transpose(tp2[:, c, :],
                                            otok[:, c * 128:(c + 1) * 128],
                                            id_bf)
                    nc.vector.tensor_copy(
                        out=ownT[:, :, t * 128:(t + 1) * 128], in_=tp2)
                    nc.scalar.activation(
                        out=oT8[:, :, t * 128:(t + 1) * 128], in_=tp2,
                        func=ACTF.Copy, scale=SC_OLF)
                    if "tap_olf" in taps:
                        of = work.tile([128, D], F32, tag="tapolf")
                        nc.vector.tensor_copy(out=of, in_=otok)
                        nc.sync.dma_start(
                            taps["tap_olf"].ap()[t * 128:(t + 1) * 128, :],
                            of)

                prevTs = []
                for t in range(nch * 4, nch * 4 + 4):
                    otok = ln2_fwd(t)
                    prevTs.append((t, otok))
                    if len(prevTs) > 3:
                        ln2_back(*prevTs.pop(0))
                for pt in prevTs:
                    ln2_back(*pt)
                nc.sync.dma_start(
                    srcB[nch], oT8[:, :, nch * 512:(nch + 1) * 512])
                ps5.__exit__(None, None, None)

                # exchange this half right away
                nc.gpsimd.collective_compute(
                    "AllGather", ALU.bypass,
                    replica_groups=[[0, 1], [2, 3], [4, 5], [6, 7]],
                    ins=[srcA[nch][:].opt()], outs=[dstA[nch][:].opt()])
                nc.gpsimd.collective_compute(
                    "AllGather", ALU.bypass,
                    replica_groups=[[0, 1], [2, 3], [4, 5], [6, 7]],
                    ins=[srcB[nch][:].opt()], outs=[dstB[nch][:].opt()])
            ffnp.__exit__(None, None, None)

        # ============ STAGE B ============
        with tc.tile_pool(name="resB", bufs=1) as resB, \
             tc.tile_pool(name="whead", bufs=2) as whead, \
             tc.tile_pool(name="hb", bufs=2) as hb, \
             tc.tile_pool(name="workB", bufs=2) as workB, \
             tc.tile_pool(name="smB", bufs=4) as smB:

            ps6 = tc.tile_pool(name="psG2", bufs=2, space="PSUM")
            psG2 = ps6.__enter__()

            # mwB needed only for fc at the end; start the DMA early
            mwB8_sb = resB.tile([128, NH * C6, D], FP8, tag="mwB8")
            nc.sync.dma_start(mwB8_sb, mwB_d.ap())

            # --- pre-collective: q2 for all 4 heads from local oT8
            q2T8 = []
            for h in range(NH):
                wh8 = whead.tile([128, C6, D], FP8, tag="wh")
                nc.sync.dma_start(
                    wh8, mwA_d.ap()[:, h * C6 * D:(h + 1) * C6 * D])
                q2 = resB.tile([128, C6, T], FP8, tag=f"q2T8_{h}")
                for f in range(C6):
                    for nch in range(2):
                        ps = psG2.tile([128, 512], F32, tag="g2")
                        for j in range(3):
                            nc.tensor.matmul(
                                ps,
                                wh8[:, 2 * j:2 * j + 2,
                                    f * 128:(f + 1) * 128],
                                oT8[:, 2 * j:2 * j + 2,
                                    nch * 512:(nch + 1) * 512],
                                start=(j == 0), stop=(j == 2), perf_mode=DR)
                        nc.scalar.activation(
                            out=q2[:, f, nch * 512:(nch + 1) * 512],
                            in_=ps, func=ACTF.Copy, scale=DQ_Q2)
                q2T8.append(q2)

            # --- land collective results (no transposes needed)
            # global token chunk tt: half g = tt//8, sub-half s = (tt%8)//4
            olfT8 = resB.tile([128, C6, 2 * T], FP8, tag="olfT8")
            olftok8 = resB.tile([128, 16, 784], FP8, tag="olftok8")
            nc.vector.memset(olftok8[:, :, 768:769], ZCOL)
            for g in range(2):
                for s in range(2):
                    nc.sync.dma_start(
                        olfT8[:, :, g * T + s * 512:g * T + (s + 1) * 512],
                        dstB[s][g * 128:(g + 1) * 128, :])
            for tt in range(16):
                g, s, r = tt // 8, (tt % 8) // 4, tt % 4
                nc.sync.dma_start(
                    olftok8[:, tt, 0:768],
                    dstA[s][g * 512 + r * 128:g * 512 + (r + 1) * 128, :])

            poT8 = resB.tile([128, NH * C6, T], FP8, tag="poT8")
            fcacc = resB.tile([128, C6, T], BF16, tag="fcacc")

            ps7z = tc.tile_pool(name="psZ", bufs=2, space="PSUM")
            psZ = ps7z.__enter__()
            ps7b = tc.tile_pool(name="psBC", bufs=2, space="PSUM")
            psBC = ps7b.__enter__()
            ps7 = tc.tile_pool(name="psPV", bufs=2, space="PSUM")
            psPV = ps7.__enter__()

            def sc_issue(h, qch):
                expT8 = hb.tile([128, 16, 512], FP8, tag="expT8")
                for kt in range(16):
                    ps = psG2.tile([128, 512], F32, tag="g2")
                    for j in range(3):
                        nc.tensor.matmul(
                            ps,
                            olfT8[:, 2 * j:2 * j + 2,
                                  kt * 128:(kt + 1) * 128],
                            q2T8[h][:, 2 * j:2 * j + 2,
                                    qch * 512:(qch + 1) * 512],
                            start=(j == 0), stop=(j == 2), perf_mode=DR)
                    nc.scalar.activation(out=expT8[:, kt, :], in_=ps,
                                         func=ACTF.Exp,
                                         bias=kb16[:, kt:kt + 1],
                                         scale=ISQ_DK / 256.0)
                return expT8

            def pv_issue(h, qch, expT8):
                zp = psZ.tile([1, 512], F32, tag="z")
                for i in range(8):
                    nc.tensor.matmul(
                        zp, olftok8[:, 2 * i:2 * i + 2, 768:769],
                        expT8[:, 2 * i:2 * i + 2, :],
                        start=(i == 0), stop=(i == 7), perf_mode=DR)
                rs = smB.tile([1, 512], BF16, tag="rs2")
                with nc.allow_low_precision(
                        reason="1/Z feeds fp8-precision normalization"):
                    nc.vector.reciprocal(rs, zp)
                bc = psBC.tile([128, 512], F32, tag="bc")
                bc_sb = workB.tile([128, 512], BF16, tag="bc_sb")
                for c in range(C6):
                    pp = psPV.tile([128, 512], F32, tag="pv")
                    for i in range(8):
                        nc.tensor.matmul(
                            pp,
                            olftok8[:, 2 * i:2 * i + 2,
                                    c * 128:(c + 1) * 128],
                            expT8[:, 2 * i:2 * i + 2, :],
                            start=(i == 0), stop=(i == 7), perf_mode=DR)
                    if c == 0:
                        nc.tensor.matmul(bc, ones_bf, rs,
                                         start=True, stop=True)
                        nc.scalar.activation(out=bc_sb, in_=bc,
                                             func=ACTF.Copy)
                    nc.vector.tensor_tensor(
                        poT8[:, h * C6 + c, qch * 512:(qch + 1) * 512],
                        pp, bc_sb, ALU.mult)

            # head loop: sc issued one step ahead of pv
            prev = None
            for h in range(NH):
                for qch in range(2):
                    expT8 = sc_issue(h, qch)
                    if prev is not None:
                        pv_issue(*prev)
                    prev = (h, qch, expT8)
            pv_issue(*prev)

            ps7.__exit__(None, None, None)
            ps7b.__exit__(None, None, None)
            ps7z.__exit__(None, None, None)

            # --- output projection over concatenated heads + residual,
            # interleaved with the tail LN/max per token-half so the
            # serial LN chain overlaps the second half's fc matmuls
            ps8 = tc.tile_pool(name="psT2", bufs=3, space="PSUM")
            psT2 = ps8.__enter__()
            ps9 = tc.tile_pool(name="psTail", bufs=2, space="PSUM")
            psTail = ps9.__enter__()
            maxacc = resB.tile([128, D], F32, tag="maxacc")

            def tail_t(t):
                tp = psT2.tile([128, C6 * 128], BF16, tag="tpB")
                for c in range(C6):
                    nc.tensor.transpose(tp[:, c * 128:(c + 1) * 128],
                                        fcacc[:, c, t * 128:(t + 1) * 128],
                                        id_bf)
                ltok = workB.tile([128, D], F32, tag="ltokB")
                _ln_tile(nc, smB, tp[:, 0:384], tp[:, 384:768], tp, ltok,
                         eps_sb)
                if "tap_attn" in taps:
                    nc.sync.dma_start(
                        taps["tap_attn"].ap()[t * 128:(t + 1) * 128, :], ltok)
                if t == 0:
                    nc.vector.tensor_copy(out=maxacc, in_=ltok)
                else:
                    nc.vector.tensor_tensor(maxacc, maxacc, ltok, ALU.max)

            for nch in range(2):
                for m in range(C6):
                    ps = psG2.tile([128, 512], F32, tag="g2")
                    for j in range(NH * C6 // 2):
                        nc.tensor.matmul(
                            ps,
                            mwB8_sb[:, 2 * j:2 * j + 2,
                                    m * 128:(m + 1) * 128],
                            poT8[:, 2 * j:2 * j + 2,
                                 nch * 512:(nch + 1) * 512],
                            start=(j == 0), stop=(j == NH * C6 // 2 - 1),
                            perf_mode=DR)
                    t1 = workB.tile([128, 512], BF16, tag="t1")
                    nc.scalar.activation(out=t1, in_=ps, func=ACTF.Copy,
                                         scale=DQ_FC)
                    nc.vector.tensor_tensor(
                        fcacc[:, m, nch * 512:(nch + 1) * 512], t1,
                        ownT[:, m, nch * 512:(nch + 1) * 512], ALU.add)
                for t in range(nch * 4, nch * 4 + 4):
                    tail_t(t)
            outsb = resB.tile([128, 6], F32, tag="outsb")
            for c in range(C6):
                pt = psTail.tile([128, 128], F32, tag="tpf")
                nc.tensor.transpose(pt, maxacc[:, c * 128:(c + 1) * 128],
                                    id_f32)
                nc.vector.tensor_reduce(out=outsb[:, c:c + 1], in_=pt,
                                        axis=AX.X, op=ALU.max)
            nc.sync.dma_start(out_d.ap(), outsb)
            ps9.__exit__(None, None, None)
            ps8.__exit__(None, None, None)
            ps6.__exit__(None, None, None)

    return


# ---------------- host side ----------------

_NC_CACHE = {}


def _get_nc(debug=False):
    key = bool(debug)
    if key not in _NC_CACHE:
        _NC_CACHE[key] = build(debug=debug)
    return _NC_CACHE[key]


def _prep_in_maps(inputs):
    bf = ml_dtypes.bfloat16
    f8 = ml_dtypes.float8_e4m3
    x = np.asarray(inputs["x"])
    emb = np.asarray(inputs["emb"], np.float32)
    pos = np.asarray(inputs["pos"], np.float32)
    g_e = np.asarray(inputs["ln_e_g"], np.float32)
    b_e = np.asarray(inputs["ln_e_b"], np.float32)

    def parr(w):
        w = np.ascontiguousarray(
            np.asarray(w, np.float32).reshape(C6, 128, -1)
            .transpose(1, 0, 2)).astype(bf)
        return w.reshape(128, -1)

    def parr8(w, scale):
        w = np.clip(np.asarray(w, np.float32) * scale, -240.0, 240.0)
        w = np.ascontiguousarray(
            w.reshape(-1, 128, w.shape[-1]).transpose(1, 0, 2)).astype(f8)
        return w.reshape(128, -1)

    wts = {
        "lfwq": parr8(inputs["lf_wq"], SC_LFW),
        "lfwk": parr8(inputs["lf_wk"], SC_LFW),
        "lfwv": parr8(inputs["lf_wv"], SC_LFW),
        "lfwo": parr8(inputs["lf_wo"], SC_LFW),
    }
    w1 = np.asarray(inputs["w1"], np.float32)
    wts["w1"] = np.ascontiguousarray(
        w1.reshape(C6, 128, KC, 128).transpose(1, 2, 0, 3)
    ).astype(bf).reshape(128, KC * D)
    wts["w2"] = np.asarray(inputs["w2"], np.float32).astype(bf)

    # folded stage-B matrices, fp8 x256
    wq = np.asarray(inputs["mha_wq"], np.float32).reshape(D, NH, DK)
    wk = np.asarray(inputs["mha_wk"], np.float32).reshape(D, NH, DK)
    wv = np.asarray(inputs["mha_wv"], np.float32).reshape(D, NH, DK)
    fc = np.asarray(inputs["mha_fc"], np.float32).reshape(NH, DK, D)
    mwA = np.concatenate(
        [parr8(wq[:, h, :] @ wk[:, h, :].T, SC_W) for h in range(NH)], axis=1)
    Bcat = np.concatenate([wv[:, h, :] @ fc[h] for h in range(NH)], axis=0)
    wts["mwA"] = np.ascontiguousarray(mwA)
    wts["mwB"] = np.ascontiguousarray(parr8(Bcat, SC_W))

    in_maps = []
    for b in range(B):
        h0 = emb[x[b]] + pos                        # [S, D] f32
        mu = h0.mean(-1, keepdims=True)
        var = h0.var(-1, keepdims=True)
        hn = (h0 - mu) / np.sqrt(var + EPS) * g_e + b_e
        kbias = np.where(x[b] != 0, 0.0, NEG).astype(np.float32)
        kb16 = np.ascontiguousarray(kbias.reshape(16, 128).T)
        for p in range(2):
            start = p * T - 256
            hxe = np.zeros((EXT, D), np.float32)
            lo, hi = max(0, start), min(S, start + EXT)
            hxe[lo - start: hi - start] = hn[lo:hi]
            hxf = np.ascontiguousarray(
                hxe.reshape(EXT, C6, 128).transpose(2, 1, 0))
            hxT = hxf.astype(bf).reshape(128, C6 * EXT)
            hx8 = np.clip(hxf * SC_HX, -240.0, 240.0).astype(f8).reshape(
                128, C6 * EXT)

            qi = np.arange(128)
            kj = np.arange(640)
            m640 = np.zeros((8, 128, 640), np.float32)
            for qt in range(8):
                qg = p * T + qt * 128 + qi[:, None]
                kg = start + qt * 128 + kj[None, :]
                ok = (np.abs(kg - qg) <= W) & (kg >= 0) & (kg < S)
                # k-major: [key-in-chunk, dx-chunk, query]
                m640[qt] = np.ascontiguousarray(
                    np.where(ok, 0.0, NEG).T.reshape(5, 128, 128)
                    .transpose(1, 0, 2)).reshape(128, 640)

            m = {"hxT": hxT, "hx8": hx8, "m640": m640, "kb16": kb16}
            m.update(wts)
            in_maps.append(m)
    return in_maps


def _postprocess(results):
    out = np.zeros((B, D), np.float32)
    for b in range(B):
        m0 = np.asarray(results[2 * b]["out"]).T.reshape(D)
        m1 = np.asarray(results[2 * b + 1]["out"]).T.reshape(D)
        out[b] = np.maximum(m0, m1)
    return out


def run(inputs, debug=False, trace=False):
    nc = _get_nc(debug=debug)
    in_maps = _prep_in_maps(inputs)
    res = run_bass_kernel_spmd(nc, in_maps, core_ids=list(range(NCORES)),
                               trace=trace)
    return res


def kernel(**inputs):
    res = run(inputs, debug=False, trace=False)
    return _postprocess(res.results)


# revision 47
# speedup vs baseline: 1.4324x; 1.0020x over previous
"""Trainium2 Bass kernel for nn_LongformerEncoder (optimized v3).

Sharding: 8 cores = (batch b in 0..3, seq-half p in 0..1).
Stage A (longformer layer) runs on 1024 own tokens (+256-token halo).
A pairwise AllGather exchanges stage-A output; stage B (4-head/768-dim
MHA + max-pool) runs seq-split on queries with full keys, partial max
per core, final max across the pair on host.

v3 changes vs v2:
- Stage B entirely in fp8 (e4m3) with DoubleRow matmuls (2 contraction
  rows per PE pass): q2 projection, scores, PV, and the concatenated
  output projection. Attention contributes ~1.3% of the pre-LN signal,
  so fp8 error is negligible in the final output.
- The collective payload carries the stage-A output in fp8 in BOTH
  layouts (token-major for PV values, feature-major for score keys),
  eliminating all post-collective PE transposes in stage B.
- PV computed feature-major (lhsT = values chunk), so the attention
  output lands pre-transposed for the output projection; softmax
  denominator via a dedicated Z-column matmul, normalization via a
  ones-broadcast matmul + one DVE multiply per chunk.
- fc done once over the 4 heads' concatenated poT (single PSUM
  accumulation group; no inter-head DVE adds).
"""

import sys

sys.path.insert(0, "/opt/trn_rl_repo")

import numpy as np
import ml_dtypes

import concourse.bass as bass
import concourse.tile as tile
from concourse import bacc, mybir
from concourse.bass_utils import run_bass_kernel_spmd
from concourse.masks import make_identity

F32 = mybir.dt.float32
BF16 = mybir.dt.bfloat16
FP8 = mybir.dt.float8e4
AX = mybir.AxisListType
ALU = mybir.AluOpType
ACTF = mybir.ActivationFunctionType
DR = mybir.MatmulPerfMode.DoubleRow

B, S, D = 4, 2048, 768
W = 256
DFF = 3072
NH, DK = 4, 768
T = 1024            # own tokens per core
EXT = 1536          # own + 256 halo each side
NEG = -1e9
EPS = 1e-5
NCORES = 8
C6 = D // 128        # 6 feature chunks
KC = DFF // 128      # 24 dff chunks
ISQ_DH = 0.125       # 1/sqrt(64)
ISQ_DK = 1.0 / float(np.sqrt(DK))
SC_OLF = 16.0        # fp8 scale of stage-A output (both layouts)
SC_W = 256.0         # fp8 scale of folded stage-B weights
ZCOL = 1.0 / 16.0    # Z-helper column value so po lands at 256x true
DQ_Q2 = 1.0 / 256.0  # psum(16*256*q2) -> 16*q2
DQ_FC = 1.0 / 65536.0  # psum(256*256*fc) -> fc
SC_HX = 16.0         # fp8 scale of LN'd embeddings (stage-A input)
SC_LFW = 1024.0      # fp8 scale of longformer q/k/v/o weights
SC_AT = 32.0         # fp8 scale of stage-A attention output
DQ_QKV = 1.0 / (SC_HX * SC_LFW)
DQ_WO = 1.0 / (SC_AT * SC_LFW)


def build(debug=False):
    nc = bacc.Bacc("TRN2", target_bir_lowering=False, debug=False,
                   num_devices=NCORES)

    hxT_d = nc.dram_tensor("hxT", [128, C6 * EXT], BF16, kind="ExternalInput")
    hx8_d = nc.dram_tensor("hx8", [128, C6 * EXT], FP8, kind="ExternalInput")
    m640_d = nc.dram_tensor("m640", [8, 128, 640], F32, kind="ExternalInput")
    kb16_d = nc.dram_tensor("kb16", [128, 16], F32, kind="ExternalInput")
    lfw_d = {}
    for nm in ["lfwq", "lfwk", "lfwv", "lfwo"]:
        lfw_d[nm] = nc.dram_tensor(nm, [128, C6 * D], FP8,
                                   kind="ExternalInput")
    w1_d = nc.dram_tensor("w1", [128, KC * D], BF16, kind="ExternalInput")
    w2_d = nc.dram_tensor("w2", [DFF, D], BF16, kind="ExternalInput")
    mwA_d = nc.dram_tensor("mwA", [128, NH * C6 * D], FP8,
                           kind="ExternalInput")
    mwB_d = nc.dram_tensor("mwB", [128, NH * C6 * D], FP8,
                           kind="ExternalInput")
    out_d = nc.dram_tensor("out", [128, 6], F32, kind="ExternalOutput")
    taps = {}
    if debug:
        taps["tap_olf"] = nc.dram_tensor("tap_olf", [T, D], F32,
                                         kind="ExternalOutput")
        taps["tap_attn"] = nc.dram_tensor("tap_attn", [T, D], F32,
                                          kind="ExternalOutput")

    with tile.TileContext(nc) as tc:
        _body(nc, tc, hxT_d, hx8_d, m640_d, kb16_d, lfw_d, w1_d, w2_d,
              mwA_d, mwB_d, out_d, taps)
    nc.compile()
    return nc


def _ln_tile(nc, pool, xh0, xh1, xfull, out_tile, eps_ap):
    """out = (x - mean)/sqrt(var+eps) over 768 features. g==1, b==0.

    Reads the input (typically the PSUM transpose tile, no SBUF copy
    needed) in two halves for bn_stats; the wide normalize runs on the
    scalar engine (out = x*rstd - mu*rstd) so the DVE only carries the
    stats chain.
    """
    stats = pool.tile([128, 2, 6], F32, tag="lnstats")
    nc.vector.bn_stats(out=stats[:, 0, :], in_=xh0)
    nc.vector.bn_stats(out=stats[:, 1, :], in_=xh1)
    mv = pool.tile([128, 2], F32, tag="lnmv")
    nc.vector.bn_aggr(out=mv, in_=stats)
    rstd = pool.tile([128, 1], F32, tag="lnrstd")
    nc.scalar.activation(out=rstd, in_=mv[:, 1:2], func=ACTF.Sqrt, bias=eps_ap)
    nc.vector.reciprocal(out=rstd, in_=rstd)
    nb = pool.tile([128, 1], F32, tag="lnnb")
    nc.vector.tensor_scalar(out=nb, in0=mv[:, 0:1], scalar1=rstd,
                            scalar2=-1.0, op0=ALU.mult, op1=ALU.mult)
    nc.scalar.activation(out=out_tile, in_=xfull, func=ACTF.Identity,
                         scale=rstd, bias=nb)


def _body(nc, tc, hxT_d, hx8_d, m640_d, kb16_d, lfw_d, w1_d, w2_d,
          mwA_d, mwB_d, out_d, taps):
    import contextlib
    ctx = contextlib.ExitStack()
    with ctx:
        constg = ctx.enter_context(tc.tile_pool(name="constg", bufs=1))
        outer = ctx.enter_context(tc.tile_pool(name="outer", bufs=1))
        dram = ctx.enter_context(tc.tile_pool(name="dram", bufs=1,
                                              space="DRAM"))

        id_bf = constg.tile([128, 128], BF16, tag="id_bf")
        make_identity(nc, id_bf)
        id_f32 = constg.tile([128, 128], F32, tag="id_f32")
        make_identity(nc, id_f32)
        eps_sb = constg.tile([128, 1], F32, tag="eps")
        nc.vector.memset(eps_sb, EPS)
        kb16 = constg.tile([128, 16], F32, tag="kb16")
        nc.sync.dma_start(kb16, kb16_d.ap())
        ones_bf = constg.tile([1, 128], BF16, tag="ones_bf")
        nc.vector.memset(ones_bf, 1.0)

        # cross-stage tiles (stage-A output for stage B)
        ownT = outer.tile([128, C6, T], BF16, tag="ownT")    # 12K/part
        oT8 = outer.tile([128, C6, T], FP8, tag="oT8")       # 6K/part

        # DRAM bounce for the collectives (fp8, both layouts, split in
        # two token-halves so the first exchange overlaps the second
        # half's FFN)
        srcA = [dram.tile([T // 2, D], FP8, name=f"srcA{i}")
                for i in range(2)]
        dstA = [dram.tile([T, D], FP8, name=f"dstA{i}") for i in range(2)]
        srcB = [dram.tile([128, C6 * 512], FP8, name=f"srcB{i}")
                for i in range(2)]
        dstB = [dram.tile([256, C6 * 512], FP8, name=f"dstB{i}")
                for i in range(2)]

        # ============ STAGE A ============
        with tc.tile_pool(name="mid", bufs=1) as mid, \
             tc.tile_pool(name="sm", bufs=4) as sm, \
             tc.tile_pool(name="work", bufs=4) as work:

            with tc.tile_pool(name="inA", bufs=1) as inA, \
                 tc.tile_pool(name="attA2", bufs=1) as attA2, \
                 tc.tile_pool(name="lfw", bufs=2) as lfw:

                hx8 = inA.tile([128, C6, EXT], FP8, tag="hx8")
                hxT = inA.tile([128, C6, EXT], BF16, tag="hxT")
                aT8 = attA2.tile([128, C6, T], FP8, tag="aT8")

                with tc.tile_pool(name="attA1", bufs=1) as attA1:
                    # ---- q/k feature-major, v token-major (fp8 DoubleRow)
                    ps1 = tc.tile_pool(name="ps1", bufs=2, space="PSUM")
                    psG = ps1.__enter__()
                    # interleave the wq / hx8 loads pair-by-pair so the
                    # first accumulation group can start ~7us sooner
                    wq_sb = lfw.tile([128, C6, D], FP8, tag="lfw")
                    for j in range(3):
                        nc.sync.dma_start(
                            wq_sb[:, 2 * j:2 * j + 2, :],
                            lfw_d["lfwq"].ap()[:, 2 * j * D:(2 * j + 2) * D])
                        nc.sync.dma_start(
                            hx8[:, 2 * j:2 * j + 2, :],
                            hx8_d.ap()[:, 2 * j * EXT:(2 * j + 2) * EXT])
                    qT = attA1.tile([128, C6, T], BF16, tag="qT")
                    for f in range(C6):
                        for nch in range(2):
                            ps = psG.tile([128, 512], F32, tag="g")
                            for j in range(3):
                                nc.tensor.matmul(
                                    ps,
                                    wq_sb[:, 2 * j:2 * j + 2,
                                          f * 128:(f + 1) * 128],
                                    hx8[:, 2 * j:2 * j + 2,
                                        256 + nch * 512:
                                        256 + (nch + 1) * 512],
                                    start=(j == 0), stop=(j == 2),
                                    perf_mode=DR)
                            nc.scalar.activation(
                                out=qT[:, f, nch * 512:(nch + 1) * 512],
                                in_=ps, func=ACTF.Copy, scale=DQ_QKV)
                    wk_sb = lfw.tile([128, C6, D], FP8, tag="lfw")
                    nc.sync.dma_start(wk_sb, lfw_d["lfwk"].ap())
                    # hxT (bf16 residual) only needed at wo; load now
                    nc.sync.dma_start(hxT, hxT_d.ap())
                    kT = attA1.tile([128, C6, EXT], BF16, tag="kT")
                    for f in range(C6):
                        for nch in range(3):
                            ps = psG.tile([128, 512], F32, tag="g")
                            for j in range(3):
                                nc.tensor.matmul(
                                    ps,
                                    wk_sb[:, 2 * j:2 * j + 2,
                                          f * 128:(f + 1) * 128],
                                    hx8[:, 2 * j:2 * j + 2,
                                        nch * 512:(nch + 1) * 512],
                                    start=(j == 0), stop=(j == 2),
                                    perf_mode=DR)
                            nc.scalar.activation(
                                out=kT[:, f, nch * 512:(nch + 1) * 512],
                                in_=ps, func=ACTF.Copy, scale=DQ_QKV)
                    wv_sb = lfw.tile([128, C6, D], FP8, tag="lfw")
                    nc.sync.dma_start(wv_sb, lfw_d["lfwv"].ap())
                    # values token-major in fp8 at x32 (PV runs in fp8)
                    vtok8 = attA1.tile([128, 12, D], FP8, tag="vtok8")
                    for t in range(12):
                        for (n0, nn) in ((0, 512), (512, 256)):
                            ps = psG.tile([128, 512], F32, tag="g")
                            for j in range(3):
                                nc.tensor.matmul(
                                    ps[:, :nn],
                                    hx8[:, 2 * j:2 * j + 2,
                                        t * 128:(t + 1) * 128],
                                    wv_sb[:, 2 * j:2 * j + 2, n0:n0 + nn],
                                    start=(j == 0), stop=(j == 2),
                                    perf_mode=DR)
                            nc.scalar.activation(
                                out=vtok8[:, t, n0:n0 + nn], in_=ps[:, :nn],
                                func=ACTF.Copy, scale=DQ_QKV * SC_AT)
                    ps1.__exit__(None, None, None)

                    # ---- sliding-window attention, k-major scores so the
                    # probs land contraction-ready (no transpose matmuls);
                    # PV in fp8 DoubleRow; softmax Z via a ones-row matmul,
                    # normalization via GPSIMD partition-broadcast + DVE.
                    ps2 = tc.tile_pool(name="ps2", bufs=2, space="PSUM")
                    psS = ps2.__enter__()
                    ps2c = tc.tile_pool(name="ps2c", bufs=2, space="PSUM")
                    psV = ps2c.__enter__()
                    ps2z = tc.tile_pool(name="ps2z", bufs=2, space="PSUM")
                    psZ1 = ps2z.__enter__()

                    ones8 = constg.tile([128, 2, 16], FP8, tag="ones8")
                    nc.vector.memset(ones8, 1.0)

                    m640_t = [None] * 8

                    def a_scores(qt, pair, h2):
                        if pair == 0 and h2 == 0:
                            m640_t[qt] = work.tile([128, 640], F32,
                                                   tag="m640", name="m640")
                            nc.sync.dma_start(m640_t[qt], m640_d.ap()[qt])
                        ps = psS.tile([128, 640], F32, tag="sc")
                        rhsq = qT[h2 * 64:(h2 + 1) * 64, pair,
                                  qt * 128:(qt + 1) * 128]
                        for dx in range(5):
                            nc.tensor.matmul(
                                ps[:, dx * 128:(dx + 1) * 128],
                                kT[h2 * 64:(h2 + 1) * 64, pair,
                                   qt * 128 + dx * 128:
                                   qt * 128 + (dx + 1) * 128],
                                rhsq, start=True, stop=True,
                                tile_position=(h2 * 64, 0))
                        sb = work.tile([128, 640], F32, tag="sb")
                        nc.vector.tensor_tensor(sb, ps, m640_t[qt], ALU.add)
                        probs8 = work.tile([128, 5, 128], FP8, tag="probs8")
                        nc.scalar.activation(out=probs8, in_=sb,
                                             func=ACTF.Exp, scale=ISQ_DH)
                        return probs8

                    def a_rest(qt, pair, h2, probs8):
                        h = 2 * pair + h2
                        zq = psZ1.tile([1, 128], F32, tag="zq")
                        pvt = psV.tile([128, 128], F32, tag="pv")
                        for i in range(2):
                            nc.tensor.matmul(
                                zq, ones8[:, :, 0:1],
                                probs8[:, 2 * i:2 * i + 2, :],
                                start=(i == 0), stop=False, perf_mode=DR)
                        nc.tensor.matmul(zq, ones8[:, 0, 0:1],
                                         probs8[:, 4, :],
                                         start=False, stop=True)
                        if h2 == 0:
                            # DoubleRow requires dst partition offset 0
                            for i in range(2):
                                nc.tensor.matmul(
                                    pvt[0:64, :],
                                    vtok8[:, qt + 2 * i:qt + 2 * i + 2,
                                          h * 64:(h + 1) * 64],
                                    probs8[:, 2 * i:2 * i + 2, :],
                                    start=(i == 0), stop=False, perf_mode=DR,
                                    tile_position=(0, 0))
                            nc.tensor.matmul(
                                pvt[0:64, :],
                                vtok8[:, qt + 4, h * 64:(h + 1) * 64],
                                probs8[:, 4, :], start=False, stop=True,
                                tile_position=(0, 0))
                        else:
                            for dx in range(5):
                                nc.tensor.matmul(
                                    pvt[64:128, :],
                                    vtok8[:, qt + dx, h * 64:(h + 1) * 64],
                                    probs8[:, dx, :], start=(dx == 0),
                                    stop=(dx == 4),
                                    tile_position=(0, 64))
                        rs = sm.tile([1, 128], F32, tag="rs")
                        nc.vector.reciprocal(rs, zq)
                        rs_bc = work.tile([128, 128], F32, tag="rs_bc")
                        nc.gpsimd.partition_broadcast(rs_bc, rs)
                        nc.vector.tensor_tensor(
                            aT8[h2 * 64:(h2 + 1) * 64, pair,
                                qt * 128:(qt + 1) * 128],
                            pvt[h2 * 64:(h2 + 1) * 64, :],
                            rs_bc[h2 * 64:(h2 + 1) * 64, :], ALU.mult)

                    its = [(qt, pair, h2) for qt in range(8)
                           for pair in range(6) for h2 in range(2)]
                    prev = None
                    for it in its:
                        probs8 = a_scores(*it)
                        if prev is not None:
                            a_rest(prev[0][0], prev[0][1], prev[0][2],
                                   prev[1])
                        prev = (it, probs8)
                    a_rest(prev[0][0], prev[0][1], prev[0][2], prev[1])

                    ps2z.__exit__(None, None, None)
                    ps2c.__exit__(None, None, None)
                    ps2.__exit__(None, None, None)

                # ---- wo + residual (feature-major, fp8 DoubleRow)
                ps3 = tc.tile_pool(name="ps3", bufs=2, space="PSUM")
                psG = ps3.__enter__()
                wo_sb = lfw.tile([128, C6, D], FP8, tag="lfw")
                nc.sync.dma_start(wo_sb, lfw_d["lfwo"].ap())
                r1T = mid.tile([128, C6, T], BF16, tag="resT")
                for f in range(C6):
                    for nch in range(2):
                        ps = psG.tile([128, 512], F32, tag="g")
                        for j in range(3):
                            nc.tensor.matmul(
                                ps,
                                wo_sb[:, 2 * j:2 * j + 2,
                                      f * 128:(f + 1) * 128],
                                aT8[:, 2 * j:2 * j + 2,
                                    nch * 512:(nch + 1) * 512],
                                start=(j == 0), stop=(j == 2), perf_mode=DR)
                        t0 = work.tile([128, 512], BF16, tag="t0")
                        nc.scalar.activation(out=t0, in_=ps, func=ACTF.Copy,
                                             scale=DQ_WO)
                        nc.vector.tensor_tensor(
                            r1T[:, f, nch * 512:(nch + 1) * 512], t0,
                            hxT[:, f, 256 + nch * 512: 256 + (nch + 1) * 512],
                            ALU.add)
                ps3.__exit__(None, None, None)

            # FFN weights: w1 stays resident across both FFN passes, in a
            # pool that reuses the space just freed by the attention pools.
            ffnp = tc.tile_pool(name="ffnp", bufs=1)
            ffnpo = ffnp.__enter__()
            w1sb = ffnpo.tile([128, KC, C6, 128], BF16, tag="w1sb")
            nc.sync.dma_start(w1sb, w1_d.ap())

            # FFN g1 PSUM pool opens before LN1 so its banks stay
            # disjoint from the LN pools and the first g1 matmul of each
            # half never waits on an LN bank drain
            ps4 = tc.tile_pool(name="ps4", bufs=2, space="PSUM")
            psG = ps4.__enter__()

            # ---- LN1 (transpose to token-major, LN, transpose back)
            # software-pipelined: forward transposes of t+1 issue before the
            # back transposes of t, so the PE isn't stalled by the LN chain
            ps3b = tc.tile_pool(name="ps3b", bufs=3, space="PSUM")
            psT = ps3b.__enter__()
            h1T = mid.tile([128, C6, T], BF16, tag="h1T")

            def ln1_fwd(t):
                tp = psT.tile([128, C6 * 128], BF16, tag="tp3")
                for c in range(C6):
                    nc.tensor.transpose(tp[:, c * 128:(c + 1) * 128],
                                        r1T[:, c, t * 128:(t + 1) * 128],
                                        id_bf)
                ltok = work.tile([128, D], BF16, tag="ltok")
                _ln_tile(nc, sm, tp[:, 0:384], tp[:, 384:768], tp, ltok,
                         eps_sb)
                return ltok

            def ln1_back(t, ltok):
                tp2 = psT.tile([128, C6, 128], BF16, tag="tp3b")
                for c in range(C6):
                    nc.tensor.transpose(tp2[:, c, :],
                                        ltok[:, c * 128:(c + 1) * 128],
                                        id_bf)
                nc.vector.tensor_copy(out=h1T[:, :, t * 128:(t + 1) * 128],
                                      in_=tp2)

            prevLs = []
            for t in range(8):
                ltok = ln1_fwd(t)
                prevLs.append((t, ltok))
                if len(prevLs) > 3:
                    ln1_back(*prevLs.pop(0))
            for pl in prevLs:
                ln1_back(*pl)
            ps3b.__exit__(None, None, None)

            # ---- FFN + LN2, one token-half at a time; each half's fp8
            # payload is exchanged as soon as it is ready so the second
            # half's FFN overlaps the first collective.
            r2T = mid.tile([128, C6, T], BF16, tag="resT")
            g1all = ffnpo.tile([128, KC, 512], BF16, tag="g1all")
            for nch in range(2):
                # g1 = gelu(h1 @ w1), all 24 dff chunks
                for kc in range(KC):
                    g1p = psG.tile([128, 512], F32, tag="g")
                    for k in range(C6):
                        nc.tensor.matmul(
                            g1p, w1sb[:, kc, k, :],
                            h1T[:, k, nch * 512:(nch + 1) * 512],
                            start=(k == 0), stop=(k == 5))
                    nc.scalar.activation(out=g1all[:, kc, :], in_=g1p,
                                         func=ACTF.Gelu_apprx_tanh)
                # f2 = g1 @ w2 (+ residual)
                ps4b = tc.tile_pool(name="ps4b", bufs=1, space="PSUM")
                psF = ps4b.__enter__()
                with tc.tile_pool(name="bigw", bufs=4) as bigw:
                    f2ps = [psF.tile([128, 512], F32, tag=f"f2_{m}",
                                     name=f"f2_{m}") for m in range(C6)]
                    for kc in range(KC):
                        w2c = bigw.tile([128, D], BF16, tag="w2c")
                        nc.sync.dma_start(
                            w2c, w2_d.ap()[kc * 128:(kc + 1) * 128, :])
                        for m in range(C6):
                            nc.tensor.matmul(
                                f2ps[m], w2c[:, m * 128:(m + 1) * 128],
                                g1all[:, kc, :],
                                start=(kc == 0), stop=(kc == KC - 1))
                    for m in range(C6):
                        nc.vector.tensor_tensor(
                            r2T[:, m, nch * 512:(nch + 1) * 512], f2ps[m],
                            h1T[:, m, nch * 512:(nch + 1) * 512], ALU.add)
                ps4b.__exit__(None, None, None)

                # LN2 for this half (pipelined like LN1)
                ps5 = tc.tile_pool(name="ps5", bufs=3, space="PSUM")
                psT5 = ps5.__enter__()

                def ln2_fwd(t):
                    tp = psT5.tile([128, C6 * 128], BF16, tag="tp5")
                    for c in range(C6):
                        nc.tensor.transpose(
                            tp[:, c * 128:(c + 1) * 128],
                            r2T[:, c, t * 128:(t + 1) * 128], id_bf)
                    otok = work.tile([128, D], BF16, tag="ltok")
                    _ln_tile(nc, sm, tp[:, 0:384], tp[:, 384:768], tp, otok,
                             eps_sb)
                    return otok

                def ln2_back(t, otok):
                    otok8 = work.tile([128, D], FP8, tag="otok8")
                    nc.scalar.activation(out=otok8, in_=otok, func=ACTF.Copy,
                                         scale=SC_OLF)
                    nc.sync.dma_start(
                        srcA[t // 4][(t % 4) * 128:(t % 4 + 1) * 128, :],
                        otok8)
                    tp2 = psT5.tile([128, C6, 128], BF16, tag="tp5b")
                    for c in range(C6):
                        nc.tensor.transpose(tp2[:, c, :],
                                            otok[:, c * 128:(c + 1) * 128],
                                            id_bf)
                    nc.vector.tensor_copy(
                        out=ownT[:, :, t * 128:(t + 1) * 128], in_=tp2)
                    nc.scalar.activation(
                        out=oT8[:, :, t * 128:(t + 1) * 128], in_=tp2,
                        func=ACTF.Copy, scale=SC_OLF)
                    if "tap_olf" in taps:
                        of = work.tile([128, D], F32, tag="tapolf")
                        nc.vector.tensor_copy(out=of, in_=otok)
                        nc.sync.dma_start(
                            taps["tap_olf"].ap()[t * 128:(t + 1) * 128, :],
                            of)

                prevTs = []
                for t in range(nch * 4, nch * 4 + 4):
                    otok = ln2_fwd(t)
                    prevTs.append((t, otok))
                    if len(prevTs) > 3:
                        ln2_back(*prevTs.pop(0))
                for pt in prevTs:
                    ln2_back(*pt)
                nc.sync.dma_start(
                    srcB[nch], oT8[:, :, nch * 512:(nch + 1) * 512])
                ps5.__exit__(None, None, None)

                # exchange this half right away
                nc.gpsimd.collective_compute(
                    "AllGather", ALU.bypass,
                    replica_groups=[[0, 1], [2, 3], [4, 5], [6, 7]],
                    ins=[srcA[nch][:].opt()], outs=[dstA[nch][:].opt()])
                nc.gpsimd.collective_compute(
                    "AllGather", ALU.bypass,
                    replica_groups=[[0, 1], [2, 3], [4, 5], [6, 7]],
                    ins=[srcB[nch][:].opt()], outs=[dstB[nch][:].opt()])
            ps4.__exit__(None, None, None)
            ffnp.__exit__(None, None, None)

        # ============ STAGE B ============
        with tc.tile_pool(name="resB", bufs=1) as resB, \
             tc.tile_pool(name="whead", bufs=2) as whead, \
             tc.tile_pool(name="hb", bufs=2) as hb, \
             tc.tile_pool(name="workB", bufs=2) as workB, \
             tc.tile_pool(name="smB", bufs=4) as smB:

            ps6 = tc.tile_pool(name="psG2", bufs=2, space="PSUM")
            psG2 = ps6.__enter__()

            # mwB needed only for fc at the end; start the DMA early
            mwB8_sb = resB.tile([128, NH * C6, D], FP8, tag="mwB8")
            nc.sync.dma_start(mwB8_sb, mwB_d.ap())

            # --- pre-collective: q2 for all 4 heads from local oT8
            q2T8 = []
            for h in range(NH):
                wh8 = whead.tile([128, C6, D], FP8, tag="wh")
                nc.sync.dma_start(
                    wh8, mwA_d.ap()[:, h * C6 * D:(h + 1) * C6 * D])
                q2 = resB.tile([128, C6, T], FP8, tag=f"q2T8_{h}")
                for f in range(C6):
                    for nch in range(2):
                        ps = psG2.tile([128, 512], F32, tag="g2")
                        for j in range(3):
                            nc.tensor.matmul(
                                ps,
                                wh8[:, 2 * j:2 * j + 2,
                                    f * 128:(f + 1) * 128],
                                oT8[:, 2 * j:2 * j + 2,
                                    nch * 512:(nch + 1) * 512],
                                start=(j == 0), stop=(j == 2), perf_mode=DR)
                        nc.scalar.activation(
                            out=q2[:, f, nch * 512:(nch + 1) * 512],
                            in_=ps, func=ACTF.Copy, scale=DQ_Q2)
                q2T8.append(q2)

            # --- land collective results (no transposes needed)
            # global token chunk tt: half g = tt//8, sub-half s = (tt%8)//4
            olfT8 = resB.tile([128, C6, 2 * T], FP8, tag="olfT8")
            olftok8 = resB.tile([128, 16, 784], FP8, tag="olftok8")
            nc.vector.memset(olftok8[:, :, 768:769], ZCOL)
            for g in range(2):
                for s in range(2):
                    nc.sync.dma_start(
                        olfT8[:, :, g * T + s * 512:g * T + (s + 1) * 512],
                        dstB[s][g * 128:(g + 1) * 128, :])
            for tt in range(16):
                g, s, r = tt // 8, (tt % 8) // 4, tt % 4
                nc.sync.dma_start(
                    olftok8[:, tt, 0:768],
                    dstA[s][g * 512 + r * 128:g * 512 + (r + 1) * 128, :])

            poT8 = resB.tile([128, NH * C6, T], FP8, tag="poT8")
            fcacc = resB.tile([128, C6, T], BF16, tag="fcacc")

            ps7z = tc.tile_pool(name="psZ", bufs=2, space="PSUM")
            psZ = ps7z.__enter__()
            ps7b = tc.tile_pool(name="psBC", bufs=2, space="PSUM")
            psBC = ps7b.__enter__()
            ps7 = tc.tile_pool(name="psPV", bufs=2, space="PSUM")
            psPV = ps7.__enter__()

            def sc_issue(h, qch):
                expT8 = hb.tile([128, 16, 512], FP8, tag="expT8")
                for kt in range(16):
                    ps = psG2.tile([128, 512], F32, tag="g2")
                    for j in range(3):
                        nc.tensor.matmul(
                            ps,
                            olfT8[:, 2 * j:2 * j + 2,
                                  kt * 128:(kt + 1) * 128],
                            q2T8[h][:, 2 * j:2 * j + 2,
                                    qch * 512:(qch + 1) * 512],
                            start=(j == 0), stop=(j == 2), perf_mode=DR)
                    nc.scalar.activation(out=expT8[:, kt, :], in_=ps,
                                         func=ACTF.Exp,
                                         bias=kb16[:, kt:kt + 1],
                                         scale=ISQ_DK / 256.0)
                return expT8

            def pv_issue(h, qch, expT8):
                zp = psZ.tile([1, 512], F32, tag="z")
                for i in range(8):
                    nc.tensor.matmul(
                        zp, olftok8[:, 2 * i:2 * i + 2, 768:769],
                        expT8[:, 2 * i:2 * i + 2, :],
                        start=(i == 0), stop=(i == 7), perf_mode=DR)
                rs = smB.tile([1, 512], BF16, tag="rs2")
                with nc.allow_low_precision(
                        reason="1/Z feeds fp8-precision normalization"):
                    nc.vector.reciprocal(rs, zp)
                bc = psBC.tile([128, 512], F32, tag="bc")
                bc_sb = workB.tile([128, 512], BF16, tag="bc_sb")
                for c in range(C6):
                    pp = psPV.tile([128, 512], F32, tag="pv")
                    for i in range(8):
                        nc.tensor.matmul(
                            pp,
                            olftok8[:, 2 * i:2 * i + 2,
                                    c * 128:(c + 1) * 128],
                            expT8[:, 2 * i:2 * i + 2, :],
                            start=(i == 0), stop=(i == 7), perf_mode=DR)
                    if c == 0:
                        nc.tensor.matmul(bc, ones_bf, rs,
                                         start=True, stop=True)
                        nc.scalar.activation(out=bc_sb, in_=bc,
                                             func=ACTF.Copy)
                    nc.vector.tensor_tensor(
                        poT8[:, h * C6 + c, qch * 512:(qch + 1) * 512],
                        pp, bc_sb, ALU.mult)

            # head loop: sc issued one step ahead of pv
            prev = None
            for h in range(NH):
                for qch in range(2):
                    expT8 = sc_issue(h, qch)
                    if prev is not None:
                        pv_issue(*prev)
                    prev = (h, qch, expT8)
            pv_issue(*prev)

            ps7.__exit__(None, None, None)
            ps7b.__exit__(None, None, None)
            ps7z.__exit__(None, None, None)

            # --- output projection over concatenated heads + residual,
            # interleaved with the tail LN/max per token-half so the
            # serial LN chain overlaps the second half's fc matmuls
            ps8 = tc.tile_pool(name="psT2", bufs=3, space="PSUM")
            psT2 = ps8.__enter__()
            ps9 = tc.tile_pool(name="psTail", bufs=2, space="PSUM")
            psTail = ps9.__enter__()
            maxacc = resB.tile([128, D], F32, tag="maxacc")

            def tail_t(t):
                tp = psT2.tile([128, C6 * 128], BF16, tag="tpB")
                for c in range(C6):
                    nc.tensor.transpose(tp[:, c * 128:(c + 1) * 128],
                                        fcacc[:, c, t * 128:(t + 1) * 128],
                                        id_bf)
                ltok = workB.tile([128, D], F32, tag="ltokB")
                _ln_tile(nc, smB, tp[:, 0:384], tp[:, 384:768], tp, ltok,
                         eps_sb)
                if "tap_attn" in taps:
                    nc.sync.dma_start(
                        taps["tap_attn"].ap()[t * 128:(t + 1) * 128, :], ltok)
                if t == 0:
                    nc.vector.tensor_copy(out=maxacc, in_=ltok)
                else:
                    nc.vector.tensor_tensor(maxacc, maxacc, ltok, ALU.max)

            for nch in range(2):
                for m in range(C6):
                    ps = psG2.tile([128, 512], F32, tag="g2")
                    for j in range(NH * C6 // 2):
                        nc.tensor.matmul(
                            ps,
                            mwB8_sb[:, 2 * j:2 * j + 2,
                                    m * 128:(m + 1) * 128],
                            poT8[:, 2 * j:2 * j + 2,
                                 nch * 512:(nch + 1) * 512],
                            start=(j == 0), stop=(j == NH * C6 // 2 - 1),
                            perf_mode=DR)
                    t1 = workB.tile([128, 512], BF16, tag="t1")
                    nc.scalar.activation(out=t1, in_=ps, func=ACTF.Copy,
                                         scale=DQ_FC)
                    nc.vector.tensor_tensor(
                        fcacc[:, m, nch * 512:(nch + 1) * 512], t1,
                        ownT[:, m, nch * 512:(nch + 1) * 512], ALU.add)
                for t in range(nch * 4, nch * 4 + 4):
                    tail_t(t)
            outsb = resB.tile([128, 6], F32, tag="outsb")
            for c in range(C6):
                pt = psTail.tile([128, 128], F32, tag="tpf")
                nc.tensor.transpose(pt, maxacc[:, c * 128:(c + 1) * 128],
                                    id_f32)
                nc.vector.tensor_reduce(out=outsb[:, c:c + 1], in_=pt,
                                        axis=AX.X, op=ALU.max)
            nc.sync.dma_start(out_d.ap(), outsb)
            ps9.__exit__(None, None, None)
            ps8.__exit__(None, None, None)
            ps6.__exit__(None, None, None)

    return


# ---------------- host side ----------------

_NC_CACHE = {}


def _get_nc(debug=False):
    key = bool(debug)
    if key not in _NC_CACHE:
        _NC_CACHE[key] = build(debug=debug)
    return _NC_CACHE[key]


def _prep_in_maps(inputs):
    bf = ml_dtypes.bfloat16
    f8 = ml_dtypes.float8_e4m3
    x = np.asarray(inputs["x"])
    emb = np.asarray(inputs["emb"], np.float32)
    pos = np.asarray(inputs["pos"], np.float32)
    g_e = np.asarray(inputs["ln_e_g"], np.float32)
    b_e = np.asarray(inputs["ln_e_b"], np.float32)

    def parr(w):
        w = np.ascontiguousarray(
            np.asarray(w, np.float32).reshape(C6, 128, -1)
            .transpose(1, 0, 2)).astype(bf)
        return w.reshape(128, -1)

    def parr8(w, scale):
        w = np.clip(np.asarray(w, np.float32) * scale, -240.0, 240.0)
        w = np.ascontiguousarray(
            w.reshape(-1, 128, w.shape[-1]).transpose(1, 0, 2)).astype(f8)
        return w.reshape(128, -1)

    wts = {
        "lfwq": parr8(inputs["lf_wq"], SC_LFW),
        "lfwk": parr8(inputs["lf_wk"], SC_LFW),
        "lfwv": parr8(inputs["lf_wv"], SC_LFW),
        "lfwo": parr8(inputs["lf_wo"], SC_LFW),
    }
    w1 = np.asarray(inputs["w1"], np.float32)
    wts["w1"] = np.ascontiguousarray(
        w1.reshape(C6, 128, KC, 128).transpose(1, 2, 0, 3)
    ).astype(bf).reshape(128, KC * D)
    wts["w2"] = np.asarray(inputs["w2"], np.float32).astype(bf)

    # folded stage-B matrices, fp8 x256
    wq = np.asarray(inputs["mha_wq"], np.float32).reshape(D, NH, DK)
    wk = np.asarray(inputs["mha_wk"], np.float32).reshape(D, NH, DK)
    wv = np.asarray(inputs["mha_wv"], np.float32).reshape(D, NH, DK)
    fc = np.asarray(inputs["mha_fc"], np.float32).reshape(NH, DK, D)
    mwA = np.concatenate(
        [parr8(wq[:, h, :] @ wk[:, h, :].T, SC_W) for h in range(NH)], axis=1)
    Bcat = np.concatenate([wv[:, h, :] @ fc[h] for h in range(NH)], axis=0)
    wts["mwA"] = np.ascontiguousarray(mwA)
    wts["mwB"] = np.ascontiguousarray(parr8(Bcat, SC_W))

    in_maps = []
    for b in range(B):
        h0 = emb[x[b]] + pos                        # [S, D] f32
        mu = h0.mean(-1, keepdims=True)
        var = h0.var(-1, keepdims=True)
        hn = (h0 - mu) / np.sqrt(var + EPS) * g_e + b_e
        kbias = np.where(x[b] != 0, 0.0, NEG).astype(np.float32)
        kb16 = np.ascontiguousarray(kbias.reshape(16, 128).T)
        for p in range(2):
            start = p * T - 256
            hxe = np.zeros((EXT, D), np.float32)
            lo, hi = max(0, start), min(S, start + EXT)
            hxe[lo - start: hi - start] = hn[lo:hi]
            hxf = np.ascontiguousarray(
                hxe.reshape(EXT, C6, 128).transpose(2, 1, 0))
            hxT = hxf.astype(bf).reshape(128, C6 * EXT)
            hx8 = np.clip(hxf * SC_HX, -240.0, 240.0).astype(f8).reshape(
                128, C6 * EXT)

            qi = np.arange(128)
            kj = np.arange(640)
            m640 = np.zeros((8, 128, 640), np.float32)
            for qt in range(8):
                qg = p * T + qt * 128 + qi[:, None]
                kg = start + qt * 128 + kj[None, :]
                ok = (np.abs(kg - qg) <= W) & (kg >= 0) & (kg < S)
                # k-major: [key-in-chunk, dx-chunk, query]
                m640[qt] = np.ascontiguousarray(
                    np.where(ok, 0.0, NEG).T.reshape(5, 128, 128)
                    .transpose(1, 0, 2)).reshape(128, 640)

            m = {"hxT": hxT, "hx8": hx8, "m640": m640, "kb16": kb16}
            m.update(wts)
            in_maps.append(m)
    return in_maps


def _postprocess(results):
    out = np.zeros((B, D), np.float32)
    for b in range(B):
        m0 = np.asarray(results[2 * b]["out"]).T.reshape(D)
        m1 = np.asarray(results[2 * b + 1]["out"]).T.reshape(D)
        out[b] = np.maximum(m0, m1)
    return out


def run(inputs, debug=False, trace=False):
    nc = _get_nc(debug=debug)
    in_maps = _prep_in_maps(inputs)
    res = run_bass_kernel_spmd(nc, in_maps, core_ids=list(range(NCORES)),
                               trace=trace)
    return res


def kernel(**inputs):
    res = run(inputs, debug=False, trace=False)
    return _postprocess(res.results)
